# revision 35
# baseline (speedup 1.0000x reference)
"""Trainium2 Bass kernel for a pre-LN transformer block (B=4, T=2048, D=1024,
H=16, HS=64, FF=4096, causal attention).

Sharding: data-parallel over batches x 2-way tensor-parallel attention
(8 heads/core over all T) -> pair ReduceScatter of the attention-output
projection over the sequence dim -> sequence-parallel FFN (full FF width,
T/2 rows per core).  No AllReduce anywhere; each core emits the final
output for its own T/2 rows.

Core c (0..7): batch b = c//2, half = c%2.  half h owns t-slices
[ci*512 + h*256, ci*512 + h*256 + 256) for ci in 0..3.

v2 schedule: attention chunks run 0,1,3,2 so that all four ReduceScatters
except the last are issued mid-attention, and the FFN for the first half
of rows (prep+fc1) is interleaved into the final attention chunk as
pending units -- the tail RS and LN2 hide behind fc1/fc2 matmuls.
Attention-path tensors (weights, hT, kT, qT, oT) are fp8e4m3 with
power-of-2 scales folded into the projection epilogues; this halves
their SBUF/DMA cost (matmuls run at bf16 rate).  LN1 stats come from
M=1 PE matmuls instead of DVE adder trees.  Diagonal score tiles are
N-trimmed and their exp is windowed, with the causal mask applied only
to the [128,128] band.  W1/W2 stream through double-buffered tiles.
"""

import numpy as np
import ml_dtypes

import concourse.bacc as bacc
import concourse.bass as bass
import concourse.mybir as mybir
import concourse.tile as tile
from concourse.bass_utils import run_bass_kernel_spmd

BF16NP = ml_dtypes.bfloat16
FP8NP = ml_dtypes.float8_e4m3

B, T, D, H, HS, FF = 4, 2048, 1024, 16, 64, 4096
EPS = 1e-5
NCORES = 8
TP = 2
LH = H // TP          # 8 local heads
LHE = LH * HS         # 512 local head-embed width
LT = T // TP          # 1024 local rows (FFN/output)
KD = D // 128         # 8 d k-tiles
KHE = LHE // 128      # 4 he k-tiles
KFF = FF // 128       # 32 ff tiles
NCH = T // 512        # 4 t-chunks of 512
NST = T // 128        # 16 s-tiles of 128
PAIRS = [[0, 1], [2, 3], [4, 5], [6, 7]]
OA_LAG = 2            # psc tiles in flight between scores and o-accum

F32 = mybir.dt.float32
BF = mybir.dt.bfloat16
F8 = mybir.dt.float8e4
F85 = mybir.dt.float8e5
DRM = mybir.MatmulPerfMode.DoubleRow
# fp8 weight scales (power-of-2, folded out in the epilogues)
SQ = 256.0   # wq carries g1 and HS^-0.5 -> sigma 1/256
SK = 32.0
SV = 32.0
SO = 32.0


def _ln_math(nc, pool, ps_s, ps_q):
    """From psum row-sums (ps_s, ps_q over D) to bf16 broadcast tiles
    (Ab, Bb) so that xn = x*Ab + Bb."""
    Alu = mybir.AluOpType
    Act = mybir.ActivationFunctionType
    m = pool.tile([1, 512], F32, tag="ln_m", bufs=1, name="ln_m")
    e2 = pool.tile([1, 512], F32, tag="ln_e2", bufs=1, name="ln_e2")
    nc.vector.tensor_scalar_mul(out=m, in0=ps_s, scalar1=1.0 / D)
    nc.vector.tensor_scalar_mul(out=e2, in0=ps_q, scalar1=1.0 / D)
    msq = pool.tile([1, 512], F32, tag="ln_msq", bufs=1, name="ln_msq")
    nc.vector.tensor_mul(out=msq, in0=m, in1=m)
    var = pool.tile([1, 512], F32, tag="ln_var", bufs=1, name="ln_var")
    nc.vector.scalar_tensor_tensor(out=var, in0=e2, scalar=EPS, in1=msq,
                                   op0=Alu.add, op1=Alu.subtract)
    sd = pool.tile([1, 512], F32, tag="ln_sd", bufs=1, name="ln_sd")
    nc.scalar.activation(out=sd, in_=var, func=Act.Sqrt)
    a_row = pool.tile([1, 512], F32, tag="ln_a", bufs=1, name="ln_a")
    nc.vector.reciprocal_approx_fast(out=a_row, in_=sd)
    b_row = pool.tile([1, 512], F32, tag="ln_b", bufs=1, name="ln_b")
    nc.vector.scalar_tensor_tensor(out=b_row, in0=m, scalar=-1.0, in1=a_row,
                                   op0=Alu.mult, op1=Alu.mult)
    ac = pool.tile([1, 512], BF, tag="ln_ac", bufs=1, name="ln_ac")
    bc = pool.tile([1, 512], BF, tag="ln_bc", bufs=1, name="ln_bc")
    nc.vector.tensor_copy(out=ac, in_=a_row)
    nc.vector.tensor_copy(out=bc, in_=b_row)
    Ab = pool.tile([128, 512], BF, tag="ln_Ab", bufs=2, name="ln_Ab")
    Bb = pool.tile([128, 512], BF, tag="ln_Bb", bufs=2, name="ln_Bb")
    nc.gpsimd.partition_broadcast(Ab, ac)
    nc.gpsimd.partition_broadcast(Bb, bc)
    return Ab, Bb


def _ln_stats_pe(nc, pool, psum_pool, psum_tag, src, ones_col):
    """LN stats via M=1 PE matmuls: sum(x) directly on the x k-tiles,
    sum(x^2) on DVE-squared tiles.  src: [128, KD, 512] bf16."""
    mm = nc.tensor.matmul
    ps_s = psum_pool.tile([1, 512], F32, tag=psum_tag, bufs=2, name="ps_s")
    ps_q = psum_pool.tile([1, 512], F32, tag=psum_tag, bufs=2, name="ps_q")
    for k in range(KD):
        mm(out=ps_s, lhsT=ones_col, rhs=src[:, k, :],
           start=(k == 0), stop=(k == KD - 1))
    for k in range(KD):
        sq = pool.tile([128, 512], BF, tag="ln_sq", bufs=2, name="ln_sq")
        nc.vector.tensor_mul(out=sq, in0=src[:, k, :], in1=src[:, k, :])
        mm(out=ps_q, lhsT=ones_col, rhs=sq,
           start=(k == 0), stop=(k == KD - 1))
    return _ln_math(nc, pool, ps_s, ps_q)


def _ln_stats_tree(nc, pool, psum_pool, psum_tag, src, ones_col):
    """DVE adder-tree LN stats (kept for LN2 where the PE is contended).
    src: [128, KD, 512] AP."""
    mm = nc.tensor.matmul

    def lvl(tg, n):
        return pool.tile([128, 512], BF, tag=f"{tg}{n}", bufs=2, name=tg)

    s2, q2 = [], []
    for i in range(4):
        s = lvl("lts", 2)
        nc.vector.tensor_add(out=s, in0=src[:, 2 * i, :],
                             in1=src[:, 2 * i + 1, :])
        s2.append(s)
        sqa = pool.tile([128, 512], BF, tag="ln_sq", bufs=2, name="ln_sq")
        sqb = pool.tile([128, 512], BF, tag="ln_sq", bufs=2, name="ln_sq")
        nc.vector.tensor_mul(out=sqa, in0=src[:, 2 * i, :],
                             in1=src[:, 2 * i, :])
        nc.vector.tensor_mul(out=sqb, in0=src[:, 2 * i + 1, :],
                             in1=src[:, 2 * i + 1, :])
        q = lvl("ltq", 2)
        nc.vector.tensor_add(out=q, in0=sqa, in1=sqb)
        q2.append(q)
    s4, q4_ = [], []
    for i in range(2):
        s = lvl("lts", 4)
        nc.vector.tensor_add(out=s, in0=s2[2 * i], in1=s2[2 * i + 1])
        s4.append(s)
        q = lvl("ltq", 4)
        nc.vector.tensor_add(out=q, in0=q2[2 * i], in1=q2[2 * i + 1])
        q4_.append(q)
    s_all = lvl("lts", 8)
    nc.vector.tensor_add(out=s_all, in0=s4[0], in1=s4[1])
    q_all = lvl("ltq", 8)
    nc.vector.tensor_add(out=q_all, in0=q4_[0], in1=q4_[1])

    ps_s = psum_pool.tile([1, 512], F32, tag=psum_tag, bufs=2, name="ps_s")
    ps_q = psum_pool.tile([1, 512], F32, tag=psum_tag, bufs=2, name="ps_q")
    mm(out=ps_s, lhsT=ones_col, rhs=s_all, start=True, stop=True)
    mm(out=ps_q, lhsT=ones_col, rhs=q_all, start=True, stop=True)
    return _ln_math(nc, pool, ps_s, ps_q)


def _ln_apply(nc, pool, src_k, Ab, Bb, out_slice, eng=None, tag="ln_t1"):
    """out = src*Ab + Bb."""
    eng = eng or nc.vector
    t1 = pool.tile([128, 512], BF, tag=tag, bufs=2, name="ln_t1")
    eng.tensor_mul(out=t1, in0=src_k, in1=Ab)
    eng.tensor_add(out=out_slice, in0=t1, in1=Bb)


def _emit(nc, tc, t):
    mm = nc.tensor.matmul
    Alu = mybir.AluOpType
    Act = mybir.ActivationFunctionType

    outT_v = t["outT"]

    # ---------------- persistent pools ----------------
    dram = tc.alloc_tile_pool(name="dram", bufs=1, space="DRAM")
    rs_in = [dram.tile([TP, 128, KD, 256], BF, name=f"rsi{c}")
             for c in range(NCH)]
    rs_out = [dram.tile([128, KD, 256], BF, name=f"rso{c}")
              for c in range(NCH)]

    consts = tc.alloc_tile_pool(name="consts", bufs=1)
    ones_col = consts.tile([128, 1], BF)
    nc.vector.memset(ones_col, 1.0)

    bq_sb = consts.tile([128, KHE], F32)
    bk_sb = consts.tile([128, KHE], F32)
    bo2_sb = consts.tile([128, KD], F32)
    b2_sb = consts.tile([128, KD], F32)
    b1_sb = consts.tile([128, KFF], F32)
    for name, dst in (("bq", bq_sb), ("bk", bk_sb),
                      ("bo2", bo2_sb), ("b2", b2_sb)):
        nc.sync.dma_start(out=dst, in_=t[name].rearrange("(k p) -> p k", p=128))
    nc.sync.dma_start(out=b1_sb, in_=t["b1f"].rearrange("(k p) -> p k", p=128))
    # v bias broadcast over all partitions: [128, LHE]
    bvb = consts.tile([128, LHE], BF)
    bv_row = consts.tile([1, LHE], BF)
    nc.gpsimd.dma_start(out=bv_row,
                        in_=t["bv"].rearrange("(o e) -> o e", o=1))
    nc.gpsimd.partition_broadcast(bvb, bv_row)

    wlate = tc.alloc_tile_pool(name="wlate", bufs=1)
    wo_sb = wlate.tile([128, KHE, D], F8, tag="wo")
    nc.scalar.dma_start(out=wo_sb, in_=t["wo"])

    # single PSUM pool: ps_proj(2) + ps_sc(2x2) + po(2) = 8 banks
    pps = tc.alloc_tile_pool(name="pps", bufs=2, space="PSUM")

    # stage A long-lived pool
    ab = tc.alloc_tile_pool(name="abc", bufs=1)
    kT = ab.tile([128, LH // 2, T], F8, tag="kT")
    qT = ab.tile([128, LH // 2, T], F8, tag="qT")
    vS = ab.tile([128, NST, LH * 128], F8, tag="vS")
    # early pool: x tiles, hT, qkv weights, LN1 temps (freed before FFN)
    pe_pool = tc.alloc_tile_pool(name="pearly", bufs=1)

    wq_sb = pe_pool.tile([128, KD, LHE], F8, tag="wq")
    wk_sb = pe_pool.tile([128, KD, LHE], F8, tag="wk")
    wv_sb = pe_pool.tile([128, KD, LHE], F8, tag="wv")

    def load_x(ci):
        xf = pe_pool.tile([128, KD, 512], BF, tag="xf", bufs=2, name="xf")
        engs = (nc.sync, nc.scalar, nc.gpsimd)
        for k in range(KD):
            engs[k % 3].dma_start(out=xf[:, k:k + 1, :],
                                  in_=t["xT"][ci * 128:ci * 128 + 128,
                                              k:k + 1, :])
        return xf

    def load_qkv_weights():
        for eng, srct, dst in ((nc.scalar, t["wq"], wq_sb),
                               (nc.scalar, t["wk"], wk_sb),
                               (nc.sync, t["wv"], wv_sb)):
            eng.dma_start(out=dst, in_=srct)

    def ln1(ci, xf, gps=False):
        Ab, Bb = _ln_stats_pe(nc, pe_pool, pps, "ps_proj", xf, ones_col)
        hT = pe_pool.tile([128, KD, 512], F8, tag="hT", bufs=2, name="hT")
        for k in range(KD):
            if gps and k % 2 == 1:
                _ln_apply(nc, pe_pool, xf[:, k, :], Ab, Bb, hT[:, k, :],
                          eng=nc.gpsimd, tag="ln_t1g")
            else:
                _ln_apply(nc, pe_pool, xf[:, k, :], Ab, Bb, hT[:, k, :])
        return hT

    def proj_units(ci, hT):
        c0 = ci * 512
        units = []
        for w_sb, dst, bias, rsc in ((wk_sb, kT, bk_sb, 1.0 / SK),
                                     (wq_sb, qT, bq_sb, 1.0 / SQ)):
            for et in range(LH // 2):
                def u(w_sb=w_sb, dst=dst, bias=bias, rsc=rsc, et=et,
                      hT=hT, c0=c0):
                    ps = pps.tile([128, 512], F32, tag="ps_proj",
                                  bufs=2, name="ps_proj")
                    for kp in range(KD // 2):
                        mm(out=ps,
                           lhsT=w_sb[:, 2 * kp:2 * kp + 2,
                                     et * 128:(et + 1) * 128],
                           rhs=hT[:, 2 * kp:2 * kp + 2, :],
                           start=(kp == 0), stop=(kp == KD // 2 - 1),
                           perf_mode=DRM)
                    nc.vector.tensor_scalar(
                        out=dst[:, et, c0:c0 + 512], in0=ps,
                        scalar1=bias[:, et:et + 1], scalar2=rsc,
                        op0=Alu.add, op1=Alu.mult)
                units.append(u)
        for sti in range(4):
            st = ci * 4 + sti
            def u(sti=sti, st=st, hT=hT):
                ps = pps.tile([128, LHE], F32, tag="ps_proj", bufs=2,
                              name="ps_v")
                for kp in range(KD // 2):
                    mm(out=ps,
                       lhsT=hT[:, 2 * kp:2 * kp + 2,
                               sti * 128:sti * 128 + 128],
                       rhs=wv_sb[:, 2 * kp:2 * kp + 2, :],
                       start=(kp == 0), stop=(kp == KD // 2 - 1),
                       perf_mode=DRM)
                nc.vector.scalar_tensor_tensor(
                    out=vS[:, st, :].rearrange("p (h e) -> p h e",
                                               h=LH)[:, :, 0:64],
                    in0=ps.rearrange("p (h e) -> p h e", e=64),
                    scalar=1.0 / SV, op0=Alu.mult,
                    in1=bvb.rearrange("p (h e) -> p h e", e=64),
                    op1=Alu.add)
            units.append(u)
        return units

    def attention(ci, oT, pending):
        c0 = ci * 512
        nsp = 2 * (ci + 1)  # pairs of 128-key s-tiles
        total_steps = (LH // 2) * (nsp + OA_LAG)
        spacing = (max(1, total_steps // len(pending))
                   if pending else 0)
        stepctr = 0
        for hp in range(LH // 2):
            po = [pps.tile([128, 512], F32, tag="po", bufs=2, name="po")
                  for _ in range(2)]
            exs = [None] * nsp

            def scores(spi):
                midx = 2 * spi - 4 * ci
                w0 = max(0, midx) * 128
                # [128, j(2 s-tiles), hi(2), 512] -- 4 banks, 1 buf
                ps4 = pps.tile([128, 2, 2, 512], F32, tag="ps_sc",
                               bufs=1, name="ps_sc")
                for j in range(2):
                    wj = w0 + j * 128 if midx >= 0 else 0
                    s0 = (2 * spi + j) * 128
                    for hi in range(2):
                        mm(out=ps4[:, j, hi, wj:512],
                           lhsT=kT[hi * 64:hi * 64 + 64, hp, s0:s0 + 128],
                           rhs=qT[hi * 64:hi * 64 + 64, hp,
                                  c0 + wj:c0 + 512],
                           start=True, stop=True)
                pair = []
                for hi in range(2):
                    ex = ab.tile([128, 2, 512], F85, tag=f"ex{hi}",
                                 bufs=OA_LAG + 1, name="ex")
                    nc.scalar.activation(out=ex[:, :, w0:512],
                                         in_=ps4[:, :, hi, w0:512],
                                         func=Act.Exp)
                    if midx >= 0:
                        for j in range(2):
                            wj = w0 + j * 128
                            if wj > 0:
                                nc.gpsimd.memset(ex[:, j, 0:wj], 0.0)
                            nc.gpsimd.affine_select(
                                out=ex[:, j, wj:wj + 128],
                                in_=ex[:, j, wj:wj + 128],
                                compare_op=Alu.is_ge, fill=0.0,
                                base=0, channel_multiplier=-1,
                                pattern=[[1, 128]])
                    pair.append(ex)
                exs[spi] = pair

            def oacc(spi):
                for hi in range(2):
                    h_loc = hp * 2 + hi
                    mm(out=po[hi],
                       lhsT=vS[:, 2 * spi:2 * spi + 2,
                               h_loc * 128:h_loc * 128 + 128],
                       rhs=exs[spi][hi],
                       start=(spi == 0), stop=(spi == nsp - 1),
                       perf_mode=DRM)

            for step in range(nsp + OA_LAG):
                if step < nsp:
                    scores(step)
                if step >= OA_LAG:
                    oacc(step - OA_LAG)
                stepctr += 1
                if pending and stepctr % spacing == 0:
                    pending.pop(0)()

            for hi in range(2):
                h_loc = hp * 2 + hi
                dnr = ab.tile([1, 512], F32, tag="dnr", bufs=1,
                              name="dnr")
                nc.vector.tensor_copy(out=dnr, in_=po[hi][64:65, :])
                rcp = ab.tile([1, 512], F32, tag="rcp", bufs=2,
                              name="rcp")
                nc.vector.reciprocal_approx_fast(out=rcp, in_=dnr)
                bc = ab.tile([64, 512], F32, tag="bc", bufs=1,
                             name="bc")
                nc.gpsimd.partition_broadcast(bc, rcp)
                nc.vector.tensor_mul(
                    out=oT[hi * 64:hi * 64 + 64, hp, :],
                    in0=po[hi][0:64, :], in1=bc)
        while pending:
            pending.pop(0)()

    def wo_units(ci, oT):
        rsv = rs_in[ci]
        units = []
        for dt in range(KD):
            def u(dt=dt, oT=oT, rsv=rsv):
                ps = pps.tile([128, 512], F32, tag="ps_proj",
                              bufs=2, name="ps_wo")
                for kp in range(KHE // 2):
                    mm(out=ps,
                       lhsT=wo_sb[:, 2 * kp:2 * kp + 2,
                                  dt * 128:(dt + 1) * 128],
                       rhs=oT[:, 2 * kp:2 * kp + 2, :],
                       start=(kp == 0), stop=(kp == KHE // 2 - 1),
                       perf_mode=DRM)
                stg = ab.tile([128, 512], BF, tag="stg1", bufs=2,
                              name="stg1")
                nc.vector.tensor_scalar(
                    out=stg, in0=ps, scalar1=bo2_sb[:, dt:dt + 1],
                    scalar2=1.0 / SO, op0=Alu.add, op1=Alu.mult)
                for j in range(TP):
                    nc.sync.dma_start(
                        out=rsv[j, :, dt, :],
                        in_=stg[:, j * 256:(j + 1) * 256])
            units.append(u)
        return units

    def rs_unit(ci):
        def u():
            nc.gpsimd.collective_compute(
                "ReduceScatter", Alu.add, replica_groups=PAIRS,
                ins=[rs_in[ci].opt()], outs=[rs_out[ci].opt()])
        return u

    # ---------------- stage B (FFN) units (pool allocated later) --------
    state = {}

    def prep_units(lc):
        """residual + LN2 + apply, decomposed into pending units."""
        de = state["de"]
        units = []

        def u_load():
            xmid = de.tile([128, KD, 512], BF, tag="xmid", bufs=2,
                           name="xmid")
            xrs = t["xresT"][lc * 128:lc * 128 + 128, :, :]
            nc.sync.dma_start(out=xmid[:, 0:4, :], in_=xrs[:, 0:4, :])
            nc.scalar.dma_start(out=xmid[:, 4:8, :], in_=xrs[:, 4:8, :])
            arr = de.tile([128, KD, 2, 256], BF, tag="arr", bufs=1,
                          name="arr")
            nc.sync.dma_start(out=arr[:, 0:4, 0, :],
                              in_=rs_out[2 * lc][:, 0:4, :])
            nc.scalar.dma_start(out=arr[:, 4:8, 0, :],
                                in_=rs_out[2 * lc][:, 4:8, :])
            nc.sync.dma_start(out=arr[:, 0:4, 1, :],
                              in_=rs_out[2 * lc + 1][:, 0:4, :])
            nc.scalar.dma_start(out=arr[:, 4:8, 1, :],
                                in_=rs_out[2 * lc + 1][:, 4:8, :])
            state[f"xmid{lc}"] = xmid
            state[f"arr{lc}"] = arr
        units.append(u_load)

        def u_add():
            xmid = state[f"xmid{lc}"]
            arr = state[f"arr{lc}"]
            for k in range(KD):
                nc.vector.tensor_add(
                    out=xmid[:, k, :], in0=xmid[:, k, :],
                    in1=arr[:, k, :, :].rearrange("p j t -> p (j t)"))
        units.append(u_add)

        def u_stats():
            xmid = state[f"xmid{lc}"]
            state[f"ab{lc}"] = _ln_stats_pe(nc, de, pps, "ps_proj",
                                            xmid, ones_col)
        units.append(u_stats)

        def mk_apply(k0):
            def u_apply():
                xmid = state[f"xmid{lc}"]
                Ab2, Bb2 = state[f"ab{lc}"]
                if f"h2{lc}" not in state:
                    state[f"h2{lc}"] = de.tile([128, KD, 512], BF,
                                               tag="h2", bufs=1, name="h2")
                h2 = state[f"h2{lc}"]
                for k in range(k0, k0 + 4):
                    _ln_apply(nc, de, xmid[:, k, :], Ab2, Bb2, h2[:, k, :])
            return u_apply
        units.append(mk_apply(0))
        units.append(mk_apply(4))
        return units

    def fc1_units(lc):
        """FFN up: u = relu(h2 @ W1 + b1f); one unit per 256-wide block."""
        de = state["de"]
        units = []

        def mk(q16):
            def u():
                h2 = state[f"h2{lc}"]
                if f"u{lc}" not in state:
                    state[f"u{lc}"] = de.tile([128, KFF, 512], BF,
                                              tag="u", bufs=1, name="u")
                uu = state[f"u{lc}"]
                w1t = de.tile([128, KD, 256], BF, tag="w1t", bufs=2,
                              name="w1t")
                nc.sync.dma_start(
                    out=w1t, in_=t["w1"][q16 * 128:(q16 + 1) * 128, :, :])
                for fi in range(2):
                    fft = q16 * 2 + fi
                    ps = pps.tile([128, 512], F32, tag="ps_proj", bufs=2,
                                  name="ps_u")
                    for k in range(KD):
                        mm(out=ps,
                           lhsT=w1t[:, k, fi * 128:fi * 128 + 128],
                           rhs=h2[:, k, :],
                           start=(k == 0), stop=(k == KD - 1))
                    if lc == 0:
                        nc.vector.tensor_scalar(
                            out=uu[:, fft, :], in0=ps,
                            scalar1=b1_sb[:, fft:fft + 1], scalar2=0.0,
                            op0=Alu.add, op1=Alu.max)
                    else:
                        nc.scalar.activation(
                            out=uu[:, fft, :], in_=ps, func=Act.Relu,
                            bias=b1_sb[:, fft:fft + 1])
            units.append(u)
        for q16 in range(16):
            mk(q16)
        return units

    def fc2_units(lc):
        """FFN down + bias + residual -> store; W2 streamed per dt."""
        de = state["de"]
        c0 = lc * 512
        units = []

        def mk(dt):
            def u():
                uu = state[f"u{lc}"]
                xmid = state[f"xmid{lc}"]
                w2t = state.pop(f"w2t{dt}", None) if lc == 0 else None
                if w2t is None:
                    w2t = de.tile([128, KFF, 128], BF, tag="w2t", bufs=2,
                                  name="w2t")
                    nc.scalar.dma_start(
                        out=w2t,
                        in_=t["w2"][dt * 128:(dt + 1) * 128, :, :])
                ps = pps.tile([128, 512], F32, tag="ps_proj", bufs=2,
                              name="ps_f")
                for k2 in range(KFF):
                    mm(out=ps, lhsT=w2t[:, k2, :], rhs=uu[:, k2, :],
                       start=(k2 == 0), stop=(k2 == KFF - 1))
                o_f = de.tile([128, 512], F32, tag="o_f", bufs=2,
                              name="o_f")
                nc.vector.scalar_tensor_tensor(
                    out=o_f, in0=ps, scalar=b2_sb[:, dt:dt + 1],
                    in1=xmid[:, dt, :], op0=Alu.add, op1=Alu.add)
                nc.sync.dma_start(
                    out=outT_v[dt * 128:(dt + 1) * 128, c0:c0 + 512],
                    in_=o_f)
            units.append(u)
        for dt in range(KD):
            mk(dt)
        return units

    # ================= emission schedule =================
    # vS zero + softmax-denominator ones columns
    nc.vector.memset(vS[:, 0:8, :], 0.0)
    nc.gpsimd.memset(vS[:, 8:16, :], 0.0)
    for h in range(LH):
        nc.vector.memset(vS[:, :, h * 128 + 64:h * 128 + 65], 1.0)

    xf0 = load_x(0)
    load_qkv_weights()
    xf1 = load_x(1)

    def w2_prefetch(dt):
        def u():
            de = state["de"]
            w2t = de.tile([128, KFF, 128], BF, tag="w2t", bufs=2,
                          name="w2t")
            nc.scalar.dma_start(
                out=w2t, in_=t["w2"][dt * 128:(dt + 1) * 128, :, :])
            state[f"w2t{dt}"] = w2t
        return u

    hT0 = ln1(0, xf0, gps=True)
    for u in proj_units(0, hT0):
        u()
    hT1 = ln1(1, xf1, gps=True)

    oTs = {}
    for ci in range(NCH):
        oTs[ci] = None

    def new_oT():
        return ab.tile([128, KHE, 512], F8, tag="oT", bufs=2, name="oT")

    oTs[0] = new_oT()
    attention(0, oTs[0], list(proj_units(1, hT1)))

    xf2 = load_x(2)
    hT2 = ln1(2, xf2)
    xf3 = load_x(3)
    hT3 = ln1(3, xf3)

    oTs[1] = new_oT()
    attention(1, oTs[1],
              wo_units(0, oTs[0]) + [rs_unit(0)]
              + proj_units(2, hT2) + proj_units(3, hT3))
    pe_pool.release()
    state["de"] = tc.alloc_tile_pool(name="de", bufs=1)

    oTs[3] = new_oT()
    attention(3, oTs[3], wo_units(1, oTs[1]) + [rs_unit(1)])

    oTs[2] = new_oT()
    attention(2, oTs[2],
              wo_units(3, oTs[3]) + [rs_unit(3)]
              + prep_units(0) + [w2_prefetch(0), w2_prefetch(1)]
              + fc1_units(0))

    for u in wo_units(2, oTs[2]):
        u()
    rs_unit(2)()

    # interleave prep(1) with fc2(0), then fc1(1), fc2(1)
    p1 = prep_units(1)
    f20 = fc2_units(0)
    inter = []
    while p1 or f20:
        if f20:
            inter.append(f20.pop(0))
        if p1:
            inter.append(p1.pop(0))
    for u in inter:
        u()
    for u in fc1_units(1):
        u()
    for u in fc2_units(1):
        u()

    state["de"].release()
    ab.release()
    pps.release()
    wlate.release()
    consts.release()
    dram.release()


def _build():
    nc = bacc.Bacc("TRN2", target_bir_lowering=False, debug=False,
                   num_devices=NCORES)

    tensors = {}
    tensors["xT"] = nc.dram_tensor("xT", [NCH * 128, KD, 512], BF,
                                   kind="ExternalInput").ap()
    tensors["xresT"] = nc.dram_tensor("xresT", [TP * 128, KD, 512], BF,
                                      kind="ExternalInput").ap()
    for name, shape, dt in (
        ("wq", [128, KD, 512], F8), ("wk", [128, KD, 512], F8),
        ("wv", [128, KD, 512], F8), ("wo", [128, KHE, D], F8),
        ("w1", [16 * 128, KD, 256], BF), ("w2", [8 * 128, KFF, 128], BF),
        ("bq", [LHE], F32), ("bk", [LHE], F32), ("bv", [LHE], F32),
        ("b1f", [FF], F32), ("bo2", [D], F32), ("b2", [D], F32),
    ):
        tensors[name] = nc.dram_tensor(name, shape, dt,
                                       kind="ExternalInput").ap()
    tensors["outT"] = nc.dram_tensor("out", [D, LT], F32,
                                     kind="ExternalOutput").ap()

    with tile.TileContext(nc, num_cores=NCORES) as tc:
        _emit(nc, tc, tensors)

    nc.compile()
    return nc


_NC_CACHE = None


def _get_nc():
    global _NC_CACHE
    if _NC_CACHE is None:
        _NC_CACHE = _build()
    return _NC_CACHE


def _shard_inputs(x, Wq, Wk, Wv, Wo, bo, W1, b1, W2, b2, g1, be1, g2, be2):
    """Build the 8 per-core input maps (LN gains folded into weights)."""
    bf = lambda a: np.ascontiguousarray(a).astype(BF16NP)
    f8 = lambda a: np.ascontiguousarray(a).astype(FP8NP)
    f32 = lambda a: np.ascontiguousarray(a, dtype=np.float32)

    x = np.asarray(x, dtype=np.float32)
    Wq = np.asarray(Wq, dtype=np.float32)
    Wk = np.asarray(Wk, dtype=np.float32)
    Wv = np.asarray(Wv, dtype=np.float32)
    Wo = np.asarray(Wo, dtype=np.float32)
    W1 = np.asarray(W1, dtype=np.float32)
    W2 = np.asarray(W2, dtype=np.float32)
    g1 = np.asarray(g1, dtype=np.float32)
    be1 = np.asarray(be1, dtype=np.float32)
    g2 = np.asarray(g2, dtype=np.float32)
    be2 = np.asarray(be2, dtype=np.float32)
    b1 = np.asarray(b1, dtype=np.float32)

    scale = float(HS) ** -0.5
    # fold g1 into QKV weights, be1 into QKV biases; fold the score scale
    # into Wq/bq.  Per-head [H, D, HS] -> concat heads -> [D, H*HS].
    wq_f = (g1[None, :, None] * Wq).transpose(1, 0, 2).reshape(D, D) * scale
    wk_f = (g1[None, :, None] * Wk).transpose(1, 0, 2).reshape(D, D)
    wv_f = (g1[None, :, None] * Wv).transpose(1, 0, 2).reshape(D, D)
    bq_f = np.einsum("d,hde->he", be1, Wq).reshape(D) * scale
    bk_f = np.einsum("d,hde->he", be1, Wk).reshape(D)
    bv_f = np.einsum("d,hde->he", be1, Wv).reshape(D)
    # fold g2/be2 into W1/b1
    w1_f = g2[:, None] * W1
    b1_f = b1 + be2 @ W1

    in_maps = []
    for c in range(NCORES):
        b, half = divmod(c, TP)
        hes = slice(half * LHE, (half + 1) * LHE)
        xt = x[b].T
        xres = np.concatenate(
            [xt[:, ci * 512 + half * 256: ci * 512 + half * 256 + 256]
             for ci in range(NCH)], axis=1)
        # partition-major tiled layouts: loads become 128 contiguous
        # segments instead of 1024 scattered ones (descriptor-gen bound)
        xt_sw = xt.reshape(KD, 128, NCH, 512).transpose(2, 1, 0, 3)
        xres_sw = xres.reshape(KD, 128, TP, 512).transpose(2, 1, 0, 3)
        wq_sw = wq_f[:, hes].reshape(KD, 128, LHE).transpose(1, 0, 2)
        wk_sw = wk_f[:, hes].reshape(KD, 128, LHE).transpose(1, 0, 2)
        wv_sw = wv_f[:, hes].reshape(KD, 128, LHE).transpose(1, 0, 2)
        wo_sw = Wo[hes, :].reshape(KHE, 128, D).transpose(1, 0, 2)
        w1_sw = w1_f.reshape(KD, 128, 16, 256).transpose(2, 1, 0, 3)
        # W2 tiled dt-major so fc2 streams contiguous [128, KFF, 128] tiles
        w2_sw = W2.reshape(KFF, 128, KD, 128).transpose(2, 1, 0, 3)
        in_maps.append({
            "xT": bf(xt_sw.reshape(NCH * 128, KD, 512)),
            "xresT": bf(xres_sw.reshape(TP * 128, KD, 512)),
            "wq": f8(wq_sw * SQ), "wk": f8(wk_sw * SK), "wv": f8(wv_sw * SV),
            "bq": f32(bq_f[hes] * SQ), "bk": f32(bk_f[hes] * SK),
            "bv": f32(bv_f[hes]),
            "wo": f8(wo_sw * SO),
            "bo2": f32(np.asarray(bo, dtype=np.float32) * SO / TP),
            "w1": bf(w1_sw.reshape(16 * 128, KD, 256)), "b1f": f32(b1_f),
            "w2": bf(w2_sw.reshape(8 * 128, KFF, 128)),
            "b2": f32(np.asarray(b2, dtype=np.float32)),
        })
    return in_maps


def kernel(x, Wq, Wk, Wv, Wo, bo, W1, b1, W2, b2, g1, be1, g2, be2,
           _trace=False):
    nc = _get_nc()
    in_maps = _shard_inputs(x, Wq, Wk, Wv, Wo, bo, W1, b1, W2, b2,
                            g1, be1, g2, be2)
    res = run_bass_kernel_spmd(nc, in_maps, list(range(NCORES)),
                               trace=_trace)
    out = np.empty((B, T, D), dtype=np.float32)
    for b in range(B):
        for half in range(TP):
            o = res.results[TP * b + half]["out"]  # [D, LT]
            for ci in range(NCH):
                t0 = ci * 512 + half * 256
                out[b, t0:t0 + 256, :] = o[:, ci * 256:(ci + 1) * 256].T
    if _trace:
        kernel.last_exec_time_ns = res.exec_time_ns
        kernel.last_results = res
    return out


# revision 37
# speedup vs baseline: 1.1090x; 1.1090x over previous
"""Trainium2 Bass kernel for a pre-LN transformer block (B=4, T=2048, D=1024,
H=16, HS=64, FF=4096, causal attention).

Sharding: data-parallel over batches x 2-way tensor-parallel attention
(8 heads/core over all T) -> pair ReduceScatter of the attention-output
projection over the sequence dim -> sequence-parallel FFN (full FF width,
T/2 rows per core).  No AllReduce anywhere; each core emits the final
output for its own T/2 rows.

Core c (0..7): batch b = c//2, half = c%2.  half h owns t-slices
[ci*512 + h*256, ci*512 + h*256 + 256) for ci in 0..3.

v2 schedule: attention chunks run 0,1,3,2 so that all four ReduceScatters
except the last are issued mid-attention, and the FFN for the first half
of rows (prep+fc1) is interleaved into the final attention chunk as
pending units -- the tail RS and LN2 hide behind fc1/fc2 matmuls.
Attention-path tensors (weights, hT, kT, qT, oT) are fp8e4m3 with
power-of-2 scales folded into the projection epilogues; this halves
their SBUF/DMA cost (matmuls run at bf16 rate).  LN1 stats come from
M=1 PE matmuls instead of DVE adder trees.  Diagonal score tiles are
N-trimmed and their exp is windowed, with the causal mask applied only
to the [128,128] band.  W1/W2 stream through double-buffered tiles.
"""

import numpy as np
import ml_dtypes

import concourse.bacc as bacc
import concourse.bass as bass
import concourse.mybir as mybir
import concourse.tile as tile
from concourse.bass_utils import run_bass_kernel_spmd

BF16NP = ml_dtypes.bfloat16
FP8NP = ml_dtypes.float8_e4m3

B, T, D, H, HS, FF = 4, 2048, 1024, 16, 64, 4096
EPS = 1e-5
NCORES = 8
TP = 2
LH = H // TP          # 8 local heads
LHE = LH * HS         # 512 local head-embed width
LT = T // TP          # 1024 local rows (FFN/output)
KD = D // 128         # 8 d k-tiles
KHE = LHE // 128      # 4 he k-tiles
KFF = FF // 128       # 32 ff tiles
NCH = T // 512        # 4 t-chunks of 512
NST = T // 128        # 16 s-tiles of 128
PAIRS = [[0, 1], [2, 3], [4, 5], [6, 7]]
OA_LAG = 2            # psc tiles in flight between scores and o-accum

F32 = mybir.dt.float32
BF = mybir.dt.bfloat16
F8 = mybir.dt.float8e4
F85 = mybir.dt.float8e5
DRM = mybir.MatmulPerfMode.DoubleRow
# fp8 weight scales (power-of-2, folded out in the epilogues)
SQ = 256.0   # wq carries g1 and HS^-0.5 -> sigma 1/256
SK = 32.0
SV = 32.0
SO = 32.0


def _ln_math(nc, pool, ps_s, ps_q):
    """From psum row-sums (ps_s, ps_q over D) to bf16 broadcast tiles
    (Ab, Bb) so that xn = x*Ab + Bb."""
    Alu = mybir.AluOpType
    Act = mybir.ActivationFunctionType
    m = pool.tile([1, 512], F32, tag="ln_m", bufs=1, name="ln_m")
    e2 = pool.tile([1, 512], F32, tag="ln_e2", bufs=1, name="ln_e2")
    nc.vector.tensor_scalar_mul(out=m, in0=ps_s, scalar1=1.0 / D)
    nc.vector.tensor_scalar_mul(out=e2, in0=ps_q, scalar1=1.0 / D)
    msq = pool.tile([1, 512], F32, tag="ln_msq", bufs=1, name="ln_msq")
    nc.vector.tensor_mul(out=msq, in0=m, in1=m)
    var = pool.tile([1, 512], F32, tag="ln_var", bufs=1, name="ln_var")
    nc.vector.scalar_tensor_tensor(out=var, in0=e2, scalar=EPS, in1=msq,
                                   op0=Alu.add, op1=Alu.subtract)
    sd = pool.tile([1, 512], F32, tag="ln_sd", bufs=1, name="ln_sd")
    nc.scalar.activation(out=sd, in_=var, func=Act.Sqrt)
    a_row = pool.tile([1, 512], F32, tag="ln_a", bufs=1, name="ln_a")
    nc.vector.reciprocal_approx_fast(out=a_row, in_=sd)
    b_row = pool.tile([1, 512], F32, tag="ln_b", bufs=1, name="ln_b")
    nc.vector.scalar_tensor_tensor(out=b_row, in0=m, scalar=-1.0, in1=a_row,
                                   op0=Alu.mult, op1=Alu.mult)
    ac = pool.tile([1, 512], BF, tag="ln_ac", bufs=1, name="ln_ac")
    bc = pool.tile([1, 512], BF, tag="ln_bc", bufs=1, name="ln_bc")
    nc.vector.tensor_copy(out=ac, in_=a_row)
    nc.vector.tensor_copy(out=bc, in_=b_row)
    Ab = pool.tile([128, 512], BF, tag="ln_Ab", bufs=2, name="ln_Ab")
    Bb = pool.tile([128, 512], BF, tag="ln_Bb", bufs=2, name="ln_Bb")
    nc.gpsimd.partition_broadcast(Ab, ac)
    nc.gpsimd.partition_broadcast(Bb, bc)
    return Ab, Bb


def _ln_stats_pe(nc, pool, psum_pool, psum_tag, src, ones_col):
    """LN stats via M=1 PE matmuls: sum(x) directly on the x k-tiles,
    sum(x^2) on DVE-squared tiles.  src: [128, KD, 512] bf16."""
    mm = nc.tensor.matmul
    ps_s = psum_pool.tile([1, 512], F32, tag=psum_tag, bufs=2, name="ps_s")
    ps_q = psum_pool.tile([1, 512], F32, tag=psum_tag, bufs=2, name="ps_q")
    for k in range(KD):
        mm(out=ps_s, lhsT=ones_col, rhs=src[:, k, :],
           start=(k == 0), stop=(k == KD - 1))
    for k in range(KD):
        sq = pool.tile([128, 512], BF, tag="ln_sq", bufs=2, name="ln_sq")
        nc.vector.tensor_mul(out=sq, in0=src[:, k, :], in1=src[:, k, :])
        mm(out=ps_q, lhsT=ones_col, rhs=sq,
           start=(k == 0), stop=(k == KD - 1))
    return _ln_math(nc, pool, ps_s, ps_q)


def _ln_stats_tree(nc, pool, psum_pool, psum_tag, src, ones_col):
    """DVE adder-tree LN stats (kept for LN2 where the PE is contended).
    src: [128, KD, 512] AP."""
    mm = nc.tensor.matmul

    def lvl(tg, n):
        return pool.tile([128, 512], BF, tag=f"{tg}{n}", bufs=2, name=tg)

    s2, q2 = [], []
    for i in range(4):
        s = lvl("lts", 2)
        nc.vector.tensor_add(out=s, in0=src[:, 2 * i, :],
                             in1=src[:, 2 * i + 1, :])
        s2.append(s)
        sqa = pool.tile([128, 512], BF, tag="ln_sq", bufs=2, name="ln_sq")
        sqb = pool.tile([128, 512], BF, tag="ln_sq", bufs=2, name="ln_sq")
        nc.vector.tensor_mul(out=sqa, in0=src[:, 2 * i, :],
                             in1=src[:, 2 * i, :])
        nc.vector.tensor_mul(out=sqb, in0=src[:, 2 * i + 1, :],
                             in1=src[:, 2 * i + 1, :])
        q = lvl("ltq", 2)
        nc.vector.tensor_add(out=q, in0=sqa, in1=sqb)
        q2.append(q)
    s4, q4_ = [], []
    for i in range(2):
        s = lvl("lts", 4)
        nc.vector.tensor_add(out=s, in0=s2[2 * i], in1=s2[2 * i + 1])
        s4.append(s)
        q = lvl("ltq", 4)
        nc.vector.tensor_add(out=q, in0=q2[2 * i], in1=q2[2 * i + 1])
        q4_.append(q)
    s_all = lvl("lts", 8)
    nc.vector.tensor_add(out=s_all, in0=s4[0], in1=s4[1])
    q_all = lvl("ltq", 8)
    nc.vector.tensor_add(out=q_all, in0=q4_[0], in1=q4_[1])

    ps_s = psum_pool.tile([1, 512], F32, tag=psum_tag, bufs=2, name="ps_s")
    ps_q = psum_pool.tile([1, 512], F32, tag=psum_tag, bufs=2, name="ps_q")
    mm(out=ps_s, lhsT=ones_col, rhs=s_all, start=True, stop=True)
    mm(out=ps_q, lhsT=ones_col, rhs=q_all, start=True, stop=True)
    return _ln_math(nc, pool, ps_s, ps_q)


def _ln_apply(nc, pool, src_k, Ab, Bb, out_slice, eng=None, tag="ln_t1"):
    """out = src*Ab + Bb."""
    eng = eng or nc.vector
    t1 = pool.tile([128, 512], BF, tag=tag, bufs=2, name="ln_t1")
    eng.tensor_mul(out=t1, in0=src_k, in1=Ab)
    eng.tensor_add(out=out_slice, in0=t1, in1=Bb)


def _emit(nc, tc, t):
    mm = nc.tensor.matmul
    Alu = mybir.AluOpType
    Act = mybir.ActivationFunctionType

    outT_v = t["outT"]

    # ---------------- persistent pools ----------------
    dram = tc.alloc_tile_pool(name="dram", bufs=1, space="DRAM")
    rs_in = [dram.tile([TP, 128, KD, 256], BF, name=f"rsi{c}")
             for c in range(NCH)]
    rs_out = [dram.tile([128, KD, 256], BF, name=f"rso{c}")
              for c in range(NCH)]

    consts = tc.alloc_tile_pool(name="consts", bufs=1)
    ones_col = consts.tile([128, 1], BF)
    nc.vector.memset(ones_col, 1.0)

    bq_sb = consts.tile([128, KHE], F32)
    bk_sb = consts.tile([128, KHE], F32)
    bo2_sb = consts.tile([128, KD], F32)
    b2_sb = consts.tile([128, KD], F32)
    b1_sb = consts.tile([128, KFF], F32)
    for name, dst in (("bq", bq_sb), ("bk", bk_sb),
                      ("bo2", bo2_sb), ("b2", b2_sb)):
        nc.sync.dma_start(out=dst, in_=t[name].rearrange("(k p) -> p k", p=128))
    nc.sync.dma_start(out=b1_sb, in_=t["b1f"].rearrange("(k p) -> p k", p=128))
    # v bias broadcast over all partitions: [128, LHE]
    bvb = consts.tile([128, LHE], BF)
    bv_row = consts.tile([1, LHE], BF)
    nc.gpsimd.dma_start(out=bv_row,
                        in_=t["bv"].rearrange("(o e) -> o e", o=1))
    nc.gpsimd.partition_broadcast(bvb, bv_row)

    wlate = tc.alloc_tile_pool(name="wlate", bufs=1)
    wo_sb = wlate.tile([128, KHE, D], F8, tag="wo")
    nc.scalar.dma_start(out=wo_sb, in_=t["wo"])

    # single PSUM pool: ps_proj(2) + ps_sc(2x2) + po(2) = 8 banks
    pps = tc.alloc_tile_pool(name="pps", bufs=2, space="PSUM")

    # stage A long-lived pool
    ab = tc.alloc_tile_pool(name="abc", bufs=1)
    kT = ab.tile([128, LH // 2, T], F8, tag="kT")
    qT = ab.tile([128, LH // 2, T], F8, tag="qT")
    vS = ab.tile([128, NST, LH * 128], BF, tag="vS")
    # early pool: x tiles, hT, qkv weights, LN1 temps (freed before FFN)
    pe_pool = tc.alloc_tile_pool(name="pearly", bufs=1)

    wq_sb = pe_pool.tile([128, KD, LHE], F8, tag="wq")
    wk_sb = pe_pool.tile([128, KD, LHE], F8, tag="wk")
    wv_sb = pe_pool.tile([128, KD, LHE], F8, tag="wv")

    def load_x(ci):
        xf = pe_pool.tile([128, KD, 512], BF, tag="xf", bufs=2, name="xf")
        engs = (nc.sync, nc.scalar, nc.gpsimd)
        for k in range(KD):
            engs[k % 3].dma_start(out=xf[:, k:k + 1, :],
                                  in_=t["xT"][ci * 128:ci * 128 + 128,
                                              k:k + 1, :])
        return xf

    def load_qkv_weights():
        for eng, srct, dst in ((nc.scalar, t["wq"], wq_sb),
                               (nc.scalar, t["wk"], wk_sb),
                               (nc.sync, t["wv"], wv_sb)):
            eng.dma_start(out=dst, in_=srct)

    def ln1(ci, xf, gps=False):
        Ab, Bb = _ln_stats_pe(nc, pe_pool, pps, "ps_proj", xf, ones_col)
        hT = pe_pool.tile([128, KD, 512], F8, tag="hT", bufs=2, name="hT")
        for k in range(KD):
            if gps and k % 2 == 1:
                _ln_apply(nc, pe_pool, xf[:, k, :], Ab, Bb, hT[:, k, :],
                          eng=nc.gpsimd, tag="ln_t1g")
            else:
                _ln_apply(nc, pe_pool, xf[:, k, :], Ab, Bb, hT[:, k, :])
        return hT

    def proj_units(ci, hT):
        c0 = ci * 512
        units = []
        for w_sb, dst, bias, rsc in ((wk_sb, kT, bk_sb, 1.0 / SK),
                                     (wq_sb, qT, bq_sb, 1.0 / SQ)):
            for et in range(LH // 2):
                def u(w_sb=w_sb, dst=dst, bias=bias, rsc=rsc, et=et,
                      hT=hT, c0=c0):
                    ps = pps.tile([128, 512], F32, tag="ps_proj",
                                  bufs=2, name="ps_proj")
                    for kp in range(KD // 2):
                        mm(out=ps,
                           lhsT=w_sb[:, 2 * kp:2 * kp + 2,
                                     et * 128:(et + 1) * 128],
                           rhs=hT[:, 2 * kp:2 * kp + 2, :],
                           start=(kp == 0), stop=(kp == KD // 2 - 1),
                           perf_mode=DRM)
                    nc.vector.tensor_scalar(
                        out=dst[:, et, c0:c0 + 512], in0=ps,
                        scalar1=bias[:, et:et + 1], scalar2=rsc,
                        op0=Alu.add, op1=Alu.mult)
                units.append(u)
        for sti in range(4):
            st = ci * 4 + sti
            def u(sti=sti, st=st, hT=hT):
                ps = pps.tile([128, LHE], F32, tag="ps_proj", bufs=2,
                              name="ps_v")
                for kp in range(KD // 2):
                    mm(out=ps,
                       lhsT=hT[:, 2 * kp:2 * kp + 2,
                               sti * 128:sti * 128 + 128],
                       rhs=wv_sb[:, 2 * kp:2 * kp + 2, :],
                       start=(kp == 0), stop=(kp == KD // 2 - 1),
                       perf_mode=DRM)
                nc.vector.scalar_tensor_tensor(
                    out=vS[:, st, :].rearrange("p (h e) -> p h e",
                                               h=LH)[:, :, 0:64],
                    in0=ps.rearrange("p (h e) -> p h e", e=64),
                    scalar=1.0 / SV, op0=Alu.mult,
                    in1=bvb.rearrange("p (h e) -> p h e", e=64),
                    op1=Alu.add)
            units.append(u)
        return units

    def attention(ci, oT, pending):
        c0 = ci * 512
        nb = 4 * (ci + 1)
        total_steps = (LH // 2) * (nb + OA_LAG)
        spacing = (max(1, total_steps // len(pending))
                   if pending else 0)
        stepctr = 0
        for hp in range(LH // 2):
            po = [pps.tile([128, 512], F32, tag="po", bufs=2, name="po")
                  for _ in range(2)]
            exs = [None] * nb

            def scores(sb):
                s0 = sb * 128
                midx = sb - 4 * ci
                w0 = max(0, midx) * 128
                ps2 = pps.tile([128, 2, 512], F32, tag="ps_sc",
                               bufs=OA_LAG, name="ps_sc")
                for hi in range(2):
                    mm(out=ps2[:, hi, w0:512],
                       lhsT=kT[hi * 64:hi * 64 + 64, hp, s0:s0 + 128],
                       rhs=qT[hi * 64:hi * 64 + 64, hp,
                              c0 + w0:c0 + 512],
                       start=True, stop=True)
                ex = ab.tile([128, 2, 512], BF, tag="ex",
                             bufs=OA_LAG + 1, name="ex")
                if w0 > 0:
                    nc.gpsimd.memset(ex[:, :, 0:w0], 0.0)
                nc.scalar.activation(out=ex[:, :, w0:512],
                                     in_=ps2[:, :, w0:512], func=Act.Exp)
                if midx >= 0:
                    for hi in range(2):
                        nc.gpsimd.affine_select(
                            out=ex[:, hi, w0:w0 + 128],
                            in_=ex[:, hi, w0:w0 + 128],
                            compare_op=Alu.is_ge, fill=0.0,
                            base=0, channel_multiplier=-1,
                            pattern=[[1, 128]])
                exs[sb] = ex

            def oacc(sb):
                for hi in range(2):
                    h_loc = hp * 2 + hi
                    mm(out=po[hi],
                       lhsT=vS[:, sb, h_loc * 128:h_loc * 128 + 128],
                       rhs=exs[sb][:, hi, :],
                       start=(sb == 0), stop=(sb == nb - 1))

            for step in range(nb + OA_LAG):
                if step < nb:
                    scores(step)
                if step >= OA_LAG:
                    oacc(step - OA_LAG)
                stepctr += 1
                if pending and stepctr % spacing == 0:
                    pending.pop(0)()

            for hi in range(2):
                h_loc = hp * 2 + hi
                dnr = ab.tile([1, 512], F32, tag="dnr", bufs=1,
                              name="dnr")
                nc.vector.tensor_copy(out=dnr, in_=po[hi][64:65, :])
                rcp = ab.tile([1, 512], F32, tag="rcp", bufs=2,
                              name="rcp")
                nc.vector.reciprocal_approx_fast(out=rcp, in_=dnr)
                bc = ab.tile([64, 512], F32, tag="bc", bufs=1,
                             name="bc")
                nc.gpsimd.partition_broadcast(bc, rcp)
                nc.vector.tensor_mul(
                    out=oT[hi * 64:hi * 64 + 64, hp, :],
                    in0=po[hi][0:64, :], in1=bc)
        while pending:
            pending.pop(0)()

    def wo_units(ci, oT):
        rsv = rs_in[ci]
        units = []
        for dt in range(KD):
            def u(dt=dt, oT=oT, rsv=rsv):
                ps = pps.tile([128, 512], F32, tag="ps_proj",
                              bufs=2, name="ps_wo")
                for kp in range(KHE // 2):
                    mm(out=ps,
                       lhsT=wo_sb[:, 2 * kp:2 * kp + 2,
                                  dt * 128:(dt + 1) * 128],
                       rhs=oT[:, 2 * kp:2 * kp + 2, :],
                       start=(kp == 0), stop=(kp == KHE // 2 - 1),
                       perf_mode=DRM)
                stg = ab.tile([128, 512], BF, tag="stg1", bufs=2,
                              name="stg1")
                nc.vector.tensor_scalar(
                    out=stg, in0=ps, scalar1=bo2_sb[:, dt:dt + 1],
                    scalar2=1.0 / SO, op0=Alu.add, op1=Alu.mult)
                for j in range(TP):
                    nc.sync.dma_start(
                        out=rsv[j, :, dt, :],
                        in_=stg[:, j * 256:(j + 1) * 256])
            units.append(u)
        return units

    def rs_unit(ci):
        def u():
            nc.gpsimd.collective_compute(
                "ReduceScatter", Alu.add, replica_groups=PAIRS,
                ins=[rs_in[ci].opt()], outs=[rs_out[ci].opt()])
        return u

    # ---------------- stage B (FFN) units (pool allocated later) --------
    state = {}

    def prep_units(lc):
        """residual + LN2 + apply, decomposed into pending units."""
        de = state["de"]
        units = []

        def u_load():
            xmid = de.tile([128, KD, 512], BF, tag="xmid", bufs=2,
                           name="xmid")
            xrs = t["xresT"][lc * 128:lc * 128 + 128, :, :]
            nc.sync.dma_start(out=xmid[:, 0:4, :], in_=xrs[:, 0:4, :])
            nc.scalar.dma_start(out=xmid[:, 4:8, :], in_=xrs[:, 4:8, :])
            arr = de.tile([128, KD, 2, 256], BF, tag="arr", bufs=1,
                          name="arr")
            nc.sync.dma_start(out=arr[:, 0:4, 0, :],
                              in_=rs_out[2 * lc][:, 0:4, :])
            nc.scalar.dma_start(out=arr[:, 4:8, 0, :],
                                in_=rs_out[2 * lc][:, 4:8, :])
            nc.sync.dma_start(out=arr[:, 0:4, 1, :],
                              in_=rs_out[2 * lc + 1][:, 0:4, :])
            nc.scalar.dma_start(out=arr[:, 4:8, 1, :],
                                in_=rs_out[2 * lc + 1][:, 4:8, :])
            state[f"xmid{lc}"] = xmid
            state[f"arr{lc}"] = arr
        units.append(u_load)

        def u_add():
            xmid = state[f"xmid{lc}"]
            arr = state[f"arr{lc}"]
            for k in range(KD):
                nc.vector.tensor_add(
                    out=xmid[:, k, :], in0=xmid[:, k, :],
                    in1=arr[:, k, :, :].rearrange("p j t -> p (j t)"))
        units.append(u_add)

        def u_stats():
            xmid = state[f"xmid{lc}"]
            state[f"ab{lc}"] = _ln_stats_pe(nc, de, pps, "ps_proj",
                                            xmid, ones_col)
        units.append(u_stats)

        def mk_apply(k0):
            def u_apply():
                xmid = state[f"xmid{lc}"]
                Ab2, Bb2 = state[f"ab{lc}"]
                if f"h2{lc}" not in state:
                    state[f"h2{lc}"] = de.tile([128, KD, 512], BF,
                                               tag="h2", bufs=1, name="h2")
                h2 = state[f"h2{lc}"]
                for k in range(k0, k0 + 4):
                    _ln_apply(nc, de, xmid[:, k, :], Ab2, Bb2, h2[:, k, :])
            return u_apply
        units.append(mk_apply(0))
        units.append(mk_apply(4))
        return units

    def fc1_units(lc):
        """FFN up: u = relu(h2 @ W1 + b1f); one unit per 256-wide block."""
        de = state["de"]
        units = []

        def mk(q16):
            def u():
                h2 = state[f"h2{lc}"]
                if f"u{lc}" not in state:
                    state[f"u{lc}"] = de.tile([128, KFF, 512], BF,
                                              tag="u", bufs=1, name="u")
                uu = state[f"u{lc}"]
                w1t = de.tile([128, KD, 256], BF, tag="w1t", bufs=2,
                              name="w1t")
                nc.sync.dma_start(
                    out=w1t, in_=t["w1"][q16 * 128:(q16 + 1) * 128, :, :])
                for fi in range(2):
                    fft = q16 * 2 + fi
                    ps = pps.tile([128, 512], F32, tag="ps_proj", bufs=2,
                                  name="ps_u")
                    for k in range(KD):
                        mm(out=ps,
                           lhsT=w1t[:, k, fi * 128:fi * 128 + 128],
                           rhs=h2[:, k, :],
                           start=(k == 0), stop=(k == KD - 1))
                    if lc == 0:
                        nc.vector.tensor_scalar(
                            out=uu[:, fft, :], in0=ps,
                            scalar1=b1_sb[:, fft:fft + 1], scalar2=0.0,
                            op0=Alu.add, op1=Alu.max)
                    else:
                        nc.scalar.activation(
                            out=uu[:, fft, :], in_=ps, func=Act.Relu,
                            bias=b1_sb[:, fft:fft + 1])
            units.append(u)
        for q16 in range(16):
            mk(q16)
        return units

    def fc2_units(lc):
        """FFN down + bias + residual -> store; W2 streamed per dt."""
        de = state["de"]
        c0 = lc * 512
        units = []

        def mk(dt):
            def u():
                uu = state[f"u{lc}"]
                xmid = state[f"xmid{lc}"]
                w2t = state.pop(f"w2t{dt}", None) if lc == 0 else None
                if w2t is None:
                    w2t = de.tile([128, KFF, 128], BF, tag="w2t", bufs=2,
                                  name="w2t")
                    nc.scalar.dma_start(
                        out=w2t,
                        in_=t["w2"][dt * 128:(dt + 1) * 128, :, :])
                ps = pps.tile([128, 512], F32, tag="ps_proj", bufs=2,
                              name="ps_f")
                for k2 in range(KFF):
                    mm(out=ps, lhsT=w2t[:, k2, :], rhs=uu[:, k2, :],
                       start=(k2 == 0), stop=(k2 == KFF - 1))
                o_f = de.tile([128, 512], F32, tag="o_f", bufs=2,
                              name="o_f")
                nc.vector.scalar_tensor_tensor(
                    out=o_f, in0=ps, scalar=b2_sb[:, dt:dt + 1],
                    in1=xmid[:, dt, :], op0=Alu.add, op1=Alu.add)
                nc.sync.dma_start(
                    out=outT_v[dt * 128:(dt + 1) * 128, c0:c0 + 512],
                    in_=o_f)
            units.append(u)
        for dt in range(KD):
            mk(dt)
        return units

    # ================= emission schedule =================
    # vS zero + softmax-denominator ones columns
    nc.vector.memset(vS[:, 0:8, :], 0.0)
    nc.gpsimd.memset(vS[:, 8:16, :], 0.0)
    for h in range(LH):
        nc.vector.memset(vS[:, :, h * 128 + 64:h * 128 + 65], 1.0)

    xf0 = load_x(0)
    load_qkv_weights()
    xf1 = load_x(1)

    def w2_prefetch(dt):
        def u():
            de = state["de"]
            w2t = de.tile([128, KFF, 128], BF, tag="w2t", bufs=2,
                          name="w2t")
            nc.scalar.dma_start(
                out=w2t, in_=t["w2"][dt * 128:(dt + 1) * 128, :, :])
            state[f"w2t{dt}"] = w2t
        return u

    hT0 = ln1(0, xf0, gps=True)
    for u in proj_units(0, hT0):
        u()
    hT1 = ln1(1, xf1, gps=True)

    oTs = {}
    for ci in range(NCH):
        oTs[ci] = None

    def new_oT():
        return ab.tile([128, KHE, 512], F8, tag="oT", bufs=2, name="oT")

    oTs[0] = new_oT()
    attention(0, oTs[0], list(proj_units(1, hT1)))

    xf2 = load_x(2)
    hT2 = ln1(2, xf2)
    xf3 = load_x(3)
    hT3 = ln1(3, xf3)

    oTs[1] = new_oT()
    attention(1, oTs[1],
              wo_units(0, oTs[0]) + [rs_unit(0)]
              + proj_units(2, hT2) + proj_units(3, hT3))
    pe_pool.release()
    state["de"] = tc.alloc_tile_pool(name="de", bufs=1)

    oTs[3] = new_oT()
    attention(3, oTs[3], wo_units(1, oTs[1]) + [rs_unit(1)])

    oTs[2] = new_oT()
    attention(2, oTs[2],
              wo_units(3, oTs[3]) + [rs_unit(3)]
              + prep_units(0) + [w2_prefetch(0), w2_prefetch(1)]
              + fc1_units(0))

    for u in wo_units(2, oTs[2]):
        u()
    rs_unit(2)()

    # interleave prep(1) with fc2(0), then fc1(1), fc2(1)
    p1 = prep_units(1)
    f20 = fc2_units(0)
    inter = []
    while p1 or f20:
        if f20:
            inter.append(f20.pop(0))
        if p1:
            inter.append(p1.pop(0))
    for u in inter:
        u()
    for u in fc1_units(1):
        u()
    for u in fc2_units(1):
        u()

    state["de"].release()
    ab.release()
    pps.release()
    wlate.release()
    consts.release()
    dram.release()


def _build():
    nc = bacc.Bacc("TRN2", target_bir_lowering=False, debug=False,
                   num_devices=NCORES)

    tensors = {}
    tensors["xT"] = nc.dram_tensor("xT", [NCH * 128, KD, 512], BF,
                                   kind="ExternalInput").ap()
    tensors["xresT"] = nc.dram_tensor("xresT", [TP * 128, KD, 512], BF,
                                      kind="ExternalInput").ap()
    for name, shape, dt in (
        ("wq", [128, KD, 512], F8), ("wk", [128, KD, 512], F8),
        ("wv", [128, KD, 512], F8), ("wo", [128, KHE, D], F8),
        ("w1", [16 * 128, KD, 256], BF), ("w2", [8 * 128, KFF, 128], BF),
        ("bq", [LHE], F32), ("bk", [LHE], F32), ("bv", [LHE], F32),
        ("b1f", [FF], F32), ("bo2", [D], F32), ("b2", [D], F32),
    ):
        tensors[name] = nc.dram_tensor(name, shape, dt,
                                       kind="ExternalInput").ap()
    tensors["outT"] = nc.dram_tensor("out", [D, LT], F32,
                                     kind="ExternalOutput").ap()

    with tile.TileContext(nc, num_cores=NCORES) as tc:
        _emit(nc, tc, tensors)

    nc.compile()
    return nc


_NC_CACHE = None


def _get_nc():
    global _NC_CACHE
    if _NC_CACHE is None:
        _NC_CACHE = _build()
    return _NC_CACHE


def _shard_inputs(x, Wq, Wk, Wv, Wo, bo, W1, b1, W2, b2, g1, be1, g2, be2):
    """Build the 8 per-core input maps (LN gains folded into weights)."""
    bf = lambda a: np.ascontiguousarray(a).astype(BF16NP)
    f8 = lambda a: np.ascontiguousarray(a).astype(FP8NP)
    f32 = lambda a: np.ascontiguousarray(a, dtype=np.float32)

    x = np.asarray(x, dtype=np.float32)
    Wq = np.asarray(Wq, dtype=np.float32)
    Wk = np.asarray(Wk, dtype=np.float32)
    Wv = np.asarray(Wv, dtype=np.float32)
    Wo = np.asarray(Wo, dtype=np.float32)
    W1 = np.asarray(W1, dtype=np.float32)
    W2 = np.asarray(W2, dtype=np.float32)
    g1 = np.asarray(g1, dtype=np.float32)
    be1 = np.asarray(be1, dtype=np.float32)
    g2 = np.asarray(g2, dtype=np.float32)
    be2 = np.asarray(be2, dtype=np.float32)
    b1 = np.asarray(b1, dtype=np.float32)

    scale = float(HS) ** -0.5
    # fold g1 into QKV weights, be1 into QKV biases; fold the score scale
    # into Wq/bq.  Per-head [H, D, HS] -> concat heads -> [D, H*HS].
    wq_f = (g1[None, :, None] * Wq).transpose(1, 0, 2).reshape(D, D) * scale
    wk_f = (g1[None, :, None] * Wk).transpose(1, 0, 2).reshape(D, D)
    wv_f = (g1[None, :, None] * Wv).transpose(1, 0, 2).reshape(D, D)
    bq_f = np.einsum("d,hde->he", be1, Wq).reshape(D) * scale
    bk_f = np.einsum("d,hde->he", be1, Wk).reshape(D)
    bv_f = np.einsum("d,hde->he", be1, Wv).reshape(D)
    # fold g2/be2 into W1/b1
    w1_f = g2[:, None] * W1
    b1_f = b1 + be2 @ W1

    in_maps = []
    for c in range(NCORES):
        b, half = divmod(c, TP)
        hes = slice(half * LHE, (half + 1) * LHE)
        xt = x[b].T
        xres = np.concatenate(
            [xt[:, ci * 512 + half * 256: ci * 512 + half * 256 + 256]
             for ci in range(NCH)], axis=1)
        # partition-major tiled layouts: loads become 128 contiguous
        # segments instead of 1024 scattered ones (descriptor-gen bound)
        xt_sw = xt.reshape(KD, 128, NCH, 512).transpose(2, 1, 0, 3)
        xres_sw = xres.reshape(KD, 128, TP, 512).transpose(2, 1, 0, 3)
        wq_sw = wq_f[:, hes].reshape(KD, 128, LHE).transpose(1, 0, 2)
        wk_sw = wk_f[:, hes].reshape(KD, 128, LHE).transpose(1, 0, 2)
        wv_sw = wv_f[:, hes].reshape(KD, 128, LHE).transpose(1, 0, 2)
        wo_sw = Wo[hes, :].reshape(KHE, 128, D).transpose(1, 0, 2)
        w1_sw = w1_f.reshape(KD, 128, 16, 256).transpose(2, 1, 0, 3)
        # W2 tiled dt-major so fc2 streams contiguous [128, KFF, 128] tiles
        w2_sw = W2.reshape(KFF, 128, KD, 128).transpose(2, 1, 0, 3)
        in_maps.append({
            "xT": bf(xt_sw.reshape(NCH * 128, KD, 512)),
            "xresT": bf(xres_sw.reshape(TP * 128, KD, 512)),
            "wq": f8(wq_sw * SQ), "wk": f8(wk_sw * SK), "wv": f8(wv_sw * SV),
            "bq": f32(bq_f[hes] * SQ), "bk": f32(bk_f[hes] * SK),
            "bv": f32(bv_f[hes]),
            "wo": f8(wo_sw * SO),
            "bo2": f32(np.asarray(bo, dtype=np.float32) * SO / TP),
            "w1": bf(w1_sw.reshape(16 * 128, KD, 256)), "b1f": f32(b1_f),
            "w2": bf(w2_sw.reshape(8 * 128, KFF, 128)),
            "b2": f32(np.asarray(b2, dtype=np.float32)),
        })
    return in_maps


def kernel(x, Wq, Wk, Wv, Wo, bo, W1, b1, W2, b2, g1, be1, g2, be2,
           _trace=False):
    nc = _get_nc()
    in_maps = _shard_inputs(x, Wq, Wk, Wv, Wo, bo, W1, b1, W2, b2,
                            g1, be1, g2, be2)
    res = run_bass_kernel_spmd(nc, in_maps, list(range(NCORES)),
                               trace=_trace)
    out = np.empty((B, T, D), dtype=np.float32)
    for b in range(B):
        for half in range(TP):
            o = res.results[TP * b + half]["out"]  # [D, LT]
            for ci in range(NCH):
                t0 = ci * 512 + half * 256
                out[b, t0:t0 + 256, :] = o[:, ci * 256:(ci + 1) * 256].T
    if _trace:
        kernel.last_exec_time_ns = res.exec_time_ns
        kernel.last_results = res
    return out


# revision 40
# speedup vs baseline: 1.1212x; 1.0110x over previous
"""Trainium2 Bass kernel for a pre-LN transformer block (B=4, T=2048, D=1024,
H=16, HS=64, FF=4096, causal attention).

Sharding: data-parallel over batches x 2-way tensor-parallel attention
(8 heads/core over all T) -> pair ReduceScatter of the attention-output
projection over the sequence dim -> sequence-parallel FFN (full FF width,
T/2 rows per core).  No AllReduce anywhere; each core emits the final
output for its own T/2 rows.

Core c (0..7): batch b = c//2, half = c%2.  half h owns t-slices
[ci*512 + h*256, ci*512 + h*256 + 256) for ci in 0..3.

v2 schedule: attention chunks run 0,1,3,2 so that all four ReduceScatters
except the last are issued mid-attention, and the FFN for the first half
of rows (prep+fc1) is interleaved into the final attention chunk as
pending units -- the tail RS and LN2 hide behind fc1/fc2 matmuls.
Attention-path tensors (weights, hT, kT, qT, oT) are fp8e4m3 with
power-of-2 scales folded into the projection epilogues; this halves
their SBUF/DMA cost (matmuls run at bf16 rate).  LN1 stats come from
M=1 PE matmuls instead of DVE adder trees.  Diagonal score tiles are
N-trimmed and their exp is windowed, with the causal mask applied only
to the [128,128] band.  W1/W2 stream through double-buffered tiles.
"""

import numpy as np
import ml_dtypes

import concourse.bacc as bacc
import concourse.bass as bass
import concourse.mybir as mybir
import concourse.tile as tile
from concourse.bass_utils import run_bass_kernel_spmd

BF16NP = ml_dtypes.bfloat16
FP8NP = ml_dtypes.float8_e4m3

B, T, D, H, HS, FF = 4, 2048, 1024, 16, 64, 4096
EPS = 1e-5
NCORES = 8
TP = 2
LH = H // TP          # 8 local heads
LHE = LH * HS         # 512 local head-embed width
LT = T // TP          # 1024 local rows (FFN/output)
KD = D // 128         # 8 d k-tiles
KHE = LHE // 128      # 4 he k-tiles
KFF = FF // 128       # 32 ff tiles
NCH = T // 512        # 4 t-chunks of 512
NST = T // 128        # 16 s-tiles of 128
PAIRS = [[0, 1], [2, 3], [4, 5], [6, 7]]
OA_LAG = 2            # psc tiles in flight between scores and o-accum

F32 = mybir.dt.float32
BF = mybir.dt.bfloat16
F8 = mybir.dt.float8e4
F85 = mybir.dt.float8e5
DRM = mybir.MatmulPerfMode.DoubleRow
# fp8 weight scales (power-of-2, folded out in the epilogues)
SQ = 256.0   # wq carries g1 and HS^-0.5 -> sigma 1/256
SK = 32.0
SV = 32.0
SO = 32.0


def _ln_math(nc, pool, ps_s, ps_q):
    """From psum row-sums (ps_s, ps_q over D) to bf16 broadcast tiles
    (Ab, Bb) so that xn = x*Ab + Bb."""
    Alu = mybir.AluOpType
    Act = mybir.ActivationFunctionType
    m = pool.tile([1, 512], F32, tag="ln_m", bufs=1, name="ln_m")
    e2 = pool.tile([1, 512], F32, tag="ln_e2", bufs=1, name="ln_e2")
    nc.vector.tensor_scalar_mul(out=m, in0=ps_s, scalar1=1.0 / D)
    nc.vector.tensor_scalar_mul(out=e2, in0=ps_q, scalar1=1.0 / D)
    msq = pool.tile([1, 512], F32, tag="ln_msq", bufs=1, name="ln_msq")
    nc.vector.tensor_mul(out=msq, in0=m, in1=m)
    var = pool.tile([1, 512], F32, tag="ln_var", bufs=1, name="ln_var")
    nc.vector.scalar_tensor_tensor(out=var, in0=e2, scalar=EPS, in1=msq,
                                   op0=Alu.add, op1=Alu.subtract)
    sd = pool.tile([1, 512], F32, tag="ln_sd", bufs=1, name="ln_sd")
    nc.scalar.activation(out=sd, in_=var, func=Act.Sqrt)
    a_row = pool.tile([1, 512], F32, tag="ln_a", bufs=1, name="ln_a")
    nc.vector.reciprocal_approx_fast(out=a_row, in_=sd)
    b_row = pool.tile([1, 512], F32, tag="ln_b", bufs=1, name="ln_b")
    nc.vector.scalar_tensor_tensor(out=b_row, in0=m, scalar=-1.0, in1=a_row,
                                   op0=Alu.mult, op1=Alu.mult)
    ac = pool.tile([1, 512], BF, tag="ln_ac", bufs=1, name="ln_ac")
    bc = pool.tile([1, 512], BF, tag="ln_bc", bufs=1, name="ln_bc")
    nc.vector.tensor_copy(out=ac, in_=a_row)
    nc.vector.tensor_copy(out=bc, in_=b_row)
    Ab = pool.tile([128, 512], BF, tag="ln_Ab", bufs=2, name="ln_Ab")
    Bb = pool.tile([128, 512], BF, tag="ln_Bb", bufs=2, name="ln_Bb")
    nc.gpsimd.partition_broadcast(Ab, ac)
    nc.gpsimd.partition_broadcast(Bb, bc)
    return Ab, Bb


def _ln_stats_pe(nc, pool, psum_pool, psum_tag, src, ones_col):
    """LN stats via M=1 PE matmuls: sum(x) directly on the x k-tiles,
    sum(x^2) on DVE-squared tiles.  src: [128, KD, 512] bf16."""
    mm = nc.tensor.matmul
    ps_s = psum_pool.tile([1, 512], F32, tag=psum_tag, bufs=2, name="ps_s")
    ps_q = psum_pool.tile([1, 512], F32, tag=psum_tag, bufs=2, name="ps_q")
    for k in range(KD):
        mm(out=ps_s, lhsT=ones_col, rhs=src[:, k, :],
           start=(k == 0), stop=(k == KD - 1))
    for k in range(KD):
        sq = pool.tile([128, 512], BF, tag="ln_sq", bufs=2, name="ln_sq")
        nc.vector.tensor_mul(out=sq, in0=src[:, k, :], in1=src[:, k, :])
        mm(out=ps_q, lhsT=ones_col, rhs=sq,
           start=(k == 0), stop=(k == KD - 1))
    return _ln_math(nc, pool, ps_s, ps_q)


def _ln_stats_tree(nc, pool, psum_pool, psum_tag, src, ones_col):
    """DVE adder-tree LN stats (kept for LN2 where the PE is contended).
    src: [128, KD, 512] AP."""
    mm = nc.tensor.matmul

    def lvl(tg, n):
        return pool.tile([128, 512], BF, tag=f"{tg}{n}", bufs=2, name=tg)

    s2, q2 = [], []
    for i in range(4):
        s = lvl("lts", 2)
        nc.vector.tensor_add(out=s, in0=src[:, 2 * i, :],
                             in1=src[:, 2 * i + 1, :])
        s2.append(s)
        sqa = pool.tile([128, 512], BF, tag="ln_sq", bufs=2, name="ln_sq")
        sqb = pool.tile([128, 512], BF, tag="ln_sq", bufs=2, name="ln_sq")
        nc.vector.tensor_mul(out=sqa, in0=src[:, 2 * i, :],
                             in1=src[:, 2 * i, :])
        nc.vector.tensor_mul(out=sqb, in0=src[:, 2 * i + 1, :],
                             in1=src[:, 2 * i + 1, :])
        q = lvl("ltq", 2)
        nc.vector.tensor_add(out=q, in0=sqa, in1=sqb)
        q2.append(q)
    s4, q4_ = [], []
    for i in range(2):
        s = lvl("lts", 4)
        nc.vector.tensor_add(out=s, in0=s2[2 * i], in1=s2[2 * i + 1])
        s4.append(s)
        q = lvl("ltq", 4)
        nc.vector.tensor_add(out=q, in0=q2[2 * i], in1=q2[2 * i + 1])
        q4_.append(q)
    s_all = lvl("lts", 8)
    nc.vector.tensor_add(out=s_all, in0=s4[0], in1=s4[1])
    q_all = lvl("ltq", 8)
    nc.vector.tensor_add(out=q_all, in0=q4_[0], in1=q4_[1])

    ps_s = psum_pool.tile([1, 512], F32, tag=psum_tag, bufs=2, name="ps_s")
    ps_q = psum_pool.tile([1, 512], F32, tag=psum_tag, bufs=2, name="ps_q")
    mm(out=ps_s, lhsT=ones_col, rhs=s_all, start=True, stop=True)
    mm(out=ps_q, lhsT=ones_col, rhs=q_all, start=True, stop=True)
    return _ln_math(nc, pool, ps_s, ps_q)


def _ln_apply(nc, pool, src_k, Ab, Bb, out_slice, eng=None, tag="ln_t1"):
    """out = src*Ab + Bb."""
    eng = eng or nc.vector
    t1 = pool.tile([128, 512], BF, tag=tag, bufs=2, name="ln_t1")
    eng.tensor_mul(out=t1, in0=src_k, in1=Ab)
    eng.tensor_add(out=out_slice, in0=t1, in1=Bb)


def _emit(nc, tc, t):
    mm = nc.tensor.matmul
    Alu = mybir.AluOpType
    Act = mybir.ActivationFunctionType

    outT_v = t["outT"]

    # ---------------- persistent pools ----------------
    dram = tc.alloc_tile_pool(name="dram", bufs=1, space="DRAM")
    rs_in = [dram.tile([TP, 128, KD, 256], BF, name=f"rsi{c}")
             for c in range(NCH)]
    rs_out = [dram.tile([128, KD, 256], BF, name=f"rso{c}")
              for c in range(NCH)]

    consts = tc.alloc_tile_pool(name="consts", bufs=1)
    ones_col = consts.tile([128, 1], BF)
    nc.vector.memset(ones_col, 1.0)

    bq_sb = consts.tile([128, KHE], F32)
    bk_sb = consts.tile([128, KHE], F32)
    bo2_sb = consts.tile([128, KD], F32)
    b2_sb = consts.tile([128, KD], F32)
    b1_sb = consts.tile([128, KFF], F32)
    for name, dst in (("bq", bq_sb), ("bk", bk_sb),
                      ("bo2", bo2_sb), ("b2", b2_sb)):
        nc.sync.dma_start(out=dst, in_=t[name].rearrange("(k p) -> p k", p=128))
    nc.sync.dma_start(out=b1_sb, in_=t["b1f"].rearrange("(k p) -> p k", p=128))
    # v bias broadcast over all partitions: [128, LHE]
    bvb = consts.tile([128, LHE], BF)
    bv_row = consts.tile([1, LHE], BF)
    nc.gpsimd.dma_start(out=bv_row,
                        in_=t["bv"].rearrange("(o e) -> o e", o=1))
    nc.gpsimd.partition_broadcast(bvb, bv_row)

    wlate = tc.alloc_tile_pool(name="wlate", bufs=1)
    wo_sb = wlate.tile([128, KHE, D], F8, tag="wo")
    nc.scalar.dma_start(out=wo_sb, in_=t["wo"])

    # single PSUM pool: ps_proj(2) + ps_sc(2x2) + po(2) = 8 banks
    pps = tc.alloc_tile_pool(name="pps", bufs=2, space="PSUM")

    # stage A long-lived pool
    ab = tc.alloc_tile_pool(name="abc", bufs=1)
    kT = ab.tile([128, LH // 2, T], F8, tag="kT")
    qT = ab.tile([128, LH // 2, T], F8, tag="qT")
    vS = ab.tile([128, NST, LH * 128], BF, tag="vS")
    # early pool: x tiles, hT, qkv weights, LN1 temps (freed before FFN)
    pe_pool = tc.alloc_tile_pool(name="pearly", bufs=1)

    wq_sb = pe_pool.tile([128, KD, LHE], F8, tag="wq")
    wk_sb = pe_pool.tile([128, KD, LHE], F8, tag="wk")
    wv_sb = pe_pool.tile([128, KD, LHE], F8, tag="wv")

    def load_x(ci):
        xf = pe_pool.tile([128, KD, 512], BF, tag="xf", bufs=2, name="xf")
        engs = (nc.sync, nc.scalar, nc.gpsimd)
        for k in range(KD):
            engs[k % 3].dma_start(out=xf[:, k:k + 1, :],
                                  in_=t["xT"][ci * 128:ci * 128 + 128,
                                              k:k + 1, :])
        return xf

    def load_qkv_weights():
        for eng, srct, dst in ((nc.scalar, t["wq"], wq_sb),
                               (nc.scalar, t["wk"], wk_sb),
                               (nc.sync, t["wv"], wv_sb)):
            eng.dma_start(out=dst, in_=srct)

    def ln1(ci, xf, gps=False):
        Ab, Bb = _ln_stats_pe(nc, pe_pool, pps, "ps_proj", xf, ones_col)
        hT = pe_pool.tile([128, KD, 512], F8, tag="hT", bufs=2, name="hT")
        for k in range(KD):
            if gps and k % 2 == 1:
                _ln_apply(nc, pe_pool, xf[:, k, :], Ab, Bb, hT[:, k, :],
                          eng=nc.gpsimd, tag="ln_t1g")
            else:
                _ln_apply(nc, pe_pool, xf[:, k, :], Ab, Bb, hT[:, k, :])
        return hT

    def proj_units(ci, hT):
        c0 = ci * 512
        units = []
        for w_sb, dst, bias, rsc in ((wk_sb, kT, bk_sb, 1.0 / SK),
                                     (wq_sb, qT, bq_sb, 1.0 / SQ)):
            for et in range(LH // 2):
                def u(w_sb=w_sb, dst=dst, bias=bias, rsc=rsc, et=et,
                      hT=hT, c0=c0):
                    ps = pps.tile([128, 512], F32, tag="ps_proj",
                                  bufs=2, name="ps_proj")
                    for k in range(KD):
                        mm(out=ps,
                           lhsT=w_sb[:, k, et * 128:(et + 1) * 128],
                           rhs=hT[:, k, :],
                           start=(k == 0), stop=(k == KD - 1))
                    nc.vector.tensor_scalar(
                        out=dst[:, et, c0:c0 + 512], in0=ps,
                        scalar1=bias[:, et:et + 1], scalar2=rsc,
                        op0=Alu.add, op1=Alu.mult)
                units.append(u)
        for sti in range(4):
            st = ci * 4 + sti
            def u(sti=sti, st=st, hT=hT):
                ps = pps.tile([128, LHE], F32, tag="ps_proj", bufs=2,
                              name="ps_v")
                for k in range(KD):
                    mm(out=ps,
                       lhsT=hT[:, k, sti * 128:sti * 128 + 128],
                       rhs=wv_sb[:, k, :],
                       start=(k == 0), stop=(k == KD - 1))
                nc.vector.scalar_tensor_tensor(
                    out=vS[:, st, :].rearrange("p (h e) -> p h e",
                                               h=LH)[:, :, 0:64],
                    in0=ps.rearrange("p (h e) -> p h e", e=64),
                    scalar=1.0 / SV, op0=Alu.mult,
                    in1=bvb.rearrange("p (h e) -> p h e", e=64),
                    op1=Alu.add)
            units.append(u)
        return units

    def attention(ci, oT, pending):
        c0 = ci * 512
        nb = 4 * (ci + 1)
        total_steps = (LH // 2) * (nb + OA_LAG)
        spacing = (max(1, total_steps // len(pending))
                   if pending else 0)
        stepctr = 0
        for hp in range(LH // 2):
            po = [pps.tile([128, 512], F32, tag="po", bufs=2, name="po")
                  for _ in range(2)]
            exs = [None] * nb

            def scores(sb):
                s0 = sb * 128
                midx = sb - 4 * ci
                w0 = max(0, midx) * 128
                ps2 = pps.tile([128, 2, 512], F32, tag="ps_sc",
                               bufs=OA_LAG, name="ps_sc")
                for hi in range(2):
                    mm(out=ps2[:, hi, w0:512],
                       lhsT=kT[hi * 64:hi * 64 + 64, hp, s0:s0 + 128],
                       rhs=qT[hi * 64:hi * 64 + 64, hp,
                              c0 + w0:c0 + 512],
                       start=True, stop=True)
                ex = ab.tile([128, 2, 512], BF, tag="ex",
                             bufs=OA_LAG + 1, name="ex")
                if w0 > 0:
                    nc.gpsimd.memset(ex[:, :, 0:w0], 0.0)
                nc.scalar.activation(out=ex[:, :, w0:512],
                                     in_=ps2[:, :, w0:512], func=Act.Exp)
                if midx >= 0:
                    for hi in range(2):
                        nc.gpsimd.affine_select(
                            out=ex[:, hi, w0:w0 + 128],
                            in_=ex[:, hi, w0:w0 + 128],
                            compare_op=Alu.is_ge, fill=0.0,
                            base=0, channel_multiplier=-1,
                            pattern=[[1, 128]])
                exs[sb] = ex

            def oacc(sb):
                for hi in range(2):
                    h_loc = hp * 2 + hi
                    mm(out=po[hi],
                       lhsT=vS[:, sb, h_loc * 128:h_loc * 128 + 128],
                       rhs=exs[sb][:, hi, :],
                       start=(sb == 0), stop=(sb == nb - 1))

            for step in range(nb + OA_LAG):
                if step < nb:
                    scores(step)
                if step >= OA_LAG:
                    oacc(step - OA_LAG)
                stepctr += 1
                if pending and stepctr % spacing == 0:
                    pending.pop(0)()

            for hi in range(2):
                h_loc = hp * 2 + hi
                dnr = ab.tile([1, 512], F32, tag="dnr", bufs=1,
                              name="dnr")
                nc.vector.tensor_copy(out=dnr, in_=po[hi][64:65, :])
                rcp = ab.tile([1, 512], F32, tag="rcp", bufs=2,
                              name="rcp")
                nc.vector.reciprocal_approx_fast(out=rcp, in_=dnr)
                bc = ab.tile([64, 512], F32, tag="bc", bufs=1,
                             name="bc")
                nc.gpsimd.partition_broadcast(bc, rcp)
                nc.vector.tensor_mul(
                    out=oT[hi * 64:hi * 64 + 64, hp, :],
                    in0=po[hi][0:64, :], in1=bc)
        while pending:
            pending.pop(0)()

    def wo_units(ci, oT):
        rsv = rs_in[ci]
        units = []
        for dt in range(KD):
            def u(dt=dt, oT=oT, rsv=rsv):
                ps = pps.tile([128, 512], F32, tag="ps_proj",
                              bufs=2, name="ps_wo")
                for k in range(KHE):
                    mm(out=ps,
                       lhsT=wo_sb[:, k, dt * 128:(dt + 1) * 128],
                       rhs=oT[:, k, :],
                       start=(k == 0), stop=(k == KHE - 1))
                stg = ab.tile([128, 512], BF, tag="stg1", bufs=2,
                              name="stg1")
                nc.vector.tensor_scalar(
                    out=stg, in0=ps, scalar1=bo2_sb[:, dt:dt + 1],
                    scalar2=1.0 / SO, op0=Alu.add, op1=Alu.mult)
                for j in range(TP):
                    nc.sync.dma_start(
                        out=rsv[j, :, dt, :],
                        in_=stg[:, j * 256:(j + 1) * 256])
            units.append(u)
        return units

    def rs_unit(ci):
        def u():
            nc.gpsimd.collective_compute(
                "ReduceScatter", Alu.add, replica_groups=PAIRS,
                ins=[rs_in[ci].opt()], outs=[rs_out[ci].opt()])
        return u

    # ---------------- stage B (FFN) units (pool allocated later) --------
    state = {}

    def prep_units(lc):
        """residual + LN2 + apply, decomposed into pending units."""
        de = state["de"]
        units = []

        def u_load():
            xmid = de.tile([128, KD, 512], BF, tag="xmid", bufs=2,
                           name="xmid")
            xrs = t["xresT"][lc * 128:lc * 128 + 128, :, :]
            nc.sync.dma_start(out=xmid[:, 0:4, :], in_=xrs[:, 0:4, :])
            nc.scalar.dma_start(out=xmid[:, 4:8, :], in_=xrs[:, 4:8, :])
            arr = de.tile([128, KD, 2, 256], BF, tag="arr", bufs=1,
                          name="arr")
            nc.sync.dma_start(out=arr[:, 0:4, 0, :],
                              in_=rs_out[2 * lc][:, 0:4, :])
            nc.scalar.dma_start(out=arr[:, 4:8, 0, :],
                                in_=rs_out[2 * lc][:, 4:8, :])
            nc.sync.dma_start(out=arr[:, 0:4, 1, :],
                              in_=rs_out[2 * lc + 1][:, 0:4, :])
            nc.scalar.dma_start(out=arr[:, 4:8, 1, :],
                                in_=rs_out[2 * lc + 1][:, 4:8, :])
            state[f"xmid{lc}"] = xmid
            state[f"arr{lc}"] = arr
        units.append(u_load)

        def u_add():
            xmid = state[f"xmid{lc}"]
            arr = state[f"arr{lc}"]
            for k in range(KD):
                nc.vector.tensor_add(
                    out=xmid[:, k, :], in0=xmid[:, k, :],
                    in1=arr[:, k, :, :].rearrange("p j t -> p (j t)"))
        units.append(u_add)

        def u_stats():
            xmid = state[f"xmid{lc}"]
            state[f"ab{lc}"] = _ln_stats_pe(nc, de, pps, "ps_proj",
                                            xmid, ones_col)
        units.append(u_stats)

        def mk_apply(k0):
            def u_apply():
                xmid = state[f"xmid{lc}"]
                Ab2, Bb2 = state[f"ab{lc}"]
                if f"h2{lc}" not in state:
                    state[f"h2{lc}"] = de.tile([128, KD, 512], BF,
                                               tag="h2", bufs=1, name="h2")
                h2 = state[f"h2{lc}"]
                for k in range(k0, k0 + 4):
                    _ln_apply(nc, de, xmid[:, k, :], Ab2, Bb2, h2[:, k, :])
            return u_apply
        units.append(mk_apply(0))
        units.append(mk_apply(4))
        return units

    def fc1_units(lc):
        """FFN up: u = relu(h2 @ W1 + b1f); one unit per 256-wide block."""
        de = state["de"]
        units = []

        def mk(q16):
            def u():
                h2 = state[f"h2{lc}"]
                if f"u{lc}" not in state:
                    state[f"u{lc}"] = de.tile([128, KFF, 512], BF,
                                              tag="u", bufs=1, name="u")
                uu = state[f"u{lc}"]
                w1t = de.tile([128, KD, 256], BF, tag="w1t", bufs=2,
                              name="w1t")
                nc.sync.dma_start(
                    out=w1t, in_=t["w1"][q16 * 128:(q16 + 1) * 128, :, :])
                for fi in range(2):
                    fft = q16 * 2 + fi
                    ps = pps.tile([128, 512], F32, tag="ps_proj", bufs=2,
                                  name="ps_u")
                    for k in range(KD):
                        mm(out=ps,
                           lhsT=w1t[:, k, fi * 128:fi * 128 + 128],
                           rhs=h2[:, k, :],
                           start=(k == 0), stop=(k == KD - 1))
                    if lc == 0:
                        nc.vector.tensor_scalar(
                            out=uu[:, fft, :], in0=ps,
                            scalar1=b1_sb[:, fft:fft + 1], scalar2=0.0,
                            op0=Alu.add, op1=Alu.max)
                    else:
                        nc.scalar.activation(
                            out=uu[:, fft, :], in_=ps, func=Act.Relu,
                            bias=b1_sb[:, fft:fft + 1])
            units.append(u)
        for q16 in range(16):
            mk(q16)
        return units

    def fc2_units(lc):
        """FFN down + bias + residual -> store; W2 streamed per dt."""
        de = state["de"]
        c0 = lc * 512
        units = []

        def mk(dt):
            def u():
                uu = state[f"u{lc}"]
                xmid = state[f"xmid{lc}"]
                w2t = state.pop(f"w2t{dt}", None) if lc == 0 else None
                if w2t is None:
                    w2t = de.tile([128, KFF, 128], BF, tag="w2t", bufs=2,
                                  name="w2t")
                    nc.scalar.dma_start(
                        out=w2t,
                        in_=t["w2"][dt * 128:(dt + 1) * 128, :, :])
                ps = pps.tile([128, 512], F32, tag="ps_proj", bufs=2,
                              name="ps_f")
                for k2 in range(KFF):
                    mm(out=ps, lhsT=w2t[:, k2, :], rhs=uu[:, k2, :],
                       start=(k2 == 0), stop=(k2 == KFF - 1))
                o_f = de.tile([128, 512], F32, tag="o_f", bufs=2,
                              name="o_f")
                nc.vector.scalar_tensor_tensor(
                    out=o_f, in0=ps, scalar=b2_sb[:, dt:dt + 1],
                    in1=xmid[:, dt, :], op0=Alu.add, op1=Alu.add)
                nc.sync.dma_start(
                    out=outT_v[dt * 128:(dt + 1) * 128, c0:c0 + 512],
                    in_=o_f)
            units.append(u)
        for dt in range(KD):
            mk(dt)
        return units

    # ================= emission schedule =================
    # vS zero + softmax-denominator ones columns
    nc.vector.memset(vS[:, 0:8, :], 0.0)
    nc.gpsimd.memset(vS[:, 8:16, :], 0.0)
    for h in range(LH):
        nc.vector.memset(vS[:, :, h * 128 + 64:h * 128 + 65], 1.0)

    xf0 = load_x(0)
    load_qkv_weights()
    xf1 = load_x(1)

    def w2_prefetch(dt):
        def u():
            de = state["de"]
            w2t = de.tile([128, KFF, 128], BF, tag="w2t", bufs=2,
                          name="w2t")
            nc.scalar.dma_start(
                out=w2t, in_=t["w2"][dt * 128:(dt + 1) * 128, :, :])
            state[f"w2t{dt}"] = w2t
        return u

    hT0 = ln1(0, xf0, gps=True)
    for u in proj_units(0, hT0):
        u()
    hT1 = ln1(1, xf1, gps=True)

    oTs = {}
    for ci in range(NCH):
        oTs[ci] = None

    def new_oT():
        return ab.tile([128, KHE, 512], F8, tag="oT", bufs=2, name="oT")

    oTs[0] = new_oT()
    attention(0, oTs[0], list(proj_units(1, hT1)))

    xf2 = load_x(2)
    hT2 = ln1(2, xf2)
    xf3 = load_x(3)
    hT3 = ln1(3, xf3)

    oTs[1] = new_oT()
    attention(1, oTs[1],
              wo_units(0, oTs[0]) + [rs_unit(0)]
              + proj_units(2, hT2) + proj_units(3, hT3))
    pe_pool.release()
    state["de"] = tc.alloc_tile_pool(name="de", bufs=1)

    oTs[3] = new_oT()
    attention(3, oTs[3], wo_units(1, oTs[1]) + [rs_unit(1)])

    oTs[2] = new_oT()
    attention(2, oTs[2],
              wo_units(3, oTs[3]) + [rs_unit(3)]
              + prep_units(0) + [w2_prefetch(0), w2_prefetch(1)]
              + fc1_units(0))

    for u in wo_units(2, oTs[2]):
        u()
    rs_unit(2)()

    # interleave prep(1) with fc2(0), then fc1(1), fc2(1)
    p1 = prep_units(1)
    f20 = fc2_units(0)
    inter = []
    while p1 or f20:
        if f20:
            inter.append(f20.pop(0))
        if p1:
            inter.append(p1.pop(0))
    for u in inter:
        u()
    for u in fc1_units(1):
        u()
    for u in fc2_units(1):
        u()

    state["de"].release()
    ab.release()
    pps.release()
    wlate.release()
    consts.release()
    dram.release()


def _build():
    nc = bacc.Bacc("TRN2", target_bir_lowering=False, debug=False,
                   num_devices=NCORES)

    tensors = {}
    tensors["xT"] = nc.dram_tensor("xT", [NCH * 128, KD, 512], BF,
                                   kind="ExternalInput").ap()
    tensors["xresT"] = nc.dram_tensor("xresT", [TP * 128, KD, 512], BF,
                                      kind="ExternalInput").ap()
    for name, shape, dt in (
        ("wq", [128, KD, 512], F8), ("wk", [128, KD, 512], F8),
        ("wv", [128, KD, 512], F8), ("wo", [128, KHE, D], F8),
        ("w1", [16 * 128, KD, 256], BF), ("w2", [8 * 128, KFF, 128], BF),
        ("bq", [LHE], F32), ("bk", [LHE], F32), ("bv", [LHE], F32),
        ("b1f", [FF], F32), ("bo2", [D], F32), ("b2", [D], F32),
    ):
        tensors[name] = nc.dram_tensor(name, shape, dt,
                                       kind="ExternalInput").ap()
    tensors["outT"] = nc.dram_tensor("out", [D, LT], F32,
                                     kind="ExternalOutput").ap()

    with tile.TileContext(nc, num_cores=NCORES) as tc:
        _emit(nc, tc, tensors)

    nc.compile()
    return nc


_NC_CACHE = None


def _get_nc():
    global _NC_CACHE
    if _NC_CACHE is None:
        _NC_CACHE = _build()
    return _NC_CACHE


def _shard_inputs(x, Wq, Wk, Wv, Wo, bo, W1, b1, W2, b2, g1, be1, g2, be2):
    """Build the 8 per-core input maps (LN gains folded into weights)."""
    bf = lambda a: np.ascontiguousarray(a).astype(BF16NP)
    f8 = lambda a: np.ascontiguousarray(a).astype(FP8NP)
    f32 = lambda a: np.ascontiguousarray(a, dtype=np.float32)

    x = np.asarray(x, dtype=np.float32)
    Wq = np.asarray(Wq, dtype=np.float32)
    Wk = np.asarray(Wk, dtype=np.float32)
    Wv = np.asarray(Wv, dtype=np.float32)
    Wo = np.asarray(Wo, dtype=np.float32)
    W1 = np.asarray(W1, dtype=np.float32)
    W2 = np.asarray(W2, dtype=np.float32)
    g1 = np.asarray(g1, dtype=np.float32)
    be1 = np.asarray(be1, dtype=np.float32)
    g2 = np.asarray(g2, dtype=np.float32)
    be2 = np.asarray(be2, dtype=np.float32)
    b1 = np.asarray(b1, dtype=np.float32)

    scale = float(HS) ** -0.5
    # fold g1 into QKV weights, be1 into QKV biases; fold the score scale
    # into Wq/bq.  Per-head [H, D, HS] -> concat heads -> [D, H*HS].
    wq_f = (g1[None, :, None] * Wq).transpose(1, 0, 2).reshape(D, D) * scale
    wk_f = (g1[None, :, None] * Wk).transpose(1, 0, 2).reshape(D, D)
    wv_f = (g1[None, :, None] * Wv).transpose(1, 0, 2).reshape(D, D)
    bq_f = np.einsum("d,hde->he", be1, Wq).reshape(D) * scale
    bk_f = np.einsum("d,hde->he", be1, Wk).reshape(D)
    bv_f = np.einsum("d,hde->he", be1, Wv).reshape(D)
    # fold g2/be2 into W1/b1
    w1_f = g2[:, None] * W1
    b1_f = b1 + be2 @ W1

    in_maps = []
    for c in range(NCORES):
        b, half = divmod(c, TP)
        hes = slice(half * LHE, (half + 1) * LHE)
        xt = x[b].T
        xres = np.concatenate(
            [xt[:, ci * 512 + half * 256: ci * 512 + half * 256 + 256]
             for ci in range(NCH)], axis=1)
        # partition-major tiled layouts: loads become 128 contiguous
        # segments instead of 1024 scattered ones (descriptor-gen bound)
        xt_sw = xt.reshape(KD, 128, NCH, 512).transpose(2, 1, 0, 3)
        xres_sw = xres.reshape(KD, 128, TP, 512).transpose(2, 1, 0, 3)
        wq_sw = wq_f[:, hes].reshape(KD, 128, LHE).transpose(1, 0, 2)
        wk_sw = wk_f[:, hes].reshape(KD, 128, LHE).transpose(1, 0, 2)
        wv_sw = wv_f[:, hes].reshape(KD, 128, LHE).transpose(1, 0, 2)
        wo_sw = Wo[hes, :].reshape(KHE, 128, D).transpose(1, 0, 2)
        w1_sw = w1_f.reshape(KD, 128, 16, 256).transpose(2, 1, 0, 3)
        # W2 tiled dt-major so fc2 streams contiguous [128, KFF, 128] tiles
        w2_sw = W2.reshape(KFF, 128, KD, 128).transpose(2, 1, 0, 3)
        in_maps.append({
            "xT": bf(xt_sw.reshape(NCH * 128, KD, 512)),
            "xresT": bf(xres_sw.reshape(TP * 128, KD, 512)),
            "wq": f8(wq_sw * SQ), "wk": f8(wk_sw * SK), "wv": f8(wv_sw * SV),
            "bq": f32(bq_f[hes] * SQ), "bk": f32(bk_f[hes] * SK),
            "bv": f32(bv_f[hes]),
            "wo": f8(wo_sw * SO),
            "bo2": f32(np.asarray(bo, dtype=np.float32) * SO / TP),
            "w1": bf(w1_sw.reshape(16 * 128, KD, 256)), "b1f": f32(b1_f),
            "w2": bf(w2_sw.reshape(8 * 128, KFF, 128)),
            "b2": f32(np.asarray(b2, dtype=np.float32)),
        })
    return in_maps


def kernel(x, Wq, Wk, Wv, Wo, bo, W1, b1, W2, b2, g1, be1, g2, be2,
           _trace=False):
    nc = _get_nc()
    in_maps = _shard_inputs(x, Wq, Wk, Wv, Wo, bo, W1, b1, W2, b2,
                            g1, be1, g2, be2)
    res = run_bass_kernel_spmd(nc, in_maps, list(range(NCORES)),
                               trace=_trace)
    out = np.empty((B, T, D), dtype=np.float32)
    for b in range(B):
        for half in range(TP):
            o = res.results[TP * b + half]["out"]  # [D, LT]
            for ci in range(NCH):
                t0 = ci * 512 + half * 256
                out[b, t0:t0 + 256, :] = o[:, ci * 256:(ci + 1) * 256].T
    if _trace:
        kernel.last_exec_time_ns = res.exec_time_ns
        kernel.last_results = res
    return out


# revision 41
# speedup vs baseline: 1.1734x; 1.0466x over previous
"""Trainium2 Bass kernel for a pre-LN transformer block (B=4, T=2048, D=1024,
H=16, HS=64, FF=4096, causal attention).

Sharding: data-parallel over batches x 2-way tensor-parallel attention
(8 heads/core over all T) -> pair ReduceScatter of the attention-output
projection over the sequence dim -> sequence-parallel FFN (full FF width,
T/2 rows per core).  No AllReduce anywhere; each core emits the final
output for its own T/2 rows.

Core c (0..7): batch b = c//2, half = c%2.  half h owns t-slices
[ci*512 + h*256, ci*512 + h*256 + 256) for ci in 0..3.

v2 schedule: attention chunks run 0,1,3,2 so that all four ReduceScatters
except the last are issued mid-attention, and the FFN for the first half
of rows (prep+fc1) is interleaved into the final attention chunk as
pending units -- the tail RS and LN2 hide behind fc1/fc2 matmuls.
Attention-path tensors (weights, hT, kT, qT, oT) are fp8e4m3 with
power-of-2 scales folded into the projection epilogues; this halves
their SBUF/DMA cost (matmuls run at bf16 rate).  LN1 stats come from
M=1 PE matmuls instead of DVE adder trees.  Diagonal score tiles are
N-trimmed and their exp is windowed, with the causal mask applied only
to the [128,128] band.  W1/W2 stream through double-buffered tiles.
"""

import numpy as np
import ml_dtypes

import concourse.bacc as bacc
import concourse.bass as bass
import concourse.mybir as mybir
import concourse.tile as tile
from concourse.bass_utils import run_bass_kernel_spmd

BF16NP = ml_dtypes.bfloat16
FP8NP = ml_dtypes.float8_e4m3

B, T, D, H, HS, FF = 4, 2048, 1024, 16, 64, 4096
EPS = 1e-5
NCORES = 8
TP = 2
LH = H // TP          # 8 local heads
LHE = LH * HS         # 512 local head-embed width
LT = T // TP          # 1024 local rows (FFN/output)
KD = D // 128         # 8 d k-tiles
KHE = LHE // 128      # 4 he k-tiles
KFF = FF // 128       # 32 ff tiles
NCH = T // 512        # 4 t-chunks of 512
NST = T // 128        # 16 s-tiles of 128
PAIRS = [[0, 1], [2, 3], [4, 5], [6, 7]]
OA_LAG = 2            # psc tiles in flight between scores and o-accum

F32 = mybir.dt.float32
BF = mybir.dt.bfloat16
F8 = mybir.dt.float8e4
F85 = mybir.dt.float8e5
DRM = mybir.MatmulPerfMode.DoubleRow
# fp8 weight scales (power-of-2, folded out in the epilogues)
SQ = 256.0   # wq carries g1 and HS^-0.5 -> sigma 1/256
SK = 32.0
SV = 32.0
SO = 32.0


def _ln_math(nc, pool, ps_s, ps_q):
    """From psum row-sums (ps_s, ps_q over D) to bf16 broadcast tiles
    (Ab, Bb) so that xn = x*Ab + Bb."""
    Alu = mybir.AluOpType
    Act = mybir.ActivationFunctionType
    m = pool.tile([1, 512], F32, tag="ln_m", bufs=1, name="ln_m")
    e2 = pool.tile([1, 512], F32, tag="ln_e2", bufs=1, name="ln_e2")
    nc.vector.tensor_scalar_mul(out=m, in0=ps_s, scalar1=1.0 / D)
    nc.vector.tensor_scalar_mul(out=e2, in0=ps_q, scalar1=1.0 / D)
    msq = pool.tile([1, 512], F32, tag="ln_msq", bufs=1, name="ln_msq")
    nc.vector.tensor_mul(out=msq, in0=m, in1=m)
    var = pool.tile([1, 512], F32, tag="ln_var", bufs=1, name="ln_var")
    nc.vector.scalar_tensor_tensor(out=var, in0=e2, scalar=EPS, in1=msq,
                                   op0=Alu.add, op1=Alu.subtract)
    sd = pool.tile([1, 512], F32, tag="ln_sd", bufs=1, name="ln_sd")
    nc.scalar.activation(out=sd, in_=var, func=Act.Sqrt)
    a_row = pool.tile([1, 512], F32, tag="ln_a", bufs=1, name="ln_a")
    nc.vector.reciprocal_approx_fast(out=a_row, in_=sd)
    b_row = pool.tile([1, 512], F32, tag="ln_b", bufs=1, name="ln_b")
    nc.vector.scalar_tensor_tensor(out=b_row, in0=m, scalar=-1.0, in1=a_row,
                                   op0=Alu.mult, op1=Alu.mult)
    ac = pool.tile([1, 512], BF, tag="ln_ac", bufs=1, name="ln_ac")
    bc = pool.tile([1, 512], BF, tag="ln_bc", bufs=1, name="ln_bc")
    nc.vector.tensor_copy(out=ac, in_=a_row)
    nc.vector.tensor_copy(out=bc, in_=b_row)
    Ab = pool.tile([128, 512], BF, tag="ln_Ab", bufs=2, name="ln_Ab")
    Bb = pool.tile([128, 512], BF, tag="ln_Bb", bufs=2, name="ln_Bb")
    nc.gpsimd.partition_broadcast(Ab, ac)
    nc.gpsimd.partition_broadcast(Bb, bc)
    return Ab, Bb


def _ln_stats_pe(nc, pool, psum_pool, psum_tag, src, ones_col):
    """LN stats via M=1 PE matmuls: sum(x) directly on the x k-tiles,
    sum(x^2) on DVE-squared tiles.  src: [128, KD, 512] bf16."""
    mm = nc.tensor.matmul
    ps_s = psum_pool.tile([1, 512], F32, tag=psum_tag, bufs=2, name="ps_s")
    ps_q = psum_pool.tile([1, 512], F32, tag=psum_tag, bufs=2, name="ps_q")
    for k in range(KD):
        mm(out=ps_s, lhsT=ones_col, rhs=src[:, k, :],
           start=(k == 0), stop=(k == KD - 1))
    for k in range(KD):
        sq = pool.tile([128, 512], BF, tag="ln_sq", bufs=2, name="ln_sq")
        nc.vector.tensor_mul(out=sq, in0=src[:, k, :], in1=src[:, k, :])
        mm(out=ps_q, lhsT=ones_col, rhs=sq,
           start=(k == 0), stop=(k == KD - 1))
    return _ln_math(nc, pool, ps_s, ps_q)


def _ln_stats_tree(nc, pool, psum_pool, psum_tag, src, ones_col):
    """DVE adder-tree LN stats (kept for LN2 where the PE is contended).
    src: [128, KD, 512] AP."""
    mm = nc.tensor.matmul

    def lvl(tg, n):
        return pool.tile([128, 512], BF, tag=f"{tg}{n}", bufs=2, name=tg)

    s2, q2 = [], []
    for i in range(4):
        s = lvl("lts", 2)
        nc.vector.tensor_add(out=s, in0=src[:, 2 * i, :],
                             in1=src[:, 2 * i + 1, :])
        s2.append(s)
        sqa = pool.tile([128, 512], BF, tag="ln_sq", bufs=2, name="ln_sq")
        sqb = pool.tile([128, 512], BF, tag="ln_sq", bufs=2, name="ln_sq")
        nc.vector.tensor_mul(out=sqa, in0=src[:, 2 * i, :],
                             in1=src[:, 2 * i, :])
        nc.vector.tensor_mul(out=sqb, in0=src[:, 2 * i + 1, :],
                             in1=src[:, 2 * i + 1, :])
        q = lvl("ltq", 2)
        nc.vector.tensor_add(out=q, in0=sqa, in1=sqb)
        q2.append(q)
    s4, q4_ = [], []
    for i in range(2):
        s = lvl("lts", 4)
        nc.vector.tensor_add(out=s, in0=s2[2 * i], in1=s2[2 * i + 1])
        s4.append(s)
        q = lvl("ltq", 4)
        nc.vector.tensor_add(out=q, in0=q2[2 * i], in1=q2[2 * i + 1])
        q4_.append(q)
    s_all = lvl("lts", 8)
    nc.vector.tensor_add(out=s_all, in0=s4[0], in1=s4[1])
    q_all = lvl("ltq", 8)
    nc.vector.tensor_add(out=q_all, in0=q4_[0], in1=q4_[1])

    ps_s = psum_pool.tile([1, 512], F32, tag=psum_tag, bufs=2, name="ps_s")
    ps_q = psum_pool.tile([1, 512], F32, tag=psum_tag, bufs=2, name="ps_q")
    mm(out=ps_s, lhsT=ones_col, rhs=s_all, start=True, stop=True)
    mm(out=ps_q, lhsT=ones_col, rhs=q_all, start=True, stop=True)
    return _ln_math(nc, pool, ps_s, ps_q)


def _ln_apply(nc, pool, src_k, Ab, Bb, out_slice, eng=None, tag="ln_t1"):
    """out = src*Ab + Bb."""
    eng = eng or nc.vector
    t1 = pool.tile([128, 512], BF, tag=tag, bufs=2, name="ln_t1")
    eng.tensor_mul(out=t1, in0=src_k, in1=Ab)
    eng.tensor_add(out=out_slice, in0=t1, in1=Bb)


def _emit(nc, tc, t):
    mm = nc.tensor.matmul
    Alu = mybir.AluOpType
    Act = mybir.ActivationFunctionType

    outT_v = t["outT"]

    # ---------------- persistent pools ----------------
    dram = tc.alloc_tile_pool(name="dram", bufs=1, space="DRAM")
    rs_in = [dram.tile([TP, 128, KD, 256], BF, name=f"rsi{c}")
             for c in range(NCH)]
    rs_out = [dram.tile([128, KD, 256], BF, name=f"rso{c}")
              for c in range(NCH)]

    consts = tc.alloc_tile_pool(name="consts", bufs=1)
    ones_col = consts.tile([128, 1], BF)
    nc.vector.memset(ones_col, 1.0)

    bq_sb = consts.tile([128, KHE], F32)
    bk_sb = consts.tile([128, KHE], F32)
    bo2_sb = consts.tile([128, KD], F32)
    b2_sb = consts.tile([128, KD], F32)
    b1_sb = consts.tile([128, KFF], F32)
    for name, dst in (("bq", bq_sb), ("bk", bk_sb),
                      ("bo2", bo2_sb), ("b2", b2_sb)):
        nc.sync.dma_start(out=dst, in_=t[name].rearrange("(k p) -> p k", p=128))
    nc.sync.dma_start(out=b1_sb, in_=t["b1f"].rearrange("(k p) -> p k", p=128))
    # v bias broadcast over all partitions: [128, LHE]
    bvb = consts.tile([128, LHE], BF)
    bv_row = consts.tile([1, LHE], BF)
    nc.gpsimd.dma_start(out=bv_row,
                        in_=t["bv"].rearrange("(o e) -> o e", o=1))
    nc.gpsimd.partition_broadcast(bvb, bv_row)

    wlate = tc.alloc_tile_pool(name="wlate", bufs=1)
    wo_sb = wlate.tile([128, KHE, D], F8, tag="wo")
    nc.scalar.dma_start(out=wo_sb, in_=t["wo"])

    # single PSUM pool: ps_proj(2) + ps_sc(2x2) + po(2) = 8 banks
    pps = tc.alloc_tile_pool(name="pps", bufs=2, space="PSUM")

    # stage A long-lived pool
    ab = tc.alloc_tile_pool(name="abc", bufs=1)
    kT = ab.tile([128, LH // 2, T], F8, tag="kT")
    qT = ab.tile([128, LH // 2, T], F8, tag="qT")
    vS = ab.tile([128, NST, LH * 128], BF, tag="vS")
    # early pool: x tiles, hT, qkv weights, LN1 temps (freed before FFN)
    pe_pool = tc.alloc_tile_pool(name="pearly", bufs=1)

    wq_sb = pe_pool.tile([128, KD, LHE], F8, tag="wq")
    wk_sb = pe_pool.tile([128, KD, LHE], F8, tag="wk")
    wv_sb = pe_pool.tile([128, KD, LHE], F8, tag="wv")

    def load_x(ci):
        xf = pe_pool.tile([128, KD, 512], BF, tag="xf", bufs=2, name="xf")
        engs = (nc.sync, nc.scalar, nc.gpsimd)
        for k in range(KD):
            engs[k % 3].dma_start(out=xf[:, k:k + 1, :],
                                  in_=t["xT"][ci * 128:ci * 128 + 128,
                                              k:k + 1, :])
        return xf

    def load_qkv_weights():
        for eng, srct, dst in ((nc.scalar, t["wq"], wq_sb),
                               (nc.scalar, t["wk"], wk_sb),
                               (nc.sync, t["wv"], wv_sb)):
            eng.dma_start(out=dst, in_=srct)

    def ln1(ci, xf, gps=False):
        Ab, Bb = _ln_stats_pe(nc, pe_pool, pps, "ps_proj", xf, ones_col)
        hT = pe_pool.tile([128, KD, 512], F8, tag="hT", bufs=2, name="hT")
        for k in range(KD):
            if gps and k % 2 == 1:
                _ln_apply(nc, pe_pool, xf[:, k, :], Ab, Bb, hT[:, k, :],
                          eng=nc.gpsimd, tag="ln_t1g")
            else:
                _ln_apply(nc, pe_pool, xf[:, k, :], Ab, Bb, hT[:, k, :])
        return hT

    def proj_units(ci, hT):
        c0 = ci * 512
        units = []
        for w_sb, dst, bias, rsc in ((wk_sb, kT, bk_sb, 1.0 / SK),
                                     (wq_sb, qT, bq_sb, 1.0 / SQ)):
            for et in range(LH // 2):
                def u(w_sb=w_sb, dst=dst, bias=bias, rsc=rsc, et=et,
                      hT=hT, c0=c0):
                    ps = pps.tile([128, 512], F32, tag="ps_proj",
                                  bufs=2, name="ps_proj")
                    for k in range(KD):
                        mm(out=ps,
                           lhsT=w_sb[:, k, et * 128:(et + 1) * 128],
                           rhs=hT[:, k, :],
                           start=(k == 0), stop=(k == KD - 1))
                    nc.vector.tensor_scalar(
                        out=dst[:, et, c0:c0 + 512], in0=ps,
                        scalar1=bias[:, et:et + 1], scalar2=rsc,
                        op0=Alu.add, op1=Alu.mult)
                units.append(u)
        for sti in range(4):
            st = ci * 4 + sti
            def u(sti=sti, st=st, hT=hT):
                ps = pps.tile([128, LHE], F32, tag="ps_proj", bufs=2,
                              name="ps_v")
                for k in range(KD):
                    mm(out=ps,
                       lhsT=hT[:, k, sti * 128:sti * 128 + 128],
                       rhs=wv_sb[:, k, :],
                       start=(k == 0), stop=(k == KD - 1))
                nc.vector.scalar_tensor_tensor(
                    out=vS[:, st, :].rearrange("p (h e) -> p h e",
                                               h=LH)[:, :, 0:64],
                    in0=ps.rearrange("p (h e) -> p h e", e=64),
                    scalar=1.0 / SV, op0=Alu.mult,
                    in1=bvb.rearrange("p (h e) -> p h e", e=64),
                    op1=Alu.add)
            units.append(u)
        return units

    def attention(ci, oT, pending):
        c0 = ci * 512
        nb = 4 * (ci + 1)
        total_steps = (LH // 2) * (nb + OA_LAG)
        spacing = (max(1, total_steps // len(pending))
                   if pending else 0)
        stepctr = 0
        for hp in range(LH // 2):
            po = [pps.tile([128, 512], F32, tag="po", bufs=2, name="po")
                  for _ in range(2)]
            exs = [None] * nb

            def scores(sb):
                s0 = sb * 128
                midx = sb - 4 * ci
                w0 = max(0, midx) * 128
                ps2 = pps.tile([128, 2, 512], F32, tag="ps_sc",
                               bufs=OA_LAG, name="ps_sc")
                for hi in range(2):
                    mm(out=ps2[:, hi, w0:512],
                       lhsT=kT[hi * 64:hi * 64 + 64, hp, s0:s0 + 128],
                       rhs=qT[hi * 64:hi * 64 + 64, hp,
                              c0 + w0:c0 + 512],
                       start=True, stop=True)
                ex = ab.tile([128, 2, 512], BF, tag="ex",
                             bufs=OA_LAG + 1, name="ex")
                if w0 > 0:
                    nc.gpsimd.memset(ex[:, :, 0:w0], 0.0)
                nc.scalar.activation(out=ex[:, :, w0:512],
                                     in_=ps2[:, :, w0:512], func=Act.Exp)
                if midx >= 0:
                    for hi in range(2):
                        nc.gpsimd.affine_select(
                            out=ex[:, hi, w0:w0 + 128],
                            in_=ex[:, hi, w0:w0 + 128],
                            compare_op=Alu.is_ge, fill=0.0,
                            base=0, channel_multiplier=-1,
                            pattern=[[1, 128]])
                exs[sb] = ex

            def oacc(sb):
                for hi in range(2):
                    h_loc = hp * 2 + hi
                    mm(out=po[hi],
                       lhsT=vS[:, sb, h_loc * 128:h_loc * 128 + 128],
                       rhs=exs[sb][:, hi, :],
                       start=(sb == 0), stop=(sb == nb - 1))

            for step in range(nb + OA_LAG):
                if step < nb:
                    scores(step)
                if step >= OA_LAG:
                    oacc(step - OA_LAG)
                stepctr += 1
                if pending and stepctr % spacing == 0:
                    pending.pop(0)()

            for hi in range(2):
                h_loc = hp * 2 + hi
                dnr = ab.tile([1, 512], F32, tag="dnr", bufs=1,
                              name="dnr")
                nc.vector.tensor_copy(out=dnr, in_=po[hi][64:65, :])
                rcp = ab.tile([1, 512], F32, tag="rcp", bufs=2,
                              name="rcp")
                nc.vector.reciprocal_approx_fast(out=rcp, in_=dnr)
                bc = ab.tile([64, 512], F32, tag="bc", bufs=1,
                             name="bc")
                nc.gpsimd.partition_broadcast(bc, rcp)
                nc.vector.tensor_mul(
                    out=oT[hi * 64:hi * 64 + 64, hp, :],
                    in0=po[hi][0:64, :], in1=bc)
        while pending:
            pending.pop(0)()

    def wo_units(ci, oT):
        rsv = rs_in[ci]
        units = []
        for dt in range(KD):
            def u(dt=dt, oT=oT, rsv=rsv):
                ps = pps.tile([128, 512], F32, tag="ps_proj",
                              bufs=2, name="ps_wo")
                for k in range(KHE):
                    mm(out=ps,
                       lhsT=wo_sb[:, k, dt * 128:(dt + 1) * 128],
                       rhs=oT[:, k, :],
                       start=(k == 0), stop=(k == KHE - 1))
                stg = ab.tile([128, 512], BF, tag="stg1", bufs=2,
                              name="stg1")
                nc.vector.tensor_scalar(
                    out=stg, in0=ps, scalar1=bo2_sb[:, dt:dt + 1],
                    scalar2=1.0 / SO, op0=Alu.add, op1=Alu.mult)
                for j in range(TP):
                    nc.sync.dma_start(
                        out=rsv[j, :, dt, :],
                        in_=stg[:, j * 256:(j + 1) * 256])
            units.append(u)
        return units

    def rs_unit(ci):
        def u():
            nc.gpsimd.collective_compute(
                "ReduceScatter", Alu.add, replica_groups=PAIRS,
                ins=[rs_in[ci].opt()], outs=[rs_out[ci].opt()])
        return u

    # ---------------- stage B (FFN) units (pool allocated later) --------
    state = {}

    def prep_units(lc):
        """residual + LN2 + apply, decomposed into pending units."""
        de = state["de"]
        units = []

        def u_load():
            xmid = de.tile([128, KD, 512], BF, tag="xmid", bufs=2,
                           name="xmid")
            xrs = t["xresT"][lc * 128:lc * 128 + 128, :, :]
            nc.sync.dma_start(out=xmid[:, 0:4, :], in_=xrs[:, 0:4, :])
            nc.scalar.dma_start(out=xmid[:, 4:8, :], in_=xrs[:, 4:8, :])
            arr = de.tile([128, KD, 2, 256], BF, tag="arr", bufs=1,
                          name="arr")
            nc.sync.dma_start(out=arr[:, 0:4, 0, :],
                              in_=rs_out[2 * lc][:, 0:4, :])
            nc.scalar.dma_start(out=arr[:, 4:8, 0, :],
                                in_=rs_out[2 * lc][:, 4:8, :])
            nc.sync.dma_start(out=arr[:, 0:4, 1, :],
                              in_=rs_out[2 * lc + 1][:, 0:4, :])
            nc.scalar.dma_start(out=arr[:, 4:8, 1, :],
                                in_=rs_out[2 * lc + 1][:, 4:8, :])
            state[f"xmid{lc}"] = xmid
            state[f"arr{lc}"] = arr
        units.append(u_load)

        def u_add():
            xmid = state[f"xmid{lc}"]
            arr = state[f"arr{lc}"]
            for k in range(KD):
                nc.vector.tensor_add(
                    out=xmid[:, k, :], in0=xmid[:, k, :],
                    in1=arr[:, k, :, :].rearrange("p j t -> p (j t)"))
        units.append(u_add)

        def u_stats():
            xmid = state[f"xmid{lc}"]
            state[f"ab{lc}"] = _ln_stats_pe(nc, de, pps, "ps_proj",
                                            xmid, ones_col)
        units.append(u_stats)

        def mk_apply(k0):
            def u_apply():
                xmid = state[f"xmid{lc}"]
                Ab2, Bb2 = state[f"ab{lc}"]
                if f"h2{lc}" not in state:
                    state[f"h2{lc}"] = de.tile([128, KD, 512], BF,
                                               tag="h2", bufs=1, name="h2")
                h2 = state[f"h2{lc}"]
                for k in range(k0, k0 + 4):
                    _ln_apply(nc, de, xmid[:, k, :], Ab2, Bb2, h2[:, k, :])
            return u_apply
        units.append(mk_apply(0))
        units.append(mk_apply(4))
        return units

    def fc1_units(lc):
        """FFN up: u = relu(h2 @ W1 + b1f); one unit per 256-wide block."""
        de = state["de"]
        units = []

        def mk(q16):
            def u():
                h2 = state[f"h2{lc}"]
                if f"u{lc}" not in state:
                    state[f"u{lc}"] = de.tile([128, KFF, 512], BF,
                                              tag="u", bufs=1, name="u")
                uu = state[f"u{lc}"]
                w1t = de.tile([128, KD, 256], BF, tag="w1t", bufs=2,
                              name="w1t")
                nc.sync.dma_start(
                    out=w1t, in_=t["w1"][q16 * 128:(q16 + 1) * 128, :, :])
                for fi in range(2):
                    fft = q16 * 2 + fi
                    ps = pps.tile([128, 512], F32, tag="ps_proj", bufs=2,
                                  name="ps_u")
                    for k in range(KD):
                        mm(out=ps,
                           lhsT=w1t[:, k, fi * 128:fi * 128 + 128],
                           rhs=h2[:, k, :],
                           start=(k == 0), stop=(k == KD - 1))
                    if lc == 0:
                        nc.vector.tensor_scalar(
                            out=uu[:, fft, :], in0=ps,
                            scalar1=b1_sb[:, fft:fft + 1], scalar2=0.0,
                            op0=Alu.add, op1=Alu.max)
                    else:
                        nc.scalar.activation(
                            out=uu[:, fft, :], in_=ps, func=Act.Relu,
                            bias=b1_sb[:, fft:fft + 1])
            units.append(u)
        for q16 in range(16):
            mk(q16)
        return units

    def fc2_units(lc):
        """FFN down + bias + residual -> store; W2 streamed per dt."""
        de = state["de"]
        c0 = lc * 512
        units = []

        def mk(dt):
            def u():
                uu = state[f"u{lc}"]
                xmid = state[f"xmid{lc}"]
                w2t = state.pop(f"w2t{dt}", None) if lc == 0 else None
                if w2t is None:
                    w2t = de.tile([128, KFF, 128], BF, tag="w2t", bufs=2,
                                  name="w2t")
                    nc.scalar.dma_start(
                        out=w2t,
                        in_=t["w2"][dt * 128:(dt + 1) * 128, :, :])
                ps = pps.tile([128, 512], F32, tag="ps_proj", bufs=2,
                              name="ps_f")
                for k2 in range(KFF):
                    mm(out=ps, lhsT=w2t[:, k2, :], rhs=uu[:, k2, :],
                       start=(k2 == 0), stop=(k2 == KFF - 1))
                o_f = de.tile([128, 512], F32, tag="o_f", bufs=2,
                              name="o_f")
                nc.vector.scalar_tensor_tensor(
                    out=o_f, in0=ps, scalar=b2_sb[:, dt:dt + 1],
                    in1=xmid[:, dt, :], op0=Alu.add, op1=Alu.add)
                nc.sync.dma_start(
                    out=outT_v[dt * 128:(dt + 1) * 128, c0:c0 + 512],
                    in_=o_f)
            units.append(u)
        for dt in range(KD):
            mk(dt)
        return units

    # ================= emission schedule =================
    # vS zero + softmax-denominator ones columns
    nc.vector.memset(vS[:, 0:8, :], 0.0)
    nc.gpsimd.memset(vS[:, 8:16, :], 0.0)
    for h in range(LH):
        nc.vector.memset(vS[:, :, h * 128 + 64:h * 128 + 65], 1.0)

    xf0 = load_x(0)
    load_qkv_weights()
    xf1 = load_x(1)

    def w2_prefetch(dt):
        def u():
            de = state["de"]
            w2t = de.tile([128, KFF, 128], BF, tag="w2t", bufs=2,
                          name="w2t")
            nc.scalar.dma_start(
                out=w2t, in_=t["w2"][dt * 128:(dt + 1) * 128, :, :])
            state[f"w2t{dt}"] = w2t
        return u

    hT0 = ln1(0, xf0)
    for u in proj_units(0, hT0):
        u()
    hT1 = ln1(1, xf1)

    oTs = {}
    for ci in range(NCH):
        oTs[ci] = None

    def new_oT():
        return ab.tile([128, KHE, 512], F8, tag="oT", bufs=2, name="oT")

    oTs[0] = new_oT()
    attention(0, oTs[0], list(proj_units(1, hT1)))

    xf2 = load_x(2)
    hT2 = ln1(2, xf2)
    xf3 = load_x(3)
    hT3 = ln1(3, xf3)

    oTs[1] = new_oT()
    attention(1, oTs[1],
              wo_units(0, oTs[0]) + [rs_unit(0)]
              + proj_units(2, hT2) + proj_units(3, hT3))
    pe_pool.release()
    state["de"] = tc.alloc_tile_pool(name="de", bufs=1)

    oTs[3] = new_oT()
    attention(3, oTs[3], wo_units(1, oTs[1]) + [rs_unit(1)])

    oTs[2] = new_oT()
    attention(2, oTs[2],
              wo_units(3, oTs[3]) + [rs_unit(3)]
              + prep_units(0) + [w2_prefetch(0), w2_prefetch(1)]
              + fc1_units(0))

    for u in wo_units(2, oTs[2]):
        u()
    rs_unit(2)()

    # interleave prep(1) with fc2(0), then fc1(1), fc2(1)
    p1 = prep_units(1)
    f20 = fc2_units(0)
    inter = []
    while p1 or f20:
        if f20:
            inter.append(f20.pop(0))
        if p1:
            inter.append(p1.pop(0))
    for u in inter:
        u()
    for u in fc1_units(1):
        u()
    for u in fc2_units(1):
        u()

    state["de"].release()
    ab.release()
    pps.release()
    wlate.release()
    consts.release()
    dram.release()


def _build():
    nc = bacc.Bacc("TRN2", target_bir_lowering=False, debug=False,
                   num_devices=NCORES)

    tensors = {}
    tensors["xT"] = nc.dram_tensor("xT", [NCH * 128, KD, 512], BF,
                                   kind="ExternalInput").ap()
    tensors["xresT"] = nc.dram_tensor("xresT", [TP * 128, KD, 512], BF,
                                      kind="ExternalInput").ap()
    for name, shape, dt in (
        ("wq", [128, KD, 512], F8), ("wk", [128, KD, 512], F8),
        ("wv", [128, KD, 512], F8), ("wo", [128, KHE, D], F8),
        ("w1", [16 * 128, KD, 256], BF), ("w2", [8 * 128, KFF, 128], BF),
        ("bq", [LHE], F32), ("bk", [LHE], F32), ("bv", [LHE], F32),
        ("b1f", [FF], F32), ("bo2", [D], F32), ("b2", [D], F32),
    ):
        tensors[name] = nc.dram_tensor(name, shape, dt,
                                       kind="ExternalInput").ap()
    tensors["outT"] = nc.dram_tensor("out", [D, LT], F32,
                                     kind="ExternalOutput").ap()

    with tile.TileContext(nc, num_cores=NCORES) as tc:
        _emit(nc, tc, tensors)

    nc.compile()
    return nc


_NC_CACHE = None


def _get_nc():
    global _NC_CACHE
    if _NC_CACHE is None:
        _NC_CACHE = _build()
    return _NC_CACHE


def _shard_inputs(x, Wq, Wk, Wv, Wo, bo, W1, b1, W2, b2, g1, be1, g2, be2):
    """Build the 8 per-core input maps (LN gains folded into weights)."""
    bf = lambda a: np.ascontiguousarray(a).astype(BF16NP)
    f8 = lambda a: np.ascontiguousarray(a).astype(FP8NP)
    f32 = lambda a: np.ascontiguousarray(a, dtype=np.float32)

    x = np.asarray(x, dtype=np.float32)
    Wq = np.asarray(Wq, dtype=np.float32)
    Wk = np.asarray(Wk, dtype=np.float32)
    Wv = np.asarray(Wv, dtype=np.float32)
    Wo = np.asarray(Wo, dtype=np.float32)
    W1 = np.asarray(W1, dtype=np.float32)
    W2 = np.asarray(W2, dtype=np.float32)
    g1 = np.asarray(g1, dtype=np.float32)
    be1 = np.asarray(be1, dtype=np.float32)
    g2 = np.asarray(g2, dtype=np.float32)
    be2 = np.asarray(be2, dtype=np.float32)
    b1 = np.asarray(b1, dtype=np.float32)

    scale = float(HS) ** -0.5
    # fold g1 into QKV weights, be1 into QKV biases; fold the score scale
    # into Wq/bq.  Per-head [H, D, HS] -> concat heads -> [D, H*HS].
    wq_f = (g1[None, :, None] * Wq).transpose(1, 0, 2).reshape(D, D) * scale
    wk_f = (g1[None, :, None] * Wk).transpose(1, 0, 2).reshape(D, D)
    wv_f = (g1[None, :, None] * Wv).transpose(1, 0, 2).reshape(D, D)
    bq_f = np.einsum("d,hde->he", be1, Wq).reshape(D) * scale
    bk_f = np.einsum("d,hde->he", be1, Wk).reshape(D)
    bv_f = np.einsum("d,hde->he", be1, Wv).reshape(D)
    # fold g2/be2 into W1/b1
    w1_f = g2[:, None] * W1
    b1_f = b1 + be2 @ W1

    in_maps = []
    for c in range(NCORES):
        b, half = divmod(c, TP)
        hes = slice(half * LHE, (half + 1) * LHE)
        xt = x[b].T
        xres = np.concatenate(
            [xt[:, ci * 512 + half * 256: ci * 512 + half * 256 + 256]
             for ci in range(NCH)], axis=1)
        # partition-major tiled layouts: loads become 128 contiguous
        # segments instead of 1024 scattered ones (descriptor-gen bound)
        xt_sw = xt.reshape(KD, 128, NCH, 512).transpose(2, 1, 0, 3)
        xres_sw = xres.reshape(KD, 128, TP, 512).transpose(2, 1, 0, 3)
        wq_sw = wq_f[:, hes].reshape(KD, 128, LHE).transpose(1, 0, 2)
        wk_sw = wk_f[:, hes].reshape(KD, 128, LHE).transpose(1, 0, 2)
        wv_sw = wv_f[:, hes].reshape(KD, 128, LHE).transpose(1, 0, 2)
        wo_sw = Wo[hes, :].reshape(KHE, 128, D).transpose(1, 0, 2)
        w1_sw = w1_f.reshape(KD, 128, 16, 256).transpose(2, 1, 0, 3)
        # W2 tiled dt-major so fc2 streams contiguous [128, KFF, 128] tiles
        w2_sw = W2.reshape(KFF, 128, KD, 128).transpose(2, 1, 0, 3)
        in_maps.append({
            "xT": bf(xt_sw.reshape(NCH * 128, KD, 512)),
            "xresT": bf(xres_sw.reshape(TP * 128, KD, 512)),
            "wq": f8(wq_sw * SQ), "wk": f8(wk_sw * SK), "wv": f8(wv_sw * SV),
            "bq": f32(bq_f[hes] * SQ), "bk": f32(bk_f[hes] * SK),
            "bv": f32(bv_f[hes]),
            "wo": f8(wo_sw * SO),
            "bo2": f32(np.asarray(bo, dtype=np.float32) * SO / TP),
            "w1": bf(w1_sw.reshape(16 * 128, KD, 256)), "b1f": f32(b1_f),
            "w2": bf(w2_sw.reshape(8 * 128, KFF, 128)),
            "b2": f32(np.asarray(b2, dtype=np.float32)),
        })
    return in_maps


def kernel(x, Wq, Wk, Wv, Wo, bo, W1, b1, W2, b2, g1, be1, g2, be2,
           _trace=False):
    nc = _get_nc()
    in_maps = _shard_inputs(x, Wq, Wk, Wv, Wo, bo, W1, b1, W2, b2,
                            g1, be1, g2, be2)
    res = run_bass_kernel_spmd(nc, in_maps, list(range(NCORES)),
                               trace=_trace)
    out = np.empty((B, T, D), dtype=np.float32)
    for b in range(B):
        for half in range(TP):
            o = res.results[TP * b + half]["out"]  # [D, LT]
            for ci in range(NCH):
                t0 = ci * 512 + half * 256
                out[b, t0:t0 + 256, :] = o[:, ci * 256:(ci + 1) * 256].T
    if _trace:
        kernel.last_exec_time_ns = res.exec_time_ns
        kernel.last_results = res
    return out


# revision 47
# speedup vs baseline: 1.1823x; 1.0075x over previous
"""Trainium2 Bass kernel for a pre-LN transformer block (B=4, T=2048, D=1024,
H=16, HS=64, FF=4096, causal attention).

Sharding: data-parallel over batches x 2-way tensor-parallel attention
(8 heads/core over all T) -> pair ReduceScatter of the attention-output
projection over the sequence dim -> sequence-parallel FFN (full FF width,
T/2 rows per core).  No AllReduce anywhere; each core emits the final
output for its own T/2 rows.

Core c (0..7): batch b = c//2, half = c%2.  half h owns t-slices
[ci*512 + h*256, ci*512 + h*256 + 256) for ci in 0..3.

v2 schedule: attention chunks run 0,1,3,2 so that all four ReduceScatters
except the last are issued mid-attention, and the FFN for the first half
of rows (prep+fc1) is interleaved into the final attention chunk as
pending units -- the tail RS and LN2 hide behind fc1/fc2 matmuls.
Attention-path tensors (weights, hT, kT, qT, oT) are fp8e4m3 with
power-of-2 scales folded into the projection epilogues; this halves
their SBUF/DMA cost (matmuls run at bf16 rate).  LN1 stats come from
M=1 PE matmuls instead of DVE adder trees.  Diagonal score tiles are
N-trimmed and their exp is windowed, with the causal mask applied only
to the [128,128] band.  W1/W2 stream through double-buffered tiles.
"""

import numpy as np
import ml_dtypes

import concourse.bacc as bacc
import concourse.bass as bass
import concourse.mybir as mybir
import concourse.tile as tile
from concourse.bass_utils import run_bass_kernel_spmd

BF16NP = ml_dtypes.bfloat16
FP8NP = ml_dtypes.float8_e4m3

B, T, D, H, HS, FF = 4, 2048, 1024, 16, 64, 4096
EPS = 1e-5
NCORES = 8
TP = 2
LH = H // TP          # 8 local heads
LHE = LH * HS         # 512 local head-embed width
LT = T // TP          # 1024 local rows (FFN/output)
KD = D // 128         # 8 d k-tiles
KHE = LHE // 128      # 4 he k-tiles
KFF = FF // 128       # 32 ff tiles
NCH = T // 512        # 4 t-chunks of 512
NST = T // 128        # 16 s-tiles of 128
PAIRS = [[0, 1], [2, 3], [4, 5], [6, 7]]
OA_LAG = 2            # psc tiles in flight between scores and o-accum

F32 = mybir.dt.float32
BF = mybir.dt.bfloat16
F8 = mybir.dt.float8e4
F85 = mybir.dt.float8e5
DRM = mybir.MatmulPerfMode.DoubleRow
# fp8 weight scales (power-of-2, folded out in the epilogues)
SQ = 256.0   # wq carries g1 and HS^-0.5 -> sigma 1/256
SK = 32.0
SV = 32.0
SO = 32.0


def _ln_math(nc, pool, ps_s, ps_q):
    """From psum row-sums (ps_s, ps_q over D) to bf16 broadcast tiles
    (Ab, Bb) so that xn = x*Ab + Bb."""
    Alu = mybir.AluOpType
    Act = mybir.ActivationFunctionType
    m = pool.tile([1, 512], F32, tag="ln_m", bufs=1, name="ln_m")
    e2 = pool.tile([1, 512], F32, tag="ln_e2", bufs=1, name="ln_e2")
    nc.vector.tensor_scalar_mul(out=m, in0=ps_s, scalar1=1.0 / D)
    nc.vector.tensor_scalar_mul(out=e2, in0=ps_q, scalar1=1.0 / D)
    msq = pool.tile([1, 512], F32, tag="ln_msq", bufs=1, name="ln_msq")
    nc.vector.tensor_mul(out=msq, in0=m, in1=m)
    var = pool.tile([1, 512], F32, tag="ln_var", bufs=1, name="ln_var")
    nc.vector.scalar_tensor_tensor(out=var, in0=e2, scalar=EPS, in1=msq,
                                   op0=Alu.add, op1=Alu.subtract)
    sd = pool.tile([1, 512], F32, tag="ln_sd", bufs=1, name="ln_sd")
    nc.scalar.activation(out=sd, in_=var, func=Act.Sqrt)
    a_row = pool.tile([1, 512], F32, tag="ln_a", bufs=1, name="ln_a")
    nc.vector.reciprocal_approx_fast(out=a_row, in_=sd)
    b_row = pool.tile([1, 512], F32, tag="ln_b", bufs=1, name="ln_b")
    nc.vector.scalar_tensor_tensor(out=b_row, in0=m, scalar=-1.0, in1=a_row,
                                   op0=Alu.mult, op1=Alu.mult)
    ac = pool.tile([1, 512], BF, tag="ln_ac", bufs=1, name="ln_ac")
    bc = pool.tile([1, 512], BF, tag="ln_bc", bufs=1, name="ln_bc")
    nc.vector.tensor_copy(out=ac, in_=a_row)
    nc.vector.tensor_copy(out=bc, in_=b_row)
    Ab = pool.tile([128, 512], BF, tag="ln_Ab", bufs=2, name="ln_Ab")
    Bb = pool.tile([128, 512], BF, tag="ln_Bb", bufs=2, name="ln_Bb")
    nc.gpsimd.partition_broadcast(Ab, ac)
    nc.gpsimd.partition_broadcast(Bb, bc)
    return Ab, Bb


def _ln_stats_pe(nc, pool, psum_pool, psum_tag, src, ones_col):
    """LN stats via M=1 PE matmuls: sum(x) directly on the x k-tiles,
    sum(x^2) on DVE-squared tiles.  src: [128, KD, 512] bf16."""
    mm = nc.tensor.matmul
    ps_s = psum_pool.tile([1, 512], F32, tag=psum_tag, bufs=2, name="ps_s")
    ps_q = psum_pool.tile([1, 512], F32, tag=psum_tag, bufs=2, name="ps_q")
    for k in range(KD):
        mm(out=ps_s, lhsT=ones_col, rhs=src[:, k, :],
           start=(k == 0), stop=(k == KD - 1))
    for k in range(KD):
        sq = pool.tile([128, 512], BF, tag="ln_sq", bufs=2, name="ln_sq")
        nc.vector.tensor_mul(out=sq, in0=src[:, k, :], in1=src[:, k, :])
        mm(out=ps_q, lhsT=ones_col, rhs=sq,
           start=(k == 0), stop=(k == KD - 1))
    return _ln_math(nc, pool, ps_s, ps_q)


def _ln_stats_tree(nc, pool, psum_pool, psum_tag, src, ones_col):
    """DVE adder-tree LN stats (kept for LN2 where the PE is contended).
    src: [128, KD, 512] AP."""
    mm = nc.tensor.matmul

    def lvl(tg, n):
        return pool.tile([128, 512], BF, tag=f"{tg}{n}", bufs=2, name=tg)

    s2, q2 = [], []
    for i in range(4):
        s = lvl("lts", 2)
        nc.vector.tensor_add(out=s, in0=src[:, 2 * i, :],
                             in1=src[:, 2 * i + 1, :])
        s2.append(s)
        sqa = pool.tile([128, 512], BF, tag="ln_sq", bufs=2, name="ln_sq")
        sqb = pool.tile([128, 512], BF, tag="ln_sq", bufs=2, name="ln_sq")
        nc.vector.tensor_mul(out=sqa, in0=src[:, 2 * i, :],
                             in1=src[:, 2 * i, :])
        nc.vector.tensor_mul(out=sqb, in0=src[:, 2 * i + 1, :],
                             in1=src[:, 2 * i + 1, :])
        q = lvl("ltq", 2)
        nc.vector.tensor_add(out=q, in0=sqa, in1=sqb)
        q2.append(q)
    s4, q4_ = [], []
    for i in range(2):
        s = lvl("lts", 4)
        nc.vector.tensor_add(out=s, in0=s2[2 * i], in1=s2[2 * i + 1])
        s4.append(s)
        q = lvl("ltq", 4)
        nc.vector.tensor_add(out=q, in0=q2[2 * i], in1=q2[2 * i + 1])
        q4_.append(q)
    s_all = lvl("lts", 8)
    nc.vector.tensor_add(out=s_all, in0=s4[0], in1=s4[1])
    q_all = lvl("ltq", 8)
    nc.vector.tensor_add(out=q_all, in0=q4_[0], in1=q4_[1])

    ps_s = psum_pool.tile([1, 512], F32, tag=psum_tag, bufs=2, name="ps_s")
    ps_q = psum_pool.tile([1, 512], F32, tag=psum_tag, bufs=2, name="ps_q")
    mm(out=ps_s, lhsT=ones_col, rhs=s_all, start=True, stop=True)
    mm(out=ps_q, lhsT=ones_col, rhs=q_all, start=True, stop=True)
    return _ln_math(nc, pool, ps_s, ps_q)


def _ln_apply(nc, pool, src_k, Ab, Bb, out_slice, eng=None, tag="ln_t1"):
    """out = src*Ab + Bb."""
    eng = eng or nc.vector
    t1 = pool.tile([128, 512], BF, tag=tag, bufs=2, name="ln_t1")
    eng.tensor_mul(out=t1, in0=src_k, in1=Ab)
    eng.tensor_add(out=out_slice, in0=t1, in1=Bb)


def _emit(nc, tc, t):
    mm = nc.tensor.matmul
    Alu = mybir.AluOpType
    Act = mybir.ActivationFunctionType

    outT_v = t["outT"]

    # ---------------- persistent pools ----------------
    dram = tc.alloc_tile_pool(name="dram", bufs=1, space="DRAM")
    rs_in = [dram.tile([TP, 128, KD, 256], BF, name=f"rsi{c}")
             for c in range(NCH)]
    rs_out = [dram.tile([128, KD, 256], BF, name=f"rso{c}")
              for c in range(NCH)]

    consts = tc.alloc_tile_pool(name="consts", bufs=1)
    ones_col = consts.tile([128, 1], BF)
    nc.vector.memset(ones_col, 1.0)

    bq_sb = consts.tile([128, KHE], F32)
    bk_sb = consts.tile([128, KHE], F32)
    bo2_sb = consts.tile([128, KD], F32)
    b2_sb = consts.tile([128, KD], F32)
    b1_sb = consts.tile([128, KFF], F32)
    for name, dst in (("bq", bq_sb), ("bk", bk_sb),
                      ("bo2", bo2_sb), ("b2", b2_sb)):
        nc.sync.dma_start(out=dst, in_=t[name].rearrange("(k p) -> p k", p=128))
    nc.sync.dma_start(out=b1_sb, in_=t["b1f"].rearrange("(k p) -> p k", p=128))
    # v bias broadcast over all partitions: [128, LHE]
    bvb = consts.tile([128, LHE], BF)
    bv_row = consts.tile([1, LHE], BF)
    nc.gpsimd.dma_start(out=bv_row,
                        in_=t["bv"].rearrange("(o e) -> o e", o=1))
    nc.gpsimd.partition_broadcast(bvb, bv_row)

    wlate = tc.alloc_tile_pool(name="wlate", bufs=1)
    wo_sb = wlate.tile([128, KHE, D], F8, tag="wo")
    nc.scalar.dma_start(out=wo_sb, in_=t["wo"])

    # single PSUM pool: ps_proj(2) + ps_sc(2x2) + po(2) = 8 banks
    pps = tc.alloc_tile_pool(name="pps", bufs=2, space="PSUM")

    # stage A long-lived pool
    ab = tc.alloc_tile_pool(name="abc", bufs=1)
    kT = ab.tile([128, LH // 2, T], F8, tag="kT")
    qT = ab.tile([128, LH // 2, T], F8, tag="qT")
    vS = ab.tile([128, NST, LH * 128], BF, tag="vS")
    # early pool: x tiles, hT, qkv weights, LN1 temps (freed before FFN)
    pe_pool = tc.alloc_tile_pool(name="pearly", bufs=1)

    wq_sb = pe_pool.tile([128, KD, LHE], F8, tag="wq")
    wk_sb = pe_pool.tile([128, KD, LHE], F8, tag="wk")
    wv_sb = pe_pool.tile([128, KD, LHE], F8, tag="wv")

    def load_x(ci):
        xf = pe_pool.tile([128, KD, 512], BF, tag="xf", bufs=2, name="xf")
        engs = (nc.sync, nc.scalar, nc.gpsimd)
        for k in range(KD):
            engs[k % 3].dma_start(out=xf[:, k:k + 1, :],
                                  in_=t["xT"][ci * 128:ci * 128 + 128,
                                              k:k + 1, :])
        return xf

    def load_qkv_weights():
        for eng, srct, dst in ((nc.scalar, t["wq"], wq_sb),
                               (nc.scalar, t["wk"], wk_sb),
                               (nc.sync, t["wv"], wv_sb)):
            eng.dma_start(out=dst, in_=srct)

    def ln1(ci, xf, gps=False):
        Ab, Bb = _ln_stats_pe(nc, pe_pool, pps, "ps_proj", xf, ones_col)
        hT = pe_pool.tile([128, KD, 512], F8, tag="hT", bufs=2, name="hT")
        for k in range(KD):
            if gps and k % 2 == 1:
                _ln_apply(nc, pe_pool, xf[:, k, :], Ab, Bb, hT[:, k, :],
                          eng=nc.gpsimd, tag="ln_t1g")
            else:
                _ln_apply(nc, pe_pool, xf[:, k, :], Ab, Bb, hT[:, k, :])
        return hT

    def proj_units(ci, hT):
        c0 = ci * 512
        units = []
        for w_sb, dst, bias, rsc in ((wk_sb, kT, bk_sb, 1.0 / SK),
                                     (wq_sb, qT, bq_sb, 1.0 / SQ)):
            for et in range(LH // 2):
                def u(w_sb=w_sb, dst=dst, bias=bias, rsc=rsc, et=et,
                      hT=hT, c0=c0):
                    ps = pps.tile([128, 512], F32, tag="ps_proj",
                                  bufs=2, name="ps_proj")
                    for k in range(KD):
                        mm(out=ps,
                           lhsT=w_sb[:, k, et * 128:(et + 1) * 128],
                           rhs=hT[:, k, :],
                           start=(k == 0), stop=(k == KD - 1))
                    nc.vector.tensor_scalar(
                        out=dst[:, et, c0:c0 + 512], in0=ps,
                        scalar1=bias[:, et:et + 1], scalar2=rsc,
                        op0=Alu.add, op1=Alu.mult)
                units.append(u)
        for sti in range(4):
            st = ci * 4 + sti
            def u(sti=sti, st=st, hT=hT):
                ps = pps.tile([128, LHE], F32, tag="ps_proj", bufs=2,
                              name="ps_v")
                for k in range(KD):
                    mm(out=ps,
                       lhsT=hT[:, k, sti * 128:sti * 128 + 128],
                       rhs=wv_sb[:, k, :],
                       start=(k == 0), stop=(k == KD - 1))
                nc.vector.scalar_tensor_tensor(
                    out=vS[:, st, :].rearrange("p (h e) -> p h e",
                                               h=LH)[:, :, 0:64],
                    in0=ps.rearrange("p (h e) -> p h e", e=64),
                    scalar=1.0 / SV, op0=Alu.mult,
                    in1=bvb.rearrange("p (h e) -> p h e", e=64),
                    op1=Alu.add)
            units.append(u)
        return units

    def attention(ci, oT, pending):
        c0 = ci * 512
        nb = 4 * (ci + 1)
        total_steps = (LH // 2) * (nb + OA_LAG)
        spacing = (max(1, total_steps // len(pending))
                   if pending else 0)
        stepctr = 0
        for hp in range(LH // 2):
            po = [pps.tile([128, 512], F32, tag="po", bufs=2, name="po")
                  for _ in range(2)]
            exs = [None] * nb

            def scores(sb):
                s0 = sb * 128
                midx = sb - 4 * ci
                w0 = max(0, midx) * 128
                ps2 = pps.tile([128, 2, 512], F32, tag="ps_sc",
                               bufs=OA_LAG, name="ps_sc")
                for hi in range(2):
                    mm(out=ps2[:, hi, w0:512],
                       lhsT=kT[hi * 64:hi * 64 + 64, hp, s0:s0 + 128],
                       rhs=qT[hi * 64:hi * 64 + 64, hp,
                              c0 + w0:c0 + 512],
                       start=True, stop=True)
                ex = ab.tile([128, 2, 512], BF, tag="ex",
                             bufs=OA_LAG + 1, name="ex")
                if w0 > 0:
                    nc.gpsimd.memset(ex[:, :, 0:w0], 0.0)
                nc.scalar.activation(out=ex[:, :, w0:512],
                                     in_=ps2[:, :, w0:512], func=Act.Exp)
                if midx >= 0:
                    for hi in range(2):
                        nc.gpsimd.affine_select(
                            out=ex[:, hi, w0:w0 + 128],
                            in_=ex[:, hi, w0:w0 + 128],
                            compare_op=Alu.is_ge, fill=0.0,
                            base=0, channel_multiplier=-1,
                            pattern=[[1, 128]])
                exs[sb] = ex

            def oacc(sb):
                for hi in range(2):
                    h_loc = hp * 2 + hi
                    mm(out=po[hi],
                       lhsT=vS[:, sb, h_loc * 128:h_loc * 128 + 128],
                       rhs=exs[sb][:, hi, :],
                       start=(sb == 0), stop=(sb == nb - 1))

            for step in range(nb + OA_LAG):
                if step < nb:
                    scores(step)
                if step >= OA_LAG:
                    oacc(step - OA_LAG)
                stepctr += 1
                if pending and stepctr % spacing == 0:
                    pending.pop(0)()

            for hi in range(2):
                h_loc = hp * 2 + hi
                dnr = ab.tile([1, 512], F32, tag="dnr", bufs=1,
                              name="dnr")
                nc.vector.tensor_copy(out=dnr, in_=po[hi][64:65, :])
                rcp = ab.tile([1, 512], F32, tag="rcp", bufs=2,
                              name="rcp")
                nc.vector.reciprocal_approx_fast(out=rcp, in_=dnr)
                bc = ab.tile([64, 512], F32, tag="bc", bufs=1,
                             name="bc")
                nc.gpsimd.partition_broadcast(bc, rcp)
                nc.vector.tensor_mul(
                    out=oT[hi * 64:hi * 64 + 64, hp, :],
                    in0=po[hi][0:64, :], in1=bc)
        while pending:
            pending.pop(0)()

    def wo_units(ci, oT):
        rsv = rs_in[ci]
        units = []
        for dt in range(KD):
            def u(dt=dt, oT=oT, rsv=rsv):
                ps = pps.tile([128, 512], F32, tag="ps_proj",
                              bufs=2, name="ps_wo")
                for k in range(KHE):
                    mm(out=ps,
                       lhsT=wo_sb[:, k, dt * 128:(dt + 1) * 128],
                       rhs=oT[:, k, :],
                       start=(k == 0), stop=(k == KHE - 1))
                stg = ab.tile([128, 512], BF, tag="stg1", bufs=2,
                              name="stg1")
                nc.vector.tensor_scalar(
                    out=stg, in0=ps, scalar1=bo2_sb[:, dt:dt + 1],
                    scalar2=1.0 / SO, op0=Alu.add, op1=Alu.mult)
                for j in range(TP):
                    nc.sync.dma_start(
                        out=rsv[j, :, dt, :],
                        in_=stg[:, j * 256:(j + 1) * 256])
            units.append(u)
        return units

    def rs_unit(ci):
        def u():
            nc.gpsimd.collective_compute(
                "ReduceScatter", Alu.add, replica_groups=PAIRS,
                ins=[rs_in[ci].opt()], outs=[rs_out[ci].opt()])
        return u

    # ---------------- stage B (FFN) units (pool allocated later) --------
    state = {}

    def prep_units(lc):
        """residual + LN2 + apply, decomposed into pending units."""
        de = state["de"]
        units = []

        def u_load():
            xmid = de.tile([128, KD, 512], BF, tag="xmid", bufs=2,
                           name="xmid")
            xrs = t["xresT"][lc * 128:lc * 128 + 128, :, :]
            nc.sync.dma_start(out=xmid[:, 0:4, :], in_=xrs[:, 0:4, :])
            nc.scalar.dma_start(out=xmid[:, 4:8, :], in_=xrs[:, 4:8, :])
            arr = de.tile([128, KD, 2, 256], BF, tag="arr", bufs=1,
                          name="arr")
            nc.sync.dma_start(out=arr[:, 0:4, 0, :],
                              in_=rs_out[2 * lc][:, 0:4, :])
            nc.scalar.dma_start(out=arr[:, 4:8, 0, :],
                                in_=rs_out[2 * lc][:, 4:8, :])
            nc.sync.dma_start(out=arr[:, 0:4, 1, :],
                              in_=rs_out[2 * lc + 1][:, 0:4, :])
            nc.scalar.dma_start(out=arr[:, 4:8, 1, :],
                                in_=rs_out[2 * lc + 1][:, 4:8, :])
            state[f"xmid{lc}"] = xmid
            state[f"arr{lc}"] = arr
        units.append(u_load)

        def u_add():
            xmid = state[f"xmid{lc}"]
            arr = state[f"arr{lc}"]
            for k in range(KD):
                nc.vector.tensor_add(
                    out=xmid[:, k, :], in0=xmid[:, k, :],
                    in1=arr[:, k, :, :].rearrange("p j t -> p (j t)"))
        units.append(u_add)

        def u_stats():
            xmid = state[f"xmid{lc}"]
            state[f"ab{lc}"] = _ln_stats_pe(nc, de, pps, "ps_proj",
                                            xmid, ones_col)
        units.append(u_stats)

        def mk_apply(k0):
            def u_apply():
                xmid = state[f"xmid{lc}"]
                Ab2, Bb2 = state[f"ab{lc}"]
                if f"h2{lc}" not in state:
                    state[f"h2{lc}"] = de.tile([128, KD, 512], BF,
                                               tag="h2", bufs=1, name="h2")
                h2 = state[f"h2{lc}"]
                for k in range(k0, k0 + 4):
                    _ln_apply(nc, de, xmid[:, k, :], Ab2, Bb2, h2[:, k, :])
            return u_apply
        units.append(mk_apply(0))
        units.append(mk_apply(4))
        return units

    def fc1_units(lc):
        """FFN up: u = relu(h2 @ W1 + b1f); one unit per 256-wide block."""
        de = state["de"]
        units = []

        def mk(q16):
            def u():
                h2 = state[f"h2{lc}"]
                if f"u{lc}" not in state:
                    state[f"u{lc}"] = de.tile([128, KFF, 512], BF,
                                              tag="u", bufs=1, name="u")
                uu = state[f"u{lc}"]
                w1t = de.tile([128, KD, 256], BF, tag="w1t", bufs=2,
                              name="w1t")
                nc.sync.dma_start(
                    out=w1t, in_=t["w1"][q16 * 128:(q16 + 1) * 128, :, :])
                for fi in range(2):
                    fft = q16 * 2 + fi
                    ps = pps.tile([128, 512], F32, tag="ps_proj", bufs=2,
                                  name="ps_u")
                    for k in range(KD):
                        mm(out=ps,
                           lhsT=w1t[:, k, fi * 128:fi * 128 + 128],
                           rhs=h2[:, k, :],
                           start=(k == 0), stop=(k == KD - 1))
                    if lc == 0:
                        nc.vector.tensor_scalar(
                            out=uu[:, fft, :], in0=ps,
                            scalar1=b1_sb[:, fft:fft + 1], scalar2=0.0,
                            op0=Alu.add, op1=Alu.max)
                    else:
                        nc.scalar.activation(
                            out=uu[:, fft, :], in_=ps, func=Act.Relu,
                            bias=b1_sb[:, fft:fft + 1])
            units.append(u)
        for q16 in range(16):
            mk(q16)
        return units

    def fc2_units(lc):
        """FFN down + bias + residual -> store; W2 streamed per dt."""
        de = state["de"]
        c0 = lc * 512
        units = []

        def mk(dt):
            def u():
                uu = state[f"u{lc}"]
                xmid = state[f"xmid{lc}"]
                w2t = state.pop(f"w2t{dt}", None) if lc == 0 else None
                if w2t is None:
                    w2t = de.tile([128, KFF, 128], BF, tag="w2t", bufs=2,
                                  name="w2t")
                    nc.scalar.dma_start(
                        out=w2t,
                        in_=t["w2"][dt * 128:(dt + 1) * 128, :, :])
                ps = pps.tile([128, 512], F32, tag="ps_proj", bufs=2,
                              name="ps_f")
                for k2 in range(KFF):
                    mm(out=ps, lhsT=w2t[:, k2, :], rhs=uu[:, k2, :],
                       start=(k2 == 0), stop=(k2 == KFF - 1))
                o_f = de.tile([128, 512], F32, tag="o_f", bufs=2,
                              name="o_f")
                nc.vector.scalar_tensor_tensor(
                    out=o_f, in0=ps, scalar=b2_sb[:, dt:dt + 1],
                    in1=xmid[:, dt, :], op0=Alu.add, op1=Alu.add)
                nc.sync.dma_start(
                    out=outT_v[dt * 128:(dt + 1) * 128, c0:c0 + 512],
                    in_=o_f)
            units.append(u)
        for dt in range(KD):
            mk(dt)
        return units

    # ================= emission schedule =================
    xf0 = load_x(0)
    load_qkv_weights()
    xf1 = load_x(1)
    # vS zero + softmax-denominator ones columns (needed only by the
    # v-projection epilogues; emitted off the ln1(0) critical path)
    nc.gpsimd.memset(vS[:, 8:16, :], 0.0)

    def w2_prefetch(dt):
        def u():
            de = state["de"]
            w2t = de.tile([128, KFF, 128], BF, tag="w2t", bufs=2,
                          name="w2t")
            nc.scalar.dma_start(
                out=w2t, in_=t["w2"][dt * 128:(dt + 1) * 128, :, :])
            state[f"w2t{dt}"] = w2t
        return u

    hT0 = ln1(0, xf0)
    nc.gpsimd.memset(vS[:, 0:8, :], 0.0)
    for h in range(LH):
        nc.gpsimd.memset(vS[:, :, h * 128 + 64:h * 128 + 65], 1.0)
    for u in proj_units(0, hT0):
        u()
    hT1 = ln1(1, xf1)

    oTs = {}
    for ci in range(NCH):
        oTs[ci] = None

    def new_oT():
        return ab.tile([128, KHE, 512], F8, tag="oT", bufs=2, name="oT")

    oTs[0] = new_oT()
    attention(0, oTs[0], list(proj_units(1, hT1)))

    xf2 = load_x(2)
    hT2 = ln1(2, xf2)
    xf3 = load_x(3)
    hT3 = ln1(3, xf3)

    oTs[1] = new_oT()
    attention(1, oTs[1],
              wo_units(0, oTs[0])
              + proj_units(2, hT2) + proj_units(3, hT3))
    # collectives issued between attention chunks: the issuing gpsimd
    # queue blocks until the collective completes, and the next chunk
    # needs no gpsimd work for its first ~20us
    rs_unit(0)()
    pe_pool.release()
    state["de"] = tc.alloc_tile_pool(name="de", bufs=1)

    oTs[3] = new_oT()
    attention(3, oTs[3], wo_units(1, oTs[1]))
    rs_unit(1)()

    f10 = fc1_units(0)
    oTs[2] = new_oT()
    attention(2, oTs[2],
              wo_units(3, oTs[3])
              + prep_units(0) + [w2_prefetch(0), w2_prefetch(1)]
              + f10[:4])

    rs_unit(3)()
    for u in wo_units(2, oTs[2]):
        u()
    rs_unit(2)()

    # bulk of lc0 FFN here so the tail ReduceScatters hide behind it
    for u in f10[4:]:
        u()
    p1 = prep_units(1)
    f20 = fc2_units(0)
    inter = []
    while p1 or f20:
        if f20:
            inter.append(f20.pop(0))
        if p1:
            inter.append(p1.pop(0))
    for u in inter:
        u()
    for u in fc1_units(1):
        u()
    for u in fc2_units(1):
        u()

    state["de"].release()
    ab.release()
    pps.release()
    wlate.release()
    consts.release()
    dram.release()


def _build():
    nc = bacc.Bacc("TRN2", target_bir_lowering=False, debug=False,
                   num_devices=NCORES)

    tensors = {}
    tensors["xT"] = nc.dram_tensor("xT", [NCH * 128, KD, 512], BF,
                                   kind="ExternalInput").ap()
    tensors["xresT"] = nc.dram_tensor("xresT", [TP * 128, KD, 512], BF,
                                      kind="ExternalInput").ap()
    for name, shape, dt in (
        ("wq", [128, KD, 512], F8), ("wk", [128, KD, 512], F8),
        ("wv", [128, KD, 512], F8), ("wo", [128, KHE, D], F8),
        ("w1", [16 * 128, KD, 256], BF), ("w2", [8 * 128, KFF, 128], BF),
        ("bq", [LHE], F32), ("bk", [LHE], F32), ("bv", [LHE], F32),
        ("b1f", [FF], F32), ("bo2", [D], F32), ("b2", [D], F32),
    ):
        tensors[name] = nc.dram_tensor(name, shape, dt,
                                       kind="ExternalInput").ap()
    tensors["outT"] = nc.dram_tensor("out", [D, LT], F32,
                                     kind="ExternalOutput").ap()

    with tile.TileContext(nc, num_cores=NCORES) as tc:
        _emit(nc, tc, tensors)

    nc.compile()
    return nc


_NC_CACHE = None


def _get_nc():
    global _NC_CACHE
    if _NC_CACHE is None:
        _NC_CACHE = _build()
    return _NC_CACHE


def _shard_inputs(x, Wq, Wk, Wv, Wo, bo, W1, b1, W2, b2, g1, be1, g2, be2):
    """Build the 8 per-core input maps (LN gains folded into weights)."""
    bf = lambda a: np.ascontiguousarray(a).astype(BF16NP)
    f8 = lambda a: np.ascontiguousarray(a).astype(FP8NP)
    f32 = lambda a: np.ascontiguousarray(a, dtype=np.float32)

    x = np.asarray(x, dtype=np.float32)
    Wq = np.asarray(Wq, dtype=np.float32)
    Wk = np.asarray(Wk, dtype=np.float32)
    Wv = np.asarray(Wv, dtype=np.float32)
    Wo = np.asarray(Wo, dtype=np.float32)
    W1 = np.asarray(W1, dtype=np.float32)
    W2 = np.asarray(W2, dtype=np.float32)
    g1 = np.asarray(g1, dtype=np.float32)
    be1 = np.asarray(be1, dtype=np.float32)
    g2 = np.asarray(g2, dtype=np.float32)
    be2 = np.asarray(be2, dtype=np.float32)
    b1 = np.asarray(b1, dtype=np.float32)

    scale = float(HS) ** -0.5
    # fold g1 into QKV weights, be1 into QKV biases; fold the score scale
    # into Wq/bq.  Per-head [H, D, HS] -> concat heads -> [D, H*HS].
    wq_f = (g1[None, :, None] * Wq).transpose(1, 0, 2).reshape(D, D) * scale
    wk_f = (g1[None, :, None] * Wk).transpose(1, 0, 2).reshape(D, D)
    wv_f = (g1[None, :, None] * Wv).transpose(1, 0, 2).reshape(D, D)
    bq_f = np.einsum("d,hde->he", be1, Wq).reshape(D) * scale
    bk_f = np.einsum("d,hde->he", be1, Wk).reshape(D)
    bv_f = np.einsum("d,hde->he", be1, Wv).reshape(D)
    # fold g2/be2 into W1/b1
    w1_f = g2[:, None] * W1
    b1_f = b1 + be2 @ W1

    in_maps = []
    for c in range(NCORES):
        b, half = divmod(c, TP)
        hes = slice(half * LHE, (half + 1) * LHE)
        xt = x[b].T
        xres = np.concatenate(
            [xt[:, ci * 512 + half * 256: ci * 512 + half * 256 + 256]
             for ci in range(NCH)], axis=1)
        # partition-major tiled layouts: loads become 128 contiguous
        # segments instead of 1024 scattered ones (descriptor-gen bound)
        xt_sw = xt.reshape(KD, 128, NCH, 512).transpose(2, 1, 0, 3)
        xres_sw = xres.reshape(KD, 128, TP, 512).transpose(2, 1, 0, 3)
        wq_sw = wq_f[:, hes].reshape(KD, 128, LHE).transpose(1, 0, 2)
        wk_sw = wk_f[:, hes].reshape(KD, 128, LHE).transpose(1, 0, 2)
        wv_sw = wv_f[:, hes].reshape(KD, 128, LHE).transpose(1, 0, 2)
        wo_sw = Wo[hes, :].reshape(KHE, 128, D).transpose(1, 0, 2)
        w1_sw = w1_f.reshape(KD, 128, 16, 256).transpose(2, 1, 0, 3)
        # W2 tiled dt-major so fc2 streams contiguous [128, KFF, 128] tiles
        w2_sw = W2.reshape(KFF, 128, KD, 128).transpose(2, 1, 0, 3)
        in_maps.append({
            "xT": bf(xt_sw.reshape(NCH * 128, KD, 512)),
            "xresT": bf(xres_sw.reshape(TP * 128, KD, 512)),
            "wq": f8(wq_sw * SQ), "wk": f8(wk_sw * SK), "wv": f8(wv_sw * SV),
            "bq": f32(bq_f[hes] * SQ), "bk": f32(bk_f[hes] * SK),
            "bv": f32(bv_f[hes]),
            "wo": f8(wo_sw * SO),
            "bo2": f32(np.asarray(bo, dtype=np.float32) * SO / TP),
            "w1": bf(w1_sw.reshape(16 * 128, KD, 256)), "b1f": f32(b1_f),
            "w2": bf(w2_sw.reshape(8 * 128, KFF, 128)),
            "b2": f32(np.asarray(b2, dtype=np.float32)),
        })
    return in_maps


def kernel(x, Wq, Wk, Wv, Wo, bo, W1, b1, W2, b2, g1, be1, g2, be2,
           _trace=False):
    nc = _get_nc()
    in_maps = _shard_inputs(x, Wq, Wk, Wv, Wo, bo, W1, b1, W2, b2,
                            g1, be1, g2, be2)
    res = run_bass_kernel_spmd(nc, in_maps, list(range(NCORES)),
                               trace=_trace)
    out = np.empty((B, T, D), dtype=np.float32)
    for b in range(B):
        for half in range(TP):
            o = res.results[TP * b + half]["out"]  # [D, LT]
            for ci in range(NCH):
                t0 = ci * 512 + half * 256
                out[b, t0:t0 + 256, :] = o[:, ci * 256:(ci + 1) * 256].T
    if _trace:
        kernel.last_exec_time_ns = res.exec_time_ns
        kernel.last_results = res
    return out


# revision 52
# speedup vs baseline: 1.2068x; 1.0208x over previous
"""Trainium2 Bass kernel for a pre-LN transformer block (B=4, T=2048, D=1024,
H=16, HS=64, FF=4096, causal attention).

Sharding: data-parallel over batches x 2-way tensor-parallel attention
(8 heads/core over all T) -> pair ReduceScatter of the attention-output
projection over the sequence dim -> sequence-parallel FFN (full FF width,
T/2 rows per core).  No AllReduce anywhere; each core emits the final
output for its own T/2 rows.

Core c (0..7): batch b = c//2, half = c%2.  half h owns t-slices
[ci*512 + h*256, ci*512 + h*256 + 256) for ci in 0..3.

v2 schedule: attention chunks run 0,1,3,2 so that all four ReduceScatters
except the last are issued mid-attention, and the FFN for the first half
of rows (prep+fc1) is interleaved into the final attention chunk as
pending units -- the tail RS and LN2 hide behind fc1/fc2 matmuls.
Attention-path tensors (weights, hT, kT, qT, oT) are fp8e4m3 with
power-of-2 scales folded into the projection epilogues; this halves
their SBUF/DMA cost (matmuls run at bf16 rate).  LN1 stats come from
M=1 PE matmuls instead of DVE adder trees.  Diagonal score tiles are
N-trimmed and their exp is windowed, with the causal mask applied only
to the [128,128] band.  W1/W2 stream through double-buffered tiles.
"""

import numpy as np
import ml_dtypes

import concourse.bacc as bacc
import concourse.bass as bass
import concourse.mybir as mybir
import concourse.tile as tile
from concourse.bass_utils import run_bass_kernel_spmd

BF16NP = ml_dtypes.bfloat16
FP8NP = ml_dtypes.float8_e4m3

B, T, D, H, HS, FF = 4, 2048, 1024, 16, 64, 4096
EPS = 1e-5
NCORES = 8
TP = 2
LH = H // TP          # 8 local heads
LHE = LH * HS         # 512 local head-embed width
LT = T // TP          # 1024 local rows (FFN/output)
KD = D // 128         # 8 d k-tiles
KHE = LHE // 128      # 4 he k-tiles
KFF = FF // 128       # 32 ff tiles
NCH = T // 512        # 4 t-chunks of 512
NST = T // 128        # 16 s-tiles of 128
PAIRS = [[0, 1], [2, 3], [4, 5], [6, 7]]
OA_LAG = 2            # psc tiles in flight between scores and o-accum

F32 = mybir.dt.float32
BF = mybir.dt.bfloat16
F8 = mybir.dt.float8e4
F85 = mybir.dt.float8e5
DRM = mybir.MatmulPerfMode.DoubleRow
# fp8 weight scales (power-of-2, folded out in the epilogues)
SQ = 256.0   # wq carries g1 and HS^-0.5 -> sigma 1/256
SK = 32.0
SV = 32.0
SO = 32.0


def _ln_math(nc, pool, ps_s, ps_q):
    """From psum row-sums (ps_s, ps_q over D) to bf16 broadcast tiles
    (Ab, Bb) so that xn = x*Ab + Bb."""
    Alu = mybir.AluOpType
    Act = mybir.ActivationFunctionType
    m = pool.tile([1, 512], F32, tag="ln_m", bufs=1, name="ln_m")
    e2 = pool.tile([1, 512], F32, tag="ln_e2", bufs=1, name="ln_e2")
    nc.vector.tensor_scalar_mul(out=m, in0=ps_s, scalar1=1.0 / D)
    nc.vector.tensor_scalar_mul(out=e2, in0=ps_q, scalar1=1.0 / D)
    msq = pool.tile([1, 512], F32, tag="ln_msq", bufs=1, name="ln_msq")
    nc.vector.tensor_mul(out=msq, in0=m, in1=m)
    var = pool.tile([1, 512], F32, tag="ln_var", bufs=1, name="ln_var")
    nc.vector.scalar_tensor_tensor(out=var, in0=e2, scalar=EPS, in1=msq,
                                   op0=Alu.add, op1=Alu.subtract)
    sd = pool.tile([1, 512], F32, tag="ln_sd", bufs=1, name="ln_sd")
    nc.scalar.activation(out=sd, in_=var, func=Act.Sqrt)
    a_row = pool.tile([1, 512], F32, tag="ln_a", bufs=1, name="ln_a")
    nc.vector.reciprocal_approx_fast(out=a_row, in_=sd)
    b_row = pool.tile([1, 512], F32, tag="ln_b", bufs=1, name="ln_b")
    nc.vector.scalar_tensor_tensor(out=b_row, in0=m, scalar=-1.0, in1=a_row,
                                   op0=Alu.mult, op1=Alu.mult)
    ac = pool.tile([1, 512], BF, tag="ln_ac", bufs=1, name="ln_ac")
    bc = pool.tile([1, 512], BF, tag="ln_bc", bufs=1, name="ln_bc")
    nc.vector.tensor_copy(out=ac, in_=a_row)
    nc.vector.tensor_copy(out=bc, in_=b_row)
    Ab = pool.tile([128, 512], BF, tag="ln_Ab", bufs=2, name="ln_Ab")
    Bb = pool.tile([128, 512], BF, tag="ln_Bb", bufs=2, name="ln_Bb")
    nc.gpsimd.partition_broadcast(Ab, ac)
    nc.gpsimd.partition_broadcast(Bb, bc)
    return Ab, Bb


def _ln_stats_pe(nc, pool, psum_pool, psum_tag, src, ones_col):
    """LN stats via M=1 PE matmuls: sum(x) directly on the x k-tiles,
    sum(x^2) on DVE-squared tiles.  src: [128, KD, 512] bf16."""
    mm = nc.tensor.matmul
    ps_s = psum_pool.tile([1, 512], F32, tag=psum_tag, bufs=2, name="ps_s")
    ps_q = psum_pool.tile([1, 512], F32, tag=psum_tag, bufs=2, name="ps_q")
    for k in range(KD):
        mm(out=ps_s, lhsT=ones_col, rhs=src[:, k, :],
           start=(k == 0), stop=(k == KD - 1))
    for k in range(KD):
        sq = pool.tile([128, 512], BF, tag="ln_sq", bufs=2, name="ln_sq")
        nc.vector.tensor_mul(out=sq, in0=src[:, k, :], in1=src[:, k, :])
        mm(out=ps_q, lhsT=ones_col, rhs=sq,
           start=(k == 0), stop=(k == KD - 1))
    return _ln_math(nc, pool, ps_s, ps_q)


def _ln_stats_tree(nc, pool, psum_pool, psum_tag, src, ones_col):
    """DVE adder-tree LN stats (kept for LN2 where the PE is contended).
    src: [128, KD, 512] AP."""
    mm = nc.tensor.matmul

    def lvl(tg, n):
        return pool.tile([128, 512], BF, tag=f"{tg}{n}", bufs=2, name=tg)

    s2, q2 = [], []
    for i in range(4):
        s = lvl("lts", 2)
        nc.vector.tensor_add(out=s, in0=src[:, 2 * i, :],
                             in1=src[:, 2 * i + 1, :])
        s2.append(s)
        sqa = pool.tile([128, 512], BF, tag="ln_sq", bufs=2, name="ln_sq")
        sqb = pool.tile([128, 512], BF, tag="ln_sq", bufs=2, name="ln_sq")
        nc.vector.tensor_mul(out=sqa, in0=src[:, 2 * i, :],
                             in1=src[:, 2 * i, :])
        nc.vector.tensor_mul(out=sqb, in0=src[:, 2 * i + 1, :],
                             in1=src[:, 2 * i + 1, :])
        q = lvl("ltq", 2)
        nc.vector.tensor_add(out=q, in0=sqa, in1=sqb)
        q2.append(q)
    s4, q4_ = [], []
    for i in range(2):
        s = lvl("lts", 4)
        nc.vector.tensor_add(out=s, in0=s2[2 * i], in1=s2[2 * i + 1])
        s4.append(s)
        q = lvl("ltq", 4)
        nc.vector.tensor_add(out=q, in0=q2[2 * i], in1=q2[2 * i + 1])
        q4_.append(q)
    s_all = lvl("lts", 8)
    nc.vector.tensor_add(out=s_all, in0=s4[0], in1=s4[1])
    q_all = lvl("ltq", 8)
    nc.vector.tensor_add(out=q_all, in0=q4_[0], in1=q4_[1])

    ps_s = psum_pool.tile([1, 512], F32, tag=psum_tag, bufs=2, name="ps_s")
    ps_q = psum_pool.tile([1, 512], F32, tag=psum_tag, bufs=2, name="ps_q")
    mm(out=ps_s, lhsT=ones_col, rhs=s_all, start=True, stop=True)
    mm(out=ps_q, lhsT=ones_col, rhs=q_all, start=True, stop=True)
    return _ln_math(nc, pool, ps_s, ps_q)


def _ln_apply(nc, pool, src_k, Ab, Bb, out_slice, eng=None, tag="ln_t1"):
    """out = src*Ab + Bb."""
    eng = eng or nc.vector
    t1 = pool.tile([128, 512], BF, tag=tag, bufs=2, name="ln_t1")
    eng.tensor_mul(out=t1, in0=src_k, in1=Ab)
    eng.tensor_add(out=out_slice, in0=t1, in1=Bb)


def _emit(nc, tc, t):
    mm = nc.tensor.matmul
    Alu = mybir.AluOpType
    Act = mybir.ActivationFunctionType

    outT_v = t["outT"]

    # ---------------- persistent pools ----------------
    dram = tc.alloc_tile_pool(name="dram", bufs=1, space="DRAM")
    rs_in = [dram.tile([TP, 128, KD, 256], BF, name=f"rsi{c}")
             for c in range(NCH)]
    rs_out = [dram.tile([128, KD, 256], BF, name=f"rso{c}")
              for c in range(NCH)]

    consts = tc.alloc_tile_pool(name="consts", bufs=1)
    ones_col = consts.tile([128, 1], BF)
    nc.vector.memset(ones_col, 1.0)

    bq_sb = consts.tile([128, KHE], F32)
    bk_sb = consts.tile([128, KHE], F32)
    bo2_sb = consts.tile([128, KD], F32)
    b2_sb = consts.tile([128, KD], F32)
    b1_sb = consts.tile([128, KFF], F32)
    for name, dst in (("bq", bq_sb), ("bk", bk_sb),
                      ("bo2", bo2_sb), ("b2", b2_sb)):
        nc.sync.dma_start(out=dst, in_=t[name])
    nc.sync.dma_start(out=b1_sb, in_=t["b1f"])
    # v bias broadcast over all partitions: [128, LHE]
    bvb = consts.tile([128, LHE], BF)
    bv_row = consts.tile([1, LHE], BF)
    nc.gpsimd.dma_start(out=bv_row,
                        in_=t["bv"].rearrange("(o e) -> o e", o=1))
    nc.gpsimd.partition_broadcast(bvb, bv_row)

    wlate = tc.alloc_tile_pool(name="wlate", bufs=1)
    wo_sb = wlate.tile([128, KHE, D], F8, tag="wo")
    nc.scalar.dma_start(out=wo_sb, in_=t["wo"])

    # single PSUM pool: ps_proj(2) + ps_sc(2x2) + po(2) = 8 banks
    pps = tc.alloc_tile_pool(name="pps", bufs=2, space="PSUM")

    # stage A long-lived pool
    ab = tc.alloc_tile_pool(name="abc", bufs=1)
    kT = ab.tile([128, LH // 2, T], F8, tag="kT")
    qT = ab.tile([128, LH // 2, T], F8, tag="qT")
    vS = ab.tile([128, NST, LH * 128], BF, tag="vS")
    # early pool: x tiles, hT, qkv weights, LN1 temps (freed before FFN)
    pe_pool = tc.alloc_tile_pool(name="pearly", bufs=1)

    wq_sb = pe_pool.tile([128, KD, LHE], F8, tag="wq")
    wk_sb = pe_pool.tile([128, KD, LHE], F8, tag="wk")
    wv_sb = pe_pool.tile([128, KD, LHE], F8, tag="wv")

    def load_x(ci):
        xf = pe_pool.tile([128, KD, 512], BF, tag="xf", bufs=2, name="xf")
        engs = (nc.sync, nc.scalar, nc.gpsimd)
        for k in range(KD):
            engs[k % 3].dma_start(out=xf[:, k:k + 1, :],
                                  in_=t["xT"][ci * 128:ci * 128 + 128,
                                              k:k + 1, :])
        return xf

    def load_qkv_weights():
        for eng, srct, dst in ((nc.scalar, t["wq"], wq_sb),
                               (nc.scalar, t["wk"], wk_sb),
                               (nc.sync, t["wv"], wv_sb)):
            eng.dma_start(out=dst, in_=srct)

    def ln1(ci, xf, gps=False):
        Ab, Bb = _ln_stats_pe(nc, pe_pool, pps, "ps_proj", xf, ones_col)
        hT = pe_pool.tile([128, KD, 512], F8, tag="hT", bufs=2, name="hT")
        for k in range(KD):
            if gps and k % 2 == 1:
                _ln_apply(nc, pe_pool, xf[:, k, :], Ab, Bb, hT[:, k, :],
                          eng=nc.gpsimd, tag="ln_t1g")
            else:
                _ln_apply(nc, pe_pool, xf[:, k, :], Ab, Bb, hT[:, k, :])
        return hT

    def proj_units(ci, hT):
        c0 = ci * 512
        units = []
        for w_sb, dst, bias, rsc in ((wk_sb, kT, bk_sb, 1.0 / SK),
                                     (wq_sb, qT, bq_sb, 1.0 / SQ)):
            for et in range(LH // 2):
                def u(w_sb=w_sb, dst=dst, bias=bias, rsc=rsc, et=et,
                      hT=hT, c0=c0):
                    ps = pps.tile([128, 512], F32, tag="ps_proj",
                                  bufs=2, name="ps_proj")
                    for k in range(KD):
                        mm(out=ps,
                           lhsT=w_sb[:, k, et * 128:(et + 1) * 128],
                           rhs=hT[:, k, :],
                           start=(k == 0), stop=(k == KD - 1))
                    nc.vector.tensor_scalar(
                        out=dst[:, et, c0:c0 + 512], in0=ps,
                        scalar1=bias[:, et:et + 1], scalar2=rsc,
                        op0=Alu.add, op1=Alu.mult)
                units.append(u)
        for sti in range(4):
            st = ci * 4 + sti
            def u(sti=sti, st=st, hT=hT):
                ps = pps.tile([128, LHE], F32, tag="ps_proj", bufs=2,
                              name="ps_v")
                for k in range(KD):
                    mm(out=ps,
                       lhsT=hT[:, k, sti * 128:sti * 128 + 128],
                       rhs=wv_sb[:, k, :],
                       start=(k == 0), stop=(k == KD - 1))
                nc.vector.scalar_tensor_tensor(
                    out=vS[:, st, :].rearrange("p (h e) -> p h e",
                                               h=LH)[:, :, 0:64],
                    in0=ps.rearrange("p (h e) -> p h e", e=64),
                    scalar=1.0 / SV, op0=Alu.mult,
                    in1=bvb.rearrange("p (h e) -> p h e", e=64),
                    op1=Alu.add)
            units.append(u)
        return units

    def attention(ci, oT, pending):
        c0 = ci * 512
        nb = 4 * (ci + 1)
        total_steps = (LH // 2) * (nb + OA_LAG)
        spacing = (max(1, total_steps // len(pending))
                   if pending else 0)
        stepctr = 0
        for hp in range(LH // 2):
            po = [pps.tile([128, 512], F32, tag="po", bufs=2, name="po")
                  for _ in range(2)]
            exs = [None] * nb

            def scores(sb):
                s0 = sb * 128
                midx = sb - 4 * ci
                w0 = max(0, midx) * 128
                ps2 = pps.tile([128, 2, 512], F32, tag="ps_sc",
                               bufs=OA_LAG, name="ps_sc")
                for hi in range(2):
                    mm(out=ps2[:, hi, w0:512],
                       lhsT=kT[hi * 64:hi * 64 + 64, hp, s0:s0 + 128],
                       rhs=qT[hi * 64:hi * 64 + 64, hp,
                              c0 + w0:c0 + 512],
                       start=True, stop=True)
                ex = ab.tile([128, 2, 512], BF, tag="ex",
                             bufs=OA_LAG + 1, name="ex")
                nc.scalar.activation(out=ex[:, :, w0:512],
                                     in_=ps2[:, :, w0:512], func=Act.Exp)
                if midx >= 0:
                    for hi in range(2):
                        nc.gpsimd.affine_select(
                            out=ex[:, hi, w0:w0 + 128],
                            in_=ex[:, hi, w0:w0 + 128],
                            compare_op=Alu.is_ge, fill=0.0,
                            base=0, channel_multiplier=-1,
                            pattern=[[1, 128]])
                exs[sb] = ex

            def oacc(sb):
                w0 = max(0, sb - 4 * ci) * 128
                for hi in range(2):
                    h_loc = hp * 2 + hi
                    mm(out=po[hi][:, w0:512],
                       lhsT=vS[:, sb, h_loc * 128:h_loc * 128 + 128],
                       rhs=exs[sb][:, hi, w0:512],
                       start=(sb == 0), stop=(sb == nb - 1),
                       skip_group_check=(w0 > 0))

            for step in range(nb + OA_LAG):
                if step < nb:
                    scores(step)
                if step >= OA_LAG:
                    oacc(step - OA_LAG)
                stepctr += 1
                if pending and stepctr % spacing == 0:
                    pending.pop(0)()

            for hi in range(2):
                h_loc = hp * 2 + hi
                dnr = ab.tile([1, 512], F32, tag="dnr", bufs=1,
                              name="dnr")
                nc.vector.tensor_copy(out=dnr, in_=po[hi][64:65, :])
                rcp = ab.tile([1, 512], F32, tag="rcp", bufs=2,
                              name="rcp")
                nc.vector.reciprocal_approx_fast(out=rcp, in_=dnr)
                bc = ab.tile([64, 512], F32, tag="bc", bufs=1,
                             name="bc")
                nc.gpsimd.partition_broadcast(bc, rcp)
                nc.vector.tensor_mul(
                    out=oT[hi * 64:hi * 64 + 64, hp, :],
                    in0=po[hi][0:64, :], in1=bc)
        while pending:
            pending.pop(0)()

    def wo_units(ci, oT):
        rsv = rs_in[ci]
        units = []
        for dt in range(KD):
            def u(dt=dt, oT=oT, rsv=rsv):
                ps = pps.tile([128, 512], F32, tag="ps_proj",
                              bufs=2, name="ps_wo")
                for k in range(KHE):
                    mm(out=ps,
                       lhsT=wo_sb[:, k, dt * 128:(dt + 1) * 128],
                       rhs=oT[:, k, :],
                       start=(k == 0), stop=(k == KHE - 1))
                stg = ab.tile([128, 512], BF, tag="stg1", bufs=2,
                              name="stg1")
                nc.vector.tensor_scalar(
                    out=stg, in0=ps, scalar1=bo2_sb[:, dt:dt + 1],
                    scalar2=1.0 / SO, op0=Alu.add, op1=Alu.mult)
                for j in range(TP):
                    nc.sync.dma_start(
                        out=rsv[j, :, dt, :],
                        in_=stg[:, j * 256:(j + 1) * 256])
            units.append(u)
        return units

    def rs_unit(ci):
        def u():
            nc.gpsimd.collective_compute(
                "ReduceScatter", Alu.add, replica_groups=PAIRS,
                ins=[rs_in[ci].opt()], outs=[rs_out[ci].opt()])
        return u

    # ---------------- stage B (FFN) units (pool allocated later) --------
    state = {}

    def prep_units(lc):
        """residual + LN2 + apply, decomposed into pending units."""
        de = state["de"]
        units = []

        def u_load():
            xmid = de.tile([128, KD, 512], BF, tag="xmid", bufs=2,
                           name="xmid")
            xrs = t["xresT"][lc * 128:lc * 128 + 128, :, :]
            nc.sync.dma_start(out=xmid[:, 0:4, :], in_=xrs[:, 0:4, :])
            nc.scalar.dma_start(out=xmid[:, 4:8, :], in_=xrs[:, 4:8, :])
            arr = de.tile([128, KD, 2, 256], BF, tag="arr", bufs=1,
                          name="arr")
            nc.sync.dma_start(out=arr[:, 0:4, 0, :],
                              in_=rs_out[2 * lc][:, 0:4, :])
            nc.scalar.dma_start(out=arr[:, 4:8, 0, :],
                                in_=rs_out[2 * lc][:, 4:8, :])
            nc.sync.dma_start(out=arr[:, 0:4, 1, :],
                              in_=rs_out[2 * lc + 1][:, 0:4, :])
            nc.scalar.dma_start(out=arr[:, 4:8, 1, :],
                                in_=rs_out[2 * lc + 1][:, 4:8, :])
            state[f"xmid{lc}"] = xmid
            state[f"arr{lc}"] = arr
        units.append(u_load)

        def u_add():
            xmid = state[f"xmid{lc}"]
            arr = state[f"arr{lc}"]
            for k in range(KD):
                nc.vector.tensor_add(
                    out=xmid[:, k, :], in0=xmid[:, k, :],
                    in1=arr[:, k, :, :].rearrange("p j t -> p (j t)"))
        units.append(u_add)

        def u_stats():
            xmid = state[f"xmid{lc}"]
            state[f"ab{lc}"] = _ln_stats_pe(nc, de, pps, "ps_proj",
                                            xmid, ones_col)
        units.append(u_stats)

        def mk_apply(k0):
            def u_apply():
                xmid = state[f"xmid{lc}"]
                Ab2, Bb2 = state[f"ab{lc}"]
                if f"h2{lc}" not in state:
                    state[f"h2{lc}"] = de.tile([128, KD, 512], BF,
                                               tag="h2", bufs=1, name="h2")
                h2 = state[f"h2{lc}"]
                for k in range(k0, k0 + 4):
                    _ln_apply(nc, de, xmid[:, k, :], Ab2, Bb2, h2[:, k, :])
            return u_apply
        units.append(mk_apply(0))
        units.append(mk_apply(4))
        return units

    def fc1_units(lc):
        """FFN up: u = relu(h2 @ W1 + b1f); one unit per 256-wide block."""
        de = state["de"]
        units = []

        def mk(q16):
            def u():
                h2 = state[f"h2{lc}"]
                if f"u{lc}" not in state:
                    state[f"u{lc}"] = de.tile([128, KFF, 512], BF,
                                              tag="u", bufs=1, name="u")
                uu = state[f"u{lc}"]
                w1t = de.tile([128, KD, 256], BF, tag="w1t", bufs=2,
                              name="w1t")
                nc.sync.dma_start(
                    out=w1t, in_=t["w1"][q16 * 128:(q16 + 1) * 128, :, :])
                for fi in range(2):
                    fft = q16 * 2 + fi
                    ps = pps.tile([128, 512], F32, tag="ps_proj", bufs=2,
                                  name="ps_u")
                    for k in range(KD):
                        mm(out=ps,
                           lhsT=w1t[:, k, fi * 128:fi * 128 + 128],
                           rhs=h2[:, k, :],
                           start=(k == 0), stop=(k == KD - 1))
                    if lc == 0:
                        nc.vector.tensor_scalar(
                            out=uu[:, fft, :], in0=ps,
                            scalar1=b1_sb[:, fft:fft + 1], scalar2=0.0,
                            op0=Alu.add, op1=Alu.max)
                    else:
                        nc.scalar.activation(
                            out=uu[:, fft, :], in_=ps, func=Act.Relu,
                            bias=b1_sb[:, fft:fft + 1])
            units.append(u)
        for q16 in range(16):
            mk(q16)
        return units

    def fc2_units(lc):
        """FFN down + bias + residual -> store; W2 streamed per dt."""
        de = state["de"]
        c0 = lc * 512
        units = []

        def mk(dt):
            def u():
                uu = state[f"u{lc}"]
                xmid = state[f"xmid{lc}"]
                w2t = state.pop(f"w2t{dt}", None) if lc == 0 else None
                if w2t is None:
                    w2t = de.tile([128, KFF, 128], BF, tag="w2t", bufs=2,
                                  name="w2t")
                    nc.scalar.dma_start(
                        out=w2t,
                        in_=t["w2"][dt * 128:(dt + 1) * 128, :, :])
                ps = pps.tile([128, 512], F32, tag="ps_proj", bufs=2,
                              name="ps_f")
                for k2 in range(KFF):
                    mm(out=ps, lhsT=w2t[:, k2, :], rhs=uu[:, k2, :],
                       start=(k2 == 0), stop=(k2 == KFF - 1))
                o_f = de.tile([128, 512], F32, tag="o_f", bufs=2,
                              name="o_f")
                nc.vector.scalar_tensor_tensor(
                    out=o_f, in0=ps, scalar=b2_sb[:, dt:dt + 1],
                    in1=xmid[:, dt, :], op0=Alu.add, op1=Alu.add)
                nc.sync.dma_start(
                    out=outT_v[dt * 128:(dt + 1) * 128, c0:c0 + 512],
                    in_=o_f)
            units.append(u)
        for dt in range(KD):
            mk(dt)
        return units

    # ================= emission schedule =================
    xf0 = load_x(0)
    load_qkv_weights()
    xf1 = load_x(1)
    # vS zero + softmax-denominator ones columns (needed only by the
    # v-projection epilogues; emitted off the ln1(0) critical path)
    nc.gpsimd.memset(vS[:, 8:16, :], 0.0)

    def w2_prefetch(dt):
        def u():
            de = state["de"]
            w2t = de.tile([128, KFF, 128], BF, tag="w2t", bufs=2,
                          name="w2t")
            nc.scalar.dma_start(
                out=w2t, in_=t["w2"][dt * 128:(dt + 1) * 128, :, :])
            state[f"w2t{dt}"] = w2t
        return u

    hT0 = ln1(0, xf0)
    nc.gpsimd.memset(vS[:, 0:8, :], 0.0)
    for h in range(LH):
        nc.gpsimd.memset(vS[:, :, h * 128 + 64:h * 128 + 65], 1.0)
    for u in proj_units(0, hT0):
        u()
    hT1 = ln1(1, xf1)

    oTs = {}
    for ci in range(NCH):
        oTs[ci] = None

    def new_oT():
        return ab.tile([128, KHE, 512], F8, tag="oT", bufs=2, name="oT")

    oTs[0] = new_oT()
    attention(0, oTs[0], list(proj_units(1, hT1)))

    xf2 = load_x(2)
    hT2 = ln1(2, xf2)
    xf3 = load_x(3)
    hT3 = ln1(3, xf3)

    oTs[1] = new_oT()
    attention(1, oTs[1],
              wo_units(0, oTs[0])
              + proj_units(2, hT2) + proj_units(3, hT3))
    # collectives issued between attention chunks: the issuing gpsimd
    # queue blocks until the collective completes, and the next chunk
    # needs no gpsimd work for its first ~20us
    rs_unit(0)()
    pe_pool.release()
    state["de"] = tc.alloc_tile_pool(name="de", bufs=1)

    oTs[3] = new_oT()
    attention(3, oTs[3], wo_units(1, oTs[1]))
    rs_unit(1)()

    f10 = fc1_units(0)
    oTs[2] = new_oT()
    attention(2, oTs[2],
              wo_units(3, oTs[3])
              + prep_units(0) + [w2_prefetch(0), w2_prefetch(1)]
              + f10[:4])

    rs_unit(3)()
    for u in wo_units(2, oTs[2]):
        u()
    rs_unit(2)()

    # bulk of lc0 FFN here so the tail ReduceScatters hide behind it
    for u in f10[4:]:
        u()
    p1 = prep_units(1)
    f20 = fc2_units(0)
    inter = []
    while p1 or f20:
        if f20:
            inter.append(f20.pop(0))
        if p1:
            inter.append(p1.pop(0))
    for u in inter:
        u()
    for u in fc1_units(1):
        u()
    for u in fc2_units(1):
        u()

    state["de"].release()
    ab.release()
    pps.release()
    wlate.release()
    consts.release()
    dram.release()


def _build():
    nc = bacc.Bacc("TRN2", target_bir_lowering=False, debug=False,
                   num_devices=NCORES)

    tensors = {}
    tensors["xT"] = nc.dram_tensor("xT", [NCH * 128, KD, 512], BF,
                                   kind="ExternalInput").ap()
    tensors["xresT"] = nc.dram_tensor("xresT", [TP * 128, KD, 512], BF,
                                      kind="ExternalInput").ap()
    for name, shape, dt in (
        ("wq", [128, KD, 512], F8), ("wk", [128, KD, 512], F8),
        ("wv", [128, KD, 512], F8), ("wo", [128, KHE, D], F8),
        ("w1", [16 * 128, KD, 256], BF), ("w2", [8 * 128, KFF, 128], BF),
        ("bq", [128, KHE], F32), ("bk", [128, KHE], F32), ("bv", [LHE], F32),
        ("b1f", [128, KFF], F32), ("bo2", [128, KD], F32),
        ("b2", [128, KD], F32),
    ):
        tensors[name] = nc.dram_tensor(name, shape, dt,
                                       kind="ExternalInput").ap()
    tensors["outT"] = nc.dram_tensor("out", [D, LT], F32,
                                     kind="ExternalOutput").ap()

    with tile.TileContext(nc, num_cores=NCORES) as tc:
        _emit(nc, tc, tensors)

    nc.compile()
    return nc


_NC_CACHE = None


def _get_nc():
    global _NC_CACHE
    if _NC_CACHE is None:
        _NC_CACHE = _build()
    return _NC_CACHE


def _shard_inputs(x, Wq, Wk, Wv, Wo, bo, W1, b1, W2, b2, g1, be1, g2, be2):
    """Build the 8 per-core input maps (LN gains folded into weights)."""
    bf = lambda a: np.ascontiguousarray(a).astype(BF16NP)
    f8 = lambda a: np.ascontiguousarray(a).astype(FP8NP)
    f32 = lambda a: np.ascontiguousarray(a, dtype=np.float32)

    x = np.asarray(x, dtype=np.float32)
    Wq = np.asarray(Wq, dtype=np.float32)
    Wk = np.asarray(Wk, dtype=np.float32)
    Wv = np.asarray(Wv, dtype=np.float32)
    Wo = np.asarray(Wo, dtype=np.float32)
    W1 = np.asarray(W1, dtype=np.float32)
    W2 = np.asarray(W2, dtype=np.float32)
    g1 = np.asarray(g1, dtype=np.float32)
    be1 = np.asarray(be1, dtype=np.float32)
    g2 = np.asarray(g2, dtype=np.float32)
    be2 = np.asarray(be2, dtype=np.float32)
    b1 = np.asarray(b1, dtype=np.float32)

    scale = float(HS) ** -0.5
    # fold g1 into QKV weights, be1 into QKV biases; fold the score scale
    # into Wq/bq.  Per-head [H, D, HS] -> concat heads -> [D, H*HS].
    wq_f = (g1[None, :, None] * Wq).transpose(1, 0, 2).reshape(D, D) * scale
    wk_f = (g1[None, :, None] * Wk).transpose(1, 0, 2).reshape(D, D)
    wv_f = (g1[None, :, None] * Wv).transpose(1, 0, 2).reshape(D, D)
    bq_f = np.einsum("d,hde->he", be1, Wq).reshape(D) * scale
    bk_f = np.einsum("d,hde->he", be1, Wk).reshape(D)
    bv_f = np.einsum("d,hde->he", be1, Wv).reshape(D)
    # fold g2/be2 into W1/b1
    w1_f = g2[:, None] * W1
    b1_f = b1 + be2 @ W1

    in_maps = []
    for c in range(NCORES):
        b, half = divmod(c, TP)
        hes = slice(half * LHE, (half + 1) * LHE)
        xt = x[b].T
        xres = np.concatenate(
            [xt[:, ci * 512 + half * 256: ci * 512 + half * 256 + 256]
             for ci in range(NCH)], axis=1)
        # partition-major tiled layouts: loads become 128 contiguous
        # segments instead of 1024 scattered ones (descriptor-gen bound)
        xt_sw = xt.reshape(KD, 128, NCH, 512).transpose(2, 1, 0, 3)
        xres_sw = xres.reshape(KD, 128, TP, 512).transpose(2, 1, 0, 3)
        wq_sw = wq_f[:, hes].reshape(KD, 128, LHE).transpose(1, 0, 2)
        wk_sw = wk_f[:, hes].reshape(KD, 128, LHE).transpose(1, 0, 2)
        wv_sw = wv_f[:, hes].reshape(KD, 128, LHE).transpose(1, 0, 2)
        wo_sw = Wo[hes, :].reshape(KHE, 128, D).transpose(1, 0, 2)
        w1_sw = w1_f.reshape(KD, 128, 16, 256).transpose(2, 1, 0, 3)
        # W2 tiled dt-major so fc2 streams contiguous [128, KFF, 128] tiles
        w2_sw = W2.reshape(KFF, 128, KD, 128).transpose(2, 1, 0, 3)
        in_maps.append({
            "xT": bf(xt_sw.reshape(NCH * 128, KD, 512)),
            "xresT": bf(xres_sw.reshape(TP * 128, KD, 512)),
            "wq": f8(wq_sw * SQ), "wk": f8(wk_sw * SK), "wv": f8(wv_sw * SV),
            "bq": f32((bq_f[hes] * SQ).reshape(KHE, 128).T),
            "bk": f32((bk_f[hes] * SK).reshape(KHE, 128).T),
            "bv": f32(bv_f[hes]),
            "wo": f8(wo_sw * SO),
            "bo2": f32((np.asarray(bo, dtype=np.float32) * SO / TP)
                       .reshape(KD, 128).T),
            "w1": bf(w1_sw.reshape(16 * 128, KD, 256)),
            "b1f": f32(b1_f.reshape(KFF, 128).T),
            "w2": bf(w2_sw.reshape(8 * 128, KFF, 128)),
            "b2": f32(np.asarray(b2, dtype=np.float32)
                      .reshape(KD, 128).T),
        })
    return in_maps


def kernel(x, Wq, Wk, Wv, Wo, bo, W1, b1, W2, b2, g1, be1, g2, be2,
           _trace=False):
    nc = _get_nc()
    in_maps = _shard_inputs(x, Wq, Wk, Wv, Wo, bo, W1, b1, W2, b2,
                            g1, be1, g2, be2)
    res = run_bass_kernel_spmd(nc, in_maps, list(range(NCORES)),
                               trace=_trace)
    out = np.empty((B, T, D), dtype=np.float32)
    for b in range(B):
        for half in range(TP):
            o = res.results[TP * b + half]["out"]  # [D, LT]
            for ci in range(NCH):
                t0 = ci * 512 + half * 256
                out[b, t0:t0 + 256, :] = o[:, ci * 256:(ci + 1) * 256].T
    if _trace:
        kernel.last_exec_time_ns = res.exec_time_ns
        kernel.last_results = res
    return out
